# revision 1
# baseline (speedup 1.0000x reference)
"""Bass/Trainium2 kernel for chunked local attention with memory tokens
(BertSelfAttention variant). Self-contained: hardcodes all shapes.

Sharding: 8 cores, each handles 4 of the 32 (batch, chunk) pairs.
  core i -> b = i // 4, chunks 4*(i % 4) .. 4*(i % 4) + 3
No collectives; weights replicated per core; host scatters/gathers.

Per-core device computation (PE operands fp16, accumulation fp32):
  - xT [1024(8x128), 256] loaded directly via whole-chunk DMA-transpose
    of host-cast fp16 x (no PE transposes)
  - qT[j,t] = x@(Wq/8) + bq/8 (feature-major)   kT[j,t] = x@Wk + bk
  - v[t,j] = x@Wv + bv, token-major, interleaved [t, h, 66] with a ones
    column at 64 that makes the PV matmul emit softmax denominators
  - scoresT[y,x] = sum_hd kT[hd,y] qT[hd,x]; adjacent head-pairs share a
    [128,512] psum (same row group -> serial, bank-safe); the even/odd
    head of a feature tile run concurrently via row-tiled matmuls on
    separate banks
  - probs = Exp(scoresT + mask[y]) in fp16; the additive key mask rides
    the ACT bias operand (per-partition scalar)
  - out_unnorm[x, 4*(hd|den)] accumulated per head-pair in one psum
    bank; one strided reciprocal normalizes 4 blocks, DVE muls write the
    token-major output
Memory tokens (16, shared per batch) are projected once per core, inside
chunk 0 so they don't gate on the upfront weight DMA. Weight tiles are
striped across both HWDGE rings.
"""

import sys

sys.path.insert(0, "/opt/trn_rl_repo")

import numpy as np

import concourse.tile as tile
from concourse import bacc, mybir
from concourse.bass_utils import run_bass_kernel_spmd

F32 = mybir.dt.float32
F16 = mybir.dt.float16

B, S, D = 2, 4096, 1024
H, HD = 16, 64
W = 256            # attention window (chunk length)
C = S // W         # 16 chunks
M = 16             # memory tokens
N_CORES = 8
CPC = C * B // N_CORES  # 4 chunks per core
TPC = CPC * W           # 1024 chunk tokens per core
NJT = D // 128          # 8 feature tiles
NHP = H // 2            # 8 head pairs
VW = HD + 2             # v interleave width (64 hd + ones + pad)

TRACE = False
LAST_RESULTS = None


def _build_kernel():
    nc = bacc.Bacc(None, target_bir_lowering=False)

    x_d = nc.declare_dram_parameter("x", [TPC, D], F16, isOutput=False)
    mem_d = nc.declare_dram_parameter("mem", [128, NJT * M], F16, isOutput=False)
    wqT_d = nc.declare_dram_parameter("wqT", [D, D], F16, isOutput=False)
    wkT_d = nc.declare_dram_parameter("wkT", [D, D], F16, isOutput=False)
    wvT_d = nc.declare_dram_parameter("wvT", [D, D], F16, isOutput=False)
    bq_d = nc.declare_dram_parameter("bqv", [128, NJT], F32, isOutput=False)
    bk_d = nc.declare_dram_parameter("bkv", [128, NJT], F32, isOutput=False)
    bv_d = nc.declare_dram_parameter("bvrow", [1, D], F32, isOutput=False)
    msk_d = nc.declare_dram_parameter("maskvT", [128, CPC * 3], F32, isOutput=False)
    out_d = nc.declare_dram_parameter("out", [TPC, D], F32, isOutput=True)

    with tile.TileContext(nc) as tc:
        with (
            tc.tile_pool(name="const", bufs=1) as cpool,
            tc.tile_pool(name="wpool", bufs=1) as wpool,
            tc.tile_pool(name="xtpool", bufs=4) as xtpool,
            tc.tile_pool(name="qkpool", bufs=8) as qkpool,
            tc.tile_pool(name="vpool", bufs=3) as vpool,
            tc.tile_pool(name="epool", bufs=14) as epool,
            tc.tile_pool(name="spool", bufs=4) as spool,
            tc.tile_pool(name="opool", bufs=3) as opool,
            tc.tile_pool(name="pp", bufs=2, space="PSUM") as pp_pool,
            tc.tile_pool(name="ps", bufs=4, space="PSUM") as ps_pool,
            tc.tile_pool(name="po", bufs=2, space="PSUM") as po_pool,
        ):
            # ---- constants ----
            # ---- x prefetch + weights, interleaved in HWDGE FIFO order:
            # x(chunk0), mem, wq, wk, x(chunk1), wv ----
            x_tiles = {}

            def load_x(ci):
                x_t = xtpool.tile([128, NJT, W], F16, tag="xT", name="xT")
                nc.sync.dma_start_transpose(
                    x_t[:], x_d[ci * W:(ci + 1) * W, :]
                )
                x_tiles[ci] = [x_t[:, d, :] for d in range(NJT)]

            w_all = wpool.tile([128, 3 * NJT, D], F16, tag="w_all")

            def load_w(wi, wd):
                # stripe across both HWDGE rings
                engs = (nc.sync, nc.scalar)
                for o in range(NJT):
                    engs[o % 2].dma_start(
                        w_all[:, wi * NJT + o, :], wd[o * 128:(o + 1) * 128, :]
                    )



            def wq(d):
                return w_all[:, d, :]

            def wk(d):
                return w_all[:, NJT + d, :]

            def wv(d):
                return w_all[:, 2 * NJT + d, :]

            # ---- memory tokens: transpose once per core ----
            load_x(0)
            # memory tokens arrive pre-transposed from host: [128, 8, 16]
            xTm = cpool.tile([128, NJT, M], F16, tag="xTm")
            nc.sync.dma_start(
                xTm[:], mem_d.rearrange("p (o m) -> p o m", m=M)
            )
            load_w(0, wqT_d)
            bqv = cpool.tile([128, NJT], F32, tag="bqv")
            nc.sync.dma_start(bqv[:], bq_d[:])
            bkv = cpool.tile([128, NJT], F32, tag="bkv")
            nc.sync.dma_start(bkv[:], bk_d[:])
            mskv = cpool.tile([128, CPC * 3], F32, tag="mskv")
            nc.sync.dma_start(mskv[:], msk_d[:])

            # bv broadcast to all partitions via ones-matmul
            bvrow = cpool.tile([1, D], F32, tag="bvrow")
            nc.sync.dma_start(bvrow[:], bv_d[:])
            ones1 = cpool.tile([1, 128], F32, tag="ones1")
            nc.vector.memset(ones1[:], 1.0)
            onesc = cpool.tile([128, 2 * H], F32, tag="onesc")
            nc.vector.memset(onesc[:], 1.0)
            bvb = cpool.tile([128, D], F32, tag="bvb")
            for half in range(2):
                ps_b = pp_pool.tile([128, 512], F32, tag="pp")
                nc.tensor.matmul(
                    ps_b[:], ones1[:], bvrow[:, half * 512:(half + 1) * 512],
                    start=True, stop=True,
                )
                nc.vector.tensor_copy(bvb[:, half * 512:(half + 1) * 512], ps_b[:])
            load_w(1, wkT_d)
            load_x(1)
            load_w(2, wvT_d)
            # kT_mem / v_mem are emitted inside chunk 0 (after its K / V
            # sections) so they do not gate on the full upfront weight DMA.
            memp = {}

            def emit_ktm():
                kTm = cpool.tile([128, NJT, M], F16, tag="kTm", name="kTm")
                for jt in range(NJT):
                    ps_k = ps_pool.tile([128, 512], F32, tag="ps", name="ps_ktm")
                    for d in range(NJT):
                        nc.tensor.matmul(
                            ps_k[:, :M], wk(d)[:, jt * 128:(jt + 1) * 128],
                            xTm[:, d, :], start=(d == 0), stop=(d == NJT - 1),
                        )
                    nc.vector.tensor_scalar_add(
                        kTm[:, jt, :], ps_k[:, :M], bkv[:, jt:jt + 1]
                    )
                memp["kTm"] = kTm

            def emit_vm():
                vm = cpool.tile([M, H, VW], F16, tag="vm", name="vm")
                nc.vector.tensor_copy(
                    vm[:, :, HD:HD + 2],
                    onesc[:M, :].rearrange("p (h o) -> p h o", h=H),
                )
                for half in range(2):
                    ps_v = pp_pool.tile([128, 512], F32, tag="pp", name="ps_vm")
                    for d in range(NJT):
                        nc.tensor.matmul(
                            ps_v[:M, :], xTm[:, d, :],
                            wv(d)[:, half * 512:(half + 1) * 512],
                            start=(d == 0), stop=(d == NJT - 1),
                        )
                    nc.vector.tensor_tensor(
                        vm[:M, half * 8:(half + 1) * 8, :HD],
                        ps_v[:M, :].rearrange("p (h f) -> p h f", h=8),
                        bvb[:M, half * 512:(half + 1) * 512].rearrange(
                            "p (h f) -> p h f", h=8
                        ),
                        mybir.AluOpType.add,
                    )
                memp["vm"] = vm

            # ---- per chunk ----
            for ci in range(CPC):
                # chunk tokens arrive pre-transposed via DMA-transpose
                if ci not in x_tiles:
                    load_x(ci)
                xT = x_tiles.pop(ci)

                for nxt in (ci + 1, ci + 2):
                    if nxt < CPC and nxt not in x_tiles:
                        load_x(nxt)

                # Q/K projections (feature-major); jt pairs share one
                # [128,512] psum group (serial same-row-group writes), one
                # broadcast-bias epilogue per pair
                qT, kT = [], []
                for which, wfn, bias, lst, tg in (
                    (0, wq, bqv, qT, "qT"),
                    (1, wk, bkv, kT, "kT"),
                ):
                    for jp in range(NJT // 2):
                        ps_q = ps_pool.tile(
                            [128, 512], F32, tag="ps", name="ps_q"
                        )
                        for u in range(2):
                            jt = 2 * jp + u
                            for d in range(NJT):
                                nc.tensor.matmul(
                                    ps_q[:, u * 256:(u + 1) * 256],
                                    wfn(d)[:, jt * 128:(jt + 1) * 128],
                                    xT[d][:],
                                    start=(u == 0 and d == 0),
                                    stop=(u == 1 and d == NJT - 1),
                                )
                        pair_t = qkpool.tile(
                            [128, 2, W], F16, tag=tg, name=f"pair_{tg}"
                        )
                        nc.vector.tensor_tensor(
                            pair_t[:],
                            ps_q[:].rearrange("p (u t) -> p u t", u=2),
                            bias[:, 2 * jp:2 * jp + 2][:, :, None].to_broadcast(
                                (128, 2, W)
                            ),
                            mybir.AluOpType.add,
                        )
                        lst.append(pair_t)
                qT = [qT[hp // 2][:, hp % 2, :] for hp in range(NJT)]
                kT = [kT[hp // 2][:, hp % 2, :] for hp in range(NJT)]
                if ci == 0:
                    emit_ktm()

                # V projection (token-major fp16, interleaved heads + ones)
                v_sb = []
                for tt in range(2):
                    v_t = vpool.tile([128, H, VW], F16, tag="v_sb")
                    nc.vector.tensor_copy(
                        v_t[:, :, HD:HD + 2],
                        onesc.rearrange("p (h o) -> p h o", h=H),
                    )
                    for half in range(2):
                        ps_v = pp_pool.tile([128, 512], F32, tag="pp")
                        for d in range(NJT):
                            nc.tensor.matmul(
                                ps_v[:], xT[d][:, tt * 128:(tt + 1) * 128],
                                wv(d)[:, half * 512:(half + 1) * 512],
                                start=(d == 0), stop=(d == NJT - 1),
                            )
                        nc.vector.tensor_tensor(
                            v_t[:, half * 8:(half + 1) * 8, :HD],
                            ps_v[:].rearrange("p (h f) -> p h f", h=8),
                            bvb[:, half * 512:(half + 1) * 512].rearrange(
                                "p (h f) -> p h f", h=8
                            ),
                            mybir.AluOpType.add,
                        )
                    v_sb.append(v_t)
                if ci == 0:
                    emit_vm()

                out_sb = [
                    opool.tile([128, D], F32, tag="out_sb", name=f"out_sb{i}")
                    for i in range(2)
                ]

                # attention. Head pairs (A=even, B=odd head of a feature
                # tile). Local scores: adjacent head-PAIRS (hpp = hp//2
                # grouping hp 2hpp, 2hpp+1) share one [128,512] psum per
                # (ab, yt) -- same row group => serial writes, bank-safe;
                # A vs B still run concurrently on different banks.
                # Memory scores: all hp packed at 32-partition strides,
                # 2 psums per ab, exp'd in 2 ops.
                kTm, vm = memp["kTm"], memp["vm"]
                for gi in range(2):
                    eloc = {}   # (hpq, ab, yt) -> [128,512] fp16 (hp pair)
                    emem = {}   # ab -> [128,256] fp16 (4 hp at 32-stride)
                    for hpq in (2 * gi, 2 * gi + 1):
                        for ab in range(2):
                            p0 = 64 * ab
                            ps_m = ps_pool.tile(
                                [128, 512], F32, tag="ps", name="ps_m"
                            )
                            for u in range(2):
                                hp = 2 * hpq + u
                                nc.tensor.matmul(
                                    ps_m[:M, u * 256:(u + 1) * 256],
                                    kTm[p0:p0 + 64, hp, :],
                                    qT[hp][p0:p0 + 64, :],
                                    start=(u == 0), stop=(u == 1),
                                    tile_position=(p0, 0),
                                )
                            e_t = epool.tile(
                                [128, 2 * W], F16, tag="exps", name="e_m"
                            )
                            nc.scalar.activation(
                                e_t[:M, :], ps_m[:M, :],
                                mybir.ActivationFunctionType.Exp,
                                bias=mskv[:M, ci * 3 + 2: ci * 3 + 3],
                            )
                            emem[(hpq, ab)] = e_t
                    for hpq in (2 * gi, 2 * gi + 1):
                        for yt in range(2):
                            for ab in range(2):
                                p0 = 64 * ab
                                ps_s = ps_pool.tile(
                                    [128, 512], F32, tag="ps", name="ps_s"
                                )
                                for u in range(2):
                                    hp = 2 * hpq + u
                                    nc.tensor.matmul(
                                        ps_s[:, u * 256:(u + 1) * 256],
                                        kT[hp][p0:p0 + 64, yt * 128:(yt + 1) * 128],
                                        qT[hp][p0:p0 + 64, :],
                                        start=(u == 0), stop=(u == 1),
                                        tile_position=(p0, 0),
                                    )
                                e_t = epool.tile([128, 2 * W], F16, tag="exps")
                                nc.scalar.activation(
                                    e_t[:], ps_s[:],
                                    mybir.ActivationFunctionType.Exp,
                                    bias=mskv[:, ci * 3 + yt: ci * 3 + yt + 1],
                                )
                                eloc[(hpq, ab, yt)] = e_t

                    for u4 in range(4):
                        hp = 4 * gi + u4
                        u = hp % 2
                        # all 4 (ab, xb) output blocks of this head pair in
                        # one psum bank; one strided reciprocal for the 4
                        # softmax denominators
                        ps_o = po_pool.tile([128, 4 * VW], F32, tag="po")
                        for ab in range(2):
                            h = 2 * hp + ab
                            e0 = eloc[(hp // 2, ab, 0)]
                            e1 = eloc[(hp // 2, ab, 1)]
                            em = emem[(hp // 2, ab)]
                            for xb in range(2):
                                o = (ab * 2 + xb) * VW
                                xs = u * 256 + xb * 128
                                nc.tensor.matmul(
                                    ps_o[:, o:o + VW],
                                    e0[:, xs:xs + 128], v_sb[0][:, h, :],
                                    start=True, stop=False,
                                )
                                nc.tensor.matmul(
                                    ps_o[:, o:o + VW],
                                    e1[:, xs:xs + 128], v_sb[1][:, h, :],
                                    start=False, stop=False,
                                )
                                nc.tensor.matmul(
                                    ps_o[:, o:o + VW],
                                    em[:M, xs:xs + 128], vm[:, h, :],
                                    start=False, stop=True,
                                )
                        rec = spool.tile([128, 4], F32, tag="rec")
                        nc.vector.reciprocal(
                            rec[:].rearrange("p (k o) -> p k o", o=1),
                            ps_o[:].rearrange("p (k w) -> p k w", w=VW)[
                                :, :, HD:HD + 1
                            ],
                        )
                        for ab in range(2):
                            h = 2 * hp + ab
                            for xb in range(2):
                                k = ab * 2 + xb
                                nc.vector.tensor_scalar_mul(
                                    out_sb[xb][:, h * HD:(h + 1) * HD],
                                    ps_o[:, k * VW: k * VW + HD],
                                    rec[:, k:k + 1],
                                )

                for xb in range(2):
                    nc.sync.dma_start(
                        out_d[ci * W + xb * 128: ci * W + (xb + 1) * 128, :],
                        out_sb[xb][:],
                    )

    nc.compile()
    return nc


_NC_CACHE = None


def kernel(hidden_states, attention_mask, self_memory, Wq, bq, Wk, bk, Wv, bv):
    global _NC_CACHE, LAST_RESULTS
    hidden_states = np.ascontiguousarray(np.asarray(hidden_states, np.float32))
    attention_mask = np.asarray(attention_mask, np.float32)
    self_memory = np.ascontiguousarray(np.asarray(self_memory, np.float32))
    wqT = np.ascontiguousarray((np.asarray(Wq, np.float32).T * 0.125).astype(np.float16))
    wkT = np.ascontiguousarray(np.asarray(Wk, np.float32).T.astype(np.float16))
    wvT = np.ascontiguousarray(np.asarray(Wv, np.float32).T.astype(np.float16))
    bqv = np.ascontiguousarray(
        np.asarray(bq, np.float32).reshape(NJT, 128).T * 0.125
    )
    bkv = np.ascontiguousarray(np.asarray(bk, np.float32).reshape(NJT, 128).T)
    bvrow = np.asarray(bv, np.float32).reshape(1, D)

    # additive mask along the key axis, per (b, c): [W local | M memory]
    am = attention_mask.reshape(B, C, W)
    chunk_has_valid = (am == 0.0).sum(axis=2) > 0
    mem_mask = np.where(chunk_has_valid, 0.0, -10000.0).astype(np.float32)

    if _NC_CACHE is None:
        _NC_CACHE = _build_kernel()
    nc = _NC_CACHE

    in_maps = []
    for core in range(N_CORES):
        b = core // (N_CORES // B)
        c0 = (core % (N_CORES // B)) * CPC
        mvT = np.zeros((128, CPC * 3), np.float32)
        for ci in range(CPC):
            mvT[:, ci * 3 + 0] = am[b, c0 + ci, 0:128]
            mvT[:, ci * 3 + 1] = am[b, c0 + ci, 128:256]
            mvT[:, ci * 3 + 2] = mem_mask[b, c0 + ci]
        in_maps.append(
            {
                "x": hidden_states[b, c0 * W:(c0 + CPC) * W, :].astype(
                    np.float16
                ),
                "mem": np.ascontiguousarray(
                    self_memory[b].T.astype(np.float16)
                    .reshape(NJT, 128, M).transpose(1, 0, 2)
                ).reshape(128, NJT * M),
                "wqT": wqT,
                "wkT": wkT,
                "wvT": wvT,
                "bqv": bqv,
                "bkv": bkv,
                "bvrow": bvrow,
                "maskvT": mvT,
            }
        )

    res = run_bass_kernel_spmd(nc, in_maps, list(range(N_CORES)), trace=TRACE)
    LAST_RESULTS = res

    out = np.empty((B, S, D), np.float32)
    for core in range(N_CORES):
        b = core // (N_CORES // B)
        c0 = (core % (N_CORES // B)) * CPC
        out[b, c0 * W:(c0 + CPC) * W, :] = res.results[core]["out"]
    return out



# revision 49
# speedup vs baseline: 1.1407x; 1.1407x over previous
"""Bass/Trainium2 kernel for chunked local attention with memory tokens
(BertSelfAttention variant). Self-contained: hardcodes all shapes.

Sharding: 8 cores, each handles 4 of the 32 (batch, chunk) pairs.
  core i -> b = i // 4, chunks 4*(i % 4) .. 4*(i % 4) + 3
No collectives; weights replicated per core; host scatters/gathers.

Per-core device computation (PE operands fp16, accumulation fp32):
  - xT [128, 8, 1024] arrives pre-transposed from host (feature-major),
    sliced per chunk; no device transposes
  - qT[jt] = x@(Wq/8) + bq/8 (feature-major)   kT[jt] = x@Wk + bk
  - v[t, yt, h, 65] token-major with a ones column at 64 that makes the
    PV matmul emit softmax denominators
  - memory-token K is materialized block-diagonally (kTm3 [128, jt, 64]:
    rows 0:64 x cols 0:16 = even head, rows 64:128 x cols 32:48 = odd) so
    one 128-contraction matmul scores 2 heads' mem keys at 32-aligned
    psum partitions; all 16 heads' mem scores share one 2-bank psum and
    a single Exp
  - local scoresT per (head-quad, yt) fill a 2-bank [128, 1024] psum
    (both 64-row ab halves, quadrant-packed) -> one Exp per fill
  - probs = Exp(scoresT + mask[y]) in fp16; additive key mask rides the
    ACT bias operand (per-partition scalar)
  - out_unnorm[x, 4*(hd|den)] per head-pair in one psum bank; strided
    reciprocal + one broadcast multiply normalize and emit fp16 out
Output fp16 on device, upcast to fp32 on host.
"""

import sys

sys.path.insert(0, "/opt/trn_rl_repo")

import numpy as np

import concourse.tile as tile
from concourse import bacc, mybir
from concourse.bass_utils import run_bass_kernel_spmd

F32 = mybir.dt.float32
F16 = mybir.dt.float16

B, S, D = 2, 4096, 1024
H, HD = 16, 64
W = 256            # attention window (chunk length)
C = S // W         # 16 chunks
M = 16             # memory tokens
N_CORES = 8
CPC = C * B // N_CORES  # 4 chunks per core
TPC = CPC * W           # 1024 chunk tokens per core
NJT = D // 128          # 8 feature tiles
VW = HD + 1             # v width (64 hd + ones col)

TRACE = False
LAST_RESULTS = None


def _mem_slot(h):
    """(partition base, free block) of head h's mem scores / probs."""
    jt, u = h // 2, h % 2
    return 64 * (jt % 2) + 32 * u, jt // 2


def _build_kernel():
    nc = bacc.Bacc(None, target_bir_lowering=False)

    xT_d = nc.declare_dram_parameter("xT", [128, NJT * TPC], F16, isOutput=False)
    memT_d = nc.declare_dram_parameter("memT", [128, NJT * M], F16, isOutput=False)
    wqT_d = nc.declare_dram_parameter("wqT", [D, D], F16, isOutput=False)
    wkT_d = nc.declare_dram_parameter("wkT", [D, D], F16, isOutput=False)
    wvT_d = nc.declare_dram_parameter("wvT", [D, D], F16, isOutput=False)
    bq_d = nc.declare_dram_parameter("bqv", [128, NJT], F32, isOutput=False)
    bk_d = nc.declare_dram_parameter("bkv", [128, NJT], F32, isOutput=False)
    bv_d = nc.declare_dram_parameter("bvrow", [1, D], F16, isOutput=False)
    msk_d = nc.declare_dram_parameter("maskvT", [128, CPC * 3], F32, isOutput=False)
    out_d = nc.declare_dram_parameter("out", [TPC, D], F16, isOutput=True)

    with tile.TileContext(nc) as tc:
        with (
            tc.tile_pool(name="const", bufs=1) as cpool,
            tc.tile_pool(name="wpool", bufs=1) as wpool,
            tc.tile_pool(name="xtpool", bufs=4) as xtpool,
            tc.tile_pool(name="qkpool", bufs=12) as qkpool,
            tc.tile_pool(name="vpool", bufs=3) as vpool,
            tc.tile_pool(name="epool", bufs=12) as epool,
            tc.tile_pool(name="empool", bufs=2) as empool,
            tc.tile_pool(name="opool", bufs=3) as opool,
            tc.tile_pool(name="pp", bufs=2, space="PSUM") as pp_pool,
            tc.tile_pool(name="ps", bufs=2, space="PSUM") as ps_pool,
            tc.tile_pool(name="po", bufs=2, space="PSUM") as po_pool,
        ):
            x_tiles = {}

            def load_x(ci, eng):
                x_t = xtpool.tile([128, NJT, W], F16, tag="xT", name="xT")
                eng.dma_start(
                    x_t[:],
                    xT_d.rearrange("p (o t) -> p o t", t=TPC)[
                        :, :, ci * W:(ci + 1) * W
                    ],
                )
                x_tiles[ci] = [x_t[:, d, :] for d in range(NJT)]

            w_all = wpool.tile([128, 3 * NJT, D], F16, tag="w_all")

            def load_w(wi, wd):
                # 2 DMAs of 4 row-blocks (1 MB) each
                for o in range(0, NJT, 4):
                    nc.sync.dma_start(
                        w_all[:, wi * NJT + o: wi * NJT + o + 4, :],
                        wd.rearrange("(o p) c -> p o c", p=128)[:, o:o + 4, :],
                    )

            def wq(d):
                return w_all[:, d, :]

            def wk(d):
                return w_all[:, NJT + d, :]

            def wv(d):
                return w_all[:, 2 * NJT + d, :]

            # ---- upfront DMAs: one queue, in consumption order; weights
            # in 1MB halves ordered by when the PE stream consumes them.
            load_x(0, nc.sync)
            bvrow = cpool.tile([1, D], F16, tag="bvrow")
            nc.sync.dma_start(bvrow[:], bv_d[:])
            xTm = cpool.tile([128, NJT, M], F16, tag="xTm")
            nc.sync.dma_start(xTm[:], memT_d.rearrange("p (o m) -> p o m", m=M))
            bqv = cpool.tile([128, NJT], F32, tag="bqv")
            nc.sync.dma_start(bqv[:], bq_d[:])
            load_w(0, wqT_d)
            load_w(1, wkT_d)
            bkv = cpool.tile([128, NJT], F32, tag="bkv")
            nc.sync.dma_start(bkv[:], bk_d[:])
            mskv = cpool.tile([128, CPC * 3], F32, tag="mskv")
            nc.sync.dma_start(mskv[:], msk_d[:])
            nc.sync.dma_start(
                w_all[:, 2 * NJT: 2 * NJT + 4, :],
                wvT_d.rearrange("(o p) c -> p o c", p=128)[:, 0:4, :],
            )
            nc.sync.dma_start(
                w_all[:, 2 * NJT + 4: 2 * NJT + 8, :],
                wvT_d.rearrange("(o p) c -> p o c", p=128)[:, 4:8, :],
            )
            load_x(1, nc.sync)
            load_x(2, nc.sync)
            load_x(3, nc.sync)

            ones1 = cpool.tile([1, 128], F16, tag="ones1")
            nc.vector.memset(ones1[:], 1.0)
            bvb = cpool.tile([128, D], F32, tag="bvb")

            # tiny dep-free matmul: starts the PE p-state ramp clock so the
            # projections hit full clock as soon as their weights land; the
            # bvb ones-matmuls (gated only on the tiny bvrow DMA) keep the
            # PE's idle gaps under the ~3us p-state reset threshold
            ps_warm = pp_pool.tile([128, 512], F32, tag="pp")
            nc.tensor.matmul(
                ps_warm[:16, :16], ones1[:, :16], ones1[:, :16],
                start=True, stop=True,
            )
            for half in range(2):
                ps_b = pp_pool.tile([128, 512], F32, tag="pp")
                nc.tensor.matmul(
                    ps_b[:], ones1[:], bvrow[:, half * 512:(half + 1) * 512],
                    start=True, stop=True,
                )
                nc.vector.tensor_copy(
                    bvb[:, half * 512:(half + 1) * 512], ps_b[:]
                )

            # ---- memory tokens (emitted inside chunk 0's flow) ----
            memp = {}

            def emit_ktm():
                # block-diagonal mem-K: [128, jt, 64]; even head rows 0:64 ->
                # cols 0:16, odd head rows 64:128 -> cols 32:48, rest zero
                kTm3 = cpool.tile([128, NJT, 64], F16, tag="kTm3", name="kTm3")
                nc.gpsimd.memset(kTm3[:], 0.0)
                ps_k = pp_pool.tile([128, 512], F32, tag="pp", name="ps_ktm")
                for jt in range(NJT):
                    for d in range(NJT):
                        nc.tensor.matmul(
                            ps_k[:, jt * M:(jt + 1) * M],
                            wk(d)[:, jt * 128:(jt + 1) * 128],
                            xTm[:, d, :],
                            start=(d == 0), stop=(d == NJT - 1),
                            skip_group_check=True,
                        )
                for u in range(2):
                    nc.vector.tensor_tensor(
                        kTm3[64 * u:64 * (u + 1), :, 32 * u:32 * u + 16],
                        ps_k[64 * u:64 * (u + 1), :NJT * M].rearrange(
                            "p (j m) -> p j m", m=M
                        ),
                        bkv[64 * u:64 * (u + 1), :][:, :, None].to_broadcast(
                            (64, NJT, M)
                        ),
                        mybir.AluOpType.add,
                    )
                memp["kTm3"] = kTm3

            def emit_vm():
                # mem-V token-major; replicated at partition bases
                # 0/32/64/96 for the 32-aligned mem-PV stationaries
                vm = cpool.tile([128, M, VW], F16, tag="vm", name="vm")
                nc.vector.memset(vm[:M, :, HD:HD + 1], 1.0)
                for half in range(2):
                    ps_v = pp_pool.tile([128, 512], F32, tag="pp", name="ps_vm")
                    for d in range(NJT):
                        nc.tensor.matmul(
                            ps_v[:M, :], xTm[:, d, :],
                            wv(d)[:, half * 512:(half + 1) * 512],
                            start=(d == 0), stop=(d == NJT - 1),
                        )
                    nc.vector.tensor_tensor(
                        vm[:M, half * 8:(half + 1) * 8, :HD],
                        ps_v[:M, :].rearrange("p (h f) -> p h f", h=8),
                        bvb[:M, half * 512:(half + 1) * 512].rearrange(
                            "p (h f) -> p h f", h=8
                        ),
                        mybir.AluOpType.add,
                    )
                for rb in range(1, 4):
                    nc.sync.dma_start(vm[32 * rb:32 * rb + M, :, :], vm[:M, :, :])
                memp["vm"] = vm

            # ---- per-chunk phases as generators; the main loop weaves
            # chunk ci's attention with chunk ci+1's projections so the PE
            # always has projection matmuls to stream while Act runs Exps.
            state = {}

            def proj_steps(ci):
                xT = x_tiles.pop(ci)
                qT, kT = [], []
                for which, wfn, bias, lst, tg in (
                    (0, wq, bqv, qT, "qT"),
                    (1, wk, bkv, kT, "kT"),
                ):
                    # pp-buf pairs of jt-pair fills, split at the 1MB
                    # weight-DMA boundary (d 0-3 | 4-7) so chunk 0's
                    # matmuls track weight arrival
                    for hw in range(2):
                        pss = []
                        for j2 in range(2):
                            jp = 2 * hw + j2
                            ps_q = pp_pool.tile(
                                [128, 512], F32, tag="pp", name="ps_q"
                            )
                            for u in range(2):
                                jt = 2 * jp + u
                                for d in range(NJT // 2):
                                    nc.tensor.matmul(
                                        ps_q[:, u * 256:(u + 1) * 256],
                                        wfn(d)[:, jt * 128:(jt + 1) * 128],
                                        xT[d][:],
                                        start=(u == 0 and d == 0),
                                        stop=False,
                                    )
                            pss.append(ps_q)
                        for j2 in range(2):
                            jp = 2 * hw + j2
                            ps_q = pss[j2]
                            for u in range(2):
                                jt = 2 * jp + u
                                for d in range(NJT // 2, NJT):
                                    nc.tensor.matmul(
                                        ps_q[:, u * 256:(u + 1) * 256],
                                        wfn(d)[:, jt * 128:(jt + 1) * 128],
                                        xT[d][:],
                                        start=False,
                                        stop=(u == 1 and d == NJT - 1),
                                    )
                            pair_t = qkpool.tile(
                                [128, 2, W], F16, tag=tg, name=f"pair_{tg}"
                            )
                            nc.vector.tensor_tensor(
                                pair_t[:],
                                ps_q[:].rearrange("p (u t) -> p u t", u=2),
                                bias[:, 2 * jp:2 * jp + 2][:, :, None]
                                .to_broadcast((128, 2, W)),
                                mybir.AluOpType.add,
                            )
                            lst.append(pair_t)
                            yield
                    if ci == 0 and which == 1:
                        emit_ktm()
                        yield

                # V projection (token-major fp16, heads + ones col)
                v_sb = vpool.tile([128, 2, H, VW], F16, tag="v_sb")
                nc.gpsimd.memset(v_sb[:, :, :, HD:HD + 1], 1.0)
                for tt in range(2):
                    pss = []
                    for half in range(2):
                        ps_v = pp_pool.tile([128, 512], F32, tag="pp")
                        for d in range(NJT // 2):
                            nc.tensor.matmul(
                                ps_v[:], xT[d][:, tt * 128:(tt + 1) * 128],
                                wv(d)[:, half * 512:(half + 1) * 512],
                                start=(d == 0), stop=False,
                            )
                        pss.append(ps_v)
                    for half in range(2):
                        ps_v = pss[half]
                        for d in range(NJT // 2, NJT):
                            nc.tensor.matmul(
                                ps_v[:], xT[d][:, tt * 128:(tt + 1) * 128],
                                wv(d)[:, half * 512:(half + 1) * 512],
                                start=False, stop=(d == NJT - 1),
                            )
                        nc.vector.tensor_tensor(
                            v_sb[:, tt, half * 8:(half + 1) * 8, :HD],
                            ps_v[:].rearrange("p (h f) -> p h f", h=8),
                            bvb[:, half * 512:(half + 1) * 512].rearrange(
                                "p (h f) -> p h f", h=8
                            ),
                            mybir.AluOpType.add,
                        )
                        yield
                if ci == 0:
                    emit_vm()
                state[ci] = (
                    [qT[jt // 2][:, jt % 2, :] for jt in range(NJT)],
                    [kT[jt // 2][:, jt % 2, :] for jt in range(NJT)],
                    v_sb,
                )

            def attn_steps(ci):
                qT, kT, v_sb = state.pop(ci)
                kTm3, vm = memp["kTm3"], memp["vm"]

                # mem scores: all 16 heads in one 2-bank psum, one Exp.
                # Head pair jt lands at 32-aligned partition bases via the
                # block-diagonal stationary and tile_position cols.
                ps_m = ps_pool.tile([128, 1024], F32, tag="ps", name="ps_ms")
                for jt in range(NJT):
                    c0 = 64 * (jt % 2)
                    g = jt // 2
                    nc.tensor.matmul(
                        ps_m[c0:c0 + 64, g * 256:(g + 1) * 256],
                        kTm3[:, jt, :],
                        qT[jt][:],
                        start=True, stop=True,
                        tile_position=(0, c0),
                        skip_group_check=True,
                    )
                em = empool.tile([128, 4, 256], F16, tag="em", name="em")
                nc.scalar.activation(
                    em[:], ps_m[:].rearrange("p (g t) -> p g t", g=4),
                    mybir.ActivationFunctionType.Exp,
                    bias=mskv[:, ci * 3 + 2: ci * 3 + 3],
                )
                yield

                out_sb = opool.tile([128, 2, D], F16, tag="out_sb", name="out_sb")
                eloc = {}   # (hpq, yt) -> [128, 1024] fp16

                def fill(hpq, yt):
                    # local scoresT for 4 heads (one quad, one key half);
                    # both 64-row ab halves quadrant-packed; one Exp
                    ps_s = ps_pool.tile([128, 1024], F32, tag="ps", name="ps_s")
                    for ab in range(2):
                        p0 = 64 * ab
                        for u in range(2):
                            jt = 2 * hpq + u
                            nc.tensor.matmul(
                                ps_s[:, ab * 512 + u * 256:
                                     ab * 512 + (u + 1) * 256],
                                kT[jt][p0:p0 + 64, yt * 128:(yt + 1) * 128],
                                qT[jt][p0:p0 + 64, :],
                                start=(u == 0), stop=(u == 1),
                                tile_position=(p0, 0),
                                skip_group_check=True,
                            )
                    e_t = epool.tile([128, 1024], F16, tag="exps")
                    nc.scalar.activation(
                        e_t[:], ps_s[:],
                        mybir.ActivationFunctionType.Exp,
                        bias=mskv[:, ci * 3 + yt: ci * 3 + yt + 1],
                    )
                    eloc[(hpq, yt)] = e_t

                def unit(hp):
                    # PV for head pair hp: one psum bank, 4 blocks (h, xb)
                    # of 65 (64 hd + denom); strided recip + broadcast mult
                    ps_o = po_pool.tile([128, 4 * VW], F32, tag="po")
                    for ab in range(2):
                        h = 2 * hp + ab
                        e0 = eloc[(hp // 2, 0)]
                        e1 = eloc[(hp // 2, 1)]
                        base, g = _mem_slot(h)
                        for xb in range(2):
                            o = (ab * 2 + xb) * VW
                            xs = (h % 2) * 512 + (hp % 2) * 256 + xb * 128
                            nc.tensor.matmul(
                                ps_o[:, o:o + VW],
                                e0[:, xs:xs + 128], v_sb[:, 0, h, :],
                                start=True, stop=False,
                            )
                            nc.tensor.matmul(
                                ps_o[:, o:o + VW],
                                e1[:, xs:xs + 128], v_sb[:, 1, h, :],
                                start=False, stop=False,
                            )
                            nc.tensor.matmul(
                                ps_o[:, o:o + VW],
                                em[base:base + M, g, xb * 128:(xb + 1) * 128],
                                vm[base:base + M, h, :],
                                start=False, stop=True,
                                tile_position=(base, 0),
                            )
                    rec = cpool.tile([128, 4], F32, tag="rec", name="rec")
                    nc.vector.reciprocal(
                        rec[:].rearrange("p (k o) -> p k o", o=1),
                        ps_o[:].rearrange("p (k w) -> p k w", w=VW)[
                            :, :, HD:HD + 1
                        ],
                    )
                    nc.vector.tensor_tensor(
                        out_sb[:, :, 2 * hp * HD:(2 * hp + 2) * HD].rearrange(
                            "p x (a f) -> p a x f", a=2
                        ),
                        ps_o[:].rearrange("p (a x w) -> p a x w", a=2, x=2)[
                            :, :, :, :HD
                        ],
                        rec[:].rearrange("p (a x) -> p a x", a=2)[
                            :, :, :, None
                        ].to_broadcast((128, 2, 2, HD)),
                        mybir.AluOpType.mult,
                    )

                def out_dma(qtr):
                    nc.sync.dma_start(
                        out_d.rearrange("(x p) c -> p x c", p=128)[
                            :, 2 * ci:2 * ci + 2, qtr * 256:(qtr + 1) * 256
                        ],
                        out_sb[:, :, qtr * 256:(qtr + 1) * 256],
                    )

                # fills run two head-pairs ahead of PV units so the Exp
                # latency hides behind interleaved projection matmuls
                fill(0, 0); yield
                fill(0, 1); yield
                fill(1, 0); yield
                fill(1, 1); yield
                unit(0); yield
                unit(1); out_dma(0); yield
                fill(2, 0); yield
                fill(2, 1); yield
                unit(2); yield
                unit(3); out_dma(1); yield
                fill(3, 0); yield
                fill(3, 1); yield
                unit(4); yield
                unit(5); out_dma(2); yield
                unit(6); yield
                unit(7); out_dma(3)

            def drain(*gens):
                gens = [g for g in gens if g is not None]
                while gens:
                    nxt = []
                    for g in gens:
                        try:
                            next(g)
                            nxt.append(g)
                        except StopIteration:
                            pass
                    gens = nxt

            drain(proj_steps(0))
            for ci in range(CPC):
                drain(
                    attn_steps(ci),
                    proj_steps(ci + 1) if ci + 1 < CPC else None,
                )

    nc.compile()
    return nc


_NC_CACHE = None


def kernel(hidden_states, attention_mask, self_memory, Wq, bq, Wk, bk, Wv, bv):
    global _NC_CACHE, LAST_RESULTS
    hidden_states = np.asarray(np.asarray(hidden_states), np.float32)
    attention_mask = np.asarray(np.asarray(attention_mask), np.float32)
    self_memory = np.asarray(np.asarray(self_memory), np.float32)
    wqT = np.ascontiguousarray(
        (np.asarray(Wq, np.float32).T * 0.125).astype(np.float16)
    )
    wkT = np.ascontiguousarray(np.asarray(Wk, np.float32).T.astype(np.float16))
    wvT = np.ascontiguousarray(np.asarray(Wv, np.float32).T.astype(np.float16))
    bqv = np.ascontiguousarray(
        np.asarray(bq, np.float32).reshape(NJT, 128).T * 0.125
    )
    bkv = np.ascontiguousarray(np.asarray(bk, np.float32).reshape(NJT, 128).T)
    bvrow = np.asarray(bv, np.float32).astype(np.float16).reshape(1, D)

    # additive mask along the key axis, per (b, c): [yt0 | yt1 | memory]
    am = attention_mask.reshape(B, C, W)
    chunk_has_valid = (am == 0.0).sum(axis=2) > 0
    mem_mask = np.where(chunk_has_valid, 0.0, -10000.0).astype(np.float32)

    if _NC_CACHE is None:
        _NC_CACHE = _build_kernel()
    nc = _NC_CACHE

    x16 = hidden_states.astype(np.float16)
    mem16 = self_memory.astype(np.float16)

    in_maps = []
    for core in range(N_CORES):
        b = core // (N_CORES // B)
        c0 = (core % (N_CORES // B)) * CPC
        mvT = np.zeros((128, CPC * 3), np.float32)
        for ci in range(CPC):
            mvT[:, ci * 3 + 0] = am[b, c0 + ci, 0:128]
            mvT[:, ci * 3 + 1] = am[b, c0 + ci, 128:256]
            mvT[:, ci * 3 + 2] = mem_mask[b, c0 + ci]
        # feature-major pre-transposed x: [128, NJT, TPC]
        xT = np.ascontiguousarray(
            x16[b, c0 * W:(c0 + CPC) * W, :]
            .T.reshape(NJT, 128, TPC).transpose(1, 0, 2)
        ).reshape(128, NJT * TPC)
        memT = np.ascontiguousarray(
            mem16[b].T.reshape(NJT, 128, M).transpose(1, 0, 2)
        ).reshape(128, NJT * M)
        in_maps.append(
            {
                "xT": xT,
                "memT": memT,
                "wqT": wqT,
                "wkT": wkT,
                "wvT": wvT,
                "bqv": bqv,
                "bkv": bkv,
                "bvrow": bvrow,
                "maskvT": mvT,
            }
        )

    res = run_bass_kernel_spmd(nc, in_maps, list(range(N_CORES)), trace=TRACE)
    LAST_RESULTS = res

    out = np.empty((B, S, D), np.float32)
    for core in range(N_CORES):
        b = core // (N_CORES // B)
        c0 = (core % (N_CORES // B)) * CPC
        out[b, c0 * W:(c0 + CPC) * W, :] = res.results[core]["out"].astype(
            np.float32
        )
    return out


# revision 63
# speedup vs baseline: 1.1518x; 1.0098x over previous
"""Bass/Trainium2 kernel for chunked local attention with memory tokens
(BertSelfAttention variant). Self-contained: hardcodes all shapes.

Sharding: 8 cores, each handles 4 of the 32 (batch, chunk) pairs.
  core i -> b = i // 4, chunks 4*(i % 4) .. 4*(i % 4) + 3
No collectives; weights replicated per core; host scatters/gathers.

Per-core device computation (PE operands fp16, accumulation fp32):
  - xT [128, 8, 1024] arrives pre-transposed from host (feature-major),
    sliced per chunk; no device transposes
  - qT[jt] = x@(Wq/8) + bq/8 (feature-major)   kT[jt] = x@Wk + bk
  - v[t, yt, h, 65] token-major with a ones column at 64 that makes the
    PV matmul emit softmax denominators
  - memory-token K is materialized block-diagonally (kTm3 [128, jt, 64]:
    rows 0:64 x cols 0:16 = even head, rows 64:128 x cols 32:48 = odd) so
    one 128-contraction matmul scores 2 heads' mem keys at 32-aligned
    psum partitions; all 16 heads' mem scores share one 2-bank psum and
    a single Exp
  - local scoresT per (head-quad, yt) fill a 2-bank [128, 1024] psum
    (both 64-row ab halves, quadrant-packed) -> one Exp per fill
  - probs = Exp(scoresT + mask[y]) in fp16; additive key mask rides the
    ACT bias operand (per-partition scalar)
  - out_unnorm[x, 4*(hd|den)] per head-pair in one psum bank; strided
    reciprocal + one broadcast multiply normalize and emit fp16 out
Output fp16 on device, upcast to fp32 on host.
"""

import sys

sys.path.insert(0, "/opt/trn_rl_repo")

import numpy as np

import concourse.tile as tile
from concourse import bacc, mybir
from concourse.bass_utils import run_bass_kernel_spmd

F32 = mybir.dt.float32
F16 = mybir.dt.float16

B, S, D = 2, 4096, 1024
H, HD = 16, 64
W = 256            # attention window (chunk length)
C = S // W         # 16 chunks
M = 16             # memory tokens
N_CORES = 8
CPC = C * B // N_CORES  # 4 chunks per core
TPC = CPC * W           # 1024 chunk tokens per core
NJT = D // 128          # 8 feature tiles
VW = HD + 1             # v width (64 hd + ones col)

TRACE = False
LAST_RESULTS = None


def _mem_slot(h):
    """(partition base, free block) of head h's mem scores / probs."""
    jt, u = h // 2, h % 2
    return 64 * (jt % 2) + 32 * u, jt // 2


def _build_kernel():
    nc = bacc.Bacc(None, target_bir_lowering=False)

    xT_d = nc.declare_dram_parameter("xT", [128, NJT * TPC], F16, isOutput=False)
    memT_d = nc.declare_dram_parameter("memT", [128, NJT * M], F16, isOutput=False)
    wqT_d = nc.declare_dram_parameter("wqT", [D, D], F16, isOutput=False)
    wkT_d = nc.declare_dram_parameter("wkT", [D, D], F16, isOutput=False)
    wvT_d = nc.declare_dram_parameter("wvT", [D, D], F16, isOutput=False)
    bq_d = nc.declare_dram_parameter("bqv", [128, NJT], F32, isOutput=False)
    bk_d = nc.declare_dram_parameter("bkv", [128, NJT], F32, isOutput=False)
    bv_d = nc.declare_dram_parameter("bvrow", [1, D], F16, isOutput=False)
    msk_d = nc.declare_dram_parameter("maskvT", [128, CPC * 3], F32, isOutput=False)
    out_d = nc.declare_dram_parameter("out", [TPC, D], F16, isOutput=True)

    with tile.TileContext(nc) as tc:
        with (
            tc.tile_pool(name="const", bufs=1) as cpool,
            tc.tile_pool(name="wpool", bufs=1) as wpool,
            tc.tile_pool(name="xtpool", bufs=4) as xtpool,
            tc.tile_pool(name="qkpool", bufs=12) as qkpool,
            tc.tile_pool(name="vpool", bufs=3) as vpool,
            tc.tile_pool(name="epool", bufs=12) as epool,
            tc.tile_pool(name="empool", bufs=2) as empool,
            tc.tile_pool(name="opool", bufs=3) as opool,
            tc.tile_pool(name="rpool", bufs=4) as rpool,
            tc.tile_pool(name="pp", bufs=2, space="PSUM") as pp_pool,
            tc.tile_pool(name="ps", bufs=3, space="PSUM") as ps_pool,
        ):
            x_tiles = {}

            def load_x(ci, eng, split=False):
                x_t = xtpool.tile([128, NJT, W], F16, tag="xT", name="xT")
                xs = xT_d.rearrange("p (o t) -> p o t", t=TPC)[
                    :, :, ci * W:(ci + 1) * W
                ]
                if split:
                    eng.dma_start(x_t[:, 0:4, :], xs[:, 0:4, :])
                    eng.dma_start(x_t[:, 4:8, :], xs[:, 4:8, :])
                else:
                    eng.dma_start(x_t[:], xs)
                x_tiles[ci] = [x_t[:, d, :] for d in range(NJT)]

            w_all = wpool.tile([128, 3 * NJT, D], F16, tag="w_all")

            def load_w(wi, wd, gran=2):
                # row-block granules so chunk-0 matmuls track arrival
                for o in range(0, NJT, gran):
                    nc.sync.dma_start(
                        w_all[:, wi * NJT + o: wi * NJT + o + gran, :],
                        wd.rearrange("(o p) c -> p o c", p=128)[:, o:o + gran, :],
                    )

            def wq(d):
                return w_all[:, d, :]

            def wk(d):
                return w_all[:, NJT + d, :]

            def wv(d):
                return w_all[:, 2 * NJT + d, :]

            # ---- upfront DMAs: one queue, in consumption order; weights
            # in 1MB halves ordered by when the PE stream consumes them.
            load_x(0, nc.sync)
            bvrow = cpool.tile([1, D], F16, tag="bvrow")
            nc.sync.dma_start(bvrow[:], bv_d[:])
            bqv = cpool.tile([128, NJT], F32, tag="bqv")
            nc.sync.dma_start(bqv[:], bq_d[:])
            load_w(0, wqT_d, gran=4)
            xTm = cpool.tile([128, NJT, M], F16, tag="xTm")
            nc.sync.dma_start(xTm[:], memT_d.rearrange("p (o m) -> p o m", m=M))
            load_w(1, wkT_d, gran=4)
            bkv = cpool.tile([128, NJT], F32, tag="bkv")
            nc.sync.dma_start(bkv[:], bk_d[:])
            mskv = cpool.tile([128, CPC * 3], F32, tag="mskv")
            nc.sync.dma_start(mskv[:], msk_d[:])
            nc.sync.dma_start(
                w_all[:, 2 * NJT: 2 * NJT + 4, :],
                wvT_d.rearrange("(o p) c -> p o c", p=128)[:, 0:4, :],
            )
            load_x(1, nc.sync)
            nc.sync.dma_start(
                w_all[:, 2 * NJT + 4: 2 * NJT + 8, :],
                wvT_d.rearrange("(o p) c -> p o c", p=128)[:, 4:8, :],
            )
            load_x(2, nc.sync)
            load_x(3, nc.sync)

            ones1 = cpool.tile([1, 128], F16, tag="ones1")
            nc.vector.memset(ones1[:], 1.0)
            bvb = cpool.tile([128, D], F32, tag="bvb")

            # tiny dep-free matmul: starts the PE p-state ramp clock so the
            # projections hit full clock as soon as their weights land; the
            # bvb ones-matmuls (gated only on the tiny bvrow DMA) keep the
            # PE's idle gaps under the ~3us p-state reset threshold
            ps_warm = pp_pool.tile([128, 512], F32, tag="pp")
            nc.tensor.matmul(
                ps_warm[:16, :16], ones1[:, :16], ones1[:, :16],
                start=True, stop=True,
            )
            x0d0 = x_tiles[0][0]
            nc.tensor.matmul(
                ps_warm[:16, 16:32], x0d0[:16, :16], x0d0[:16, :16],
                start=True, stop=True,
            )
            for half in range(2):
                ps_b = pp_pool.tile([128, 512], F32, tag="pp")
                nc.tensor.matmul(
                    ps_b[:], ones1[:], bvrow[:, half * 512:(half + 1) * 512],
                    start=True, stop=True,
                )
                nc.vector.tensor_copy(
                    bvb[:, half * 512:(half + 1) * 512], ps_b[:]
                )

            # ---- memory tokens (emitted inside chunk 0's flow) ----
            memp = {}

            def emit_ktm():
                # block-diagonal mem-K: [128, jt, 64]; even head rows 0:64 ->
                # cols 0:16, odd head rows 64:128 -> cols 32:48, rest zero
                kTm3 = cpool.tile([128, NJT, 64], F16, tag="kTm3", name="kTm3")
                nc.gpsimd.memset(kTm3[:], 0.0)
                ps_k = pp_pool.tile([128, 512], F32, tag="pp", name="ps_ktm")
                for jt in range(NJT):
                    for d in range(NJT):
                        nc.tensor.matmul(
                            ps_k[:, jt * M:(jt + 1) * M],
                            wk(d)[:, jt * 128:(jt + 1) * 128],
                            xTm[:, d, :],
                            start=(d == 0), stop=(d == NJT - 1),
                            skip_group_check=True,
                        )
                for u in range(2):
                    nc.vector.tensor_tensor(
                        kTm3[64 * u:64 * (u + 1), :, 32 * u:32 * u + 16],
                        ps_k[64 * u:64 * (u + 1), :NJT * M].rearrange(
                            "p (j m) -> p j m", m=M
                        ),
                        bkv[64 * u:64 * (u + 1), :][:, :, None].to_broadcast(
                            (64, NJT, M)
                        ),
                        mybir.AluOpType.add,
                    )
                memp["kTm3"] = kTm3

            def emit_vm():
                # mem-V token-major; replicated at partition bases
                # 0/32/64/96 for the 32-aligned mem-PV stationaries
                vm = cpool.tile([128, M, VW], F16, tag="vm", name="vm")
                nc.vector.memset(vm[:M, :, HD:HD + 1], 1.0)
                for half in range(2):
                    ps_v = pp_pool.tile([128, 512], F32, tag="pp", name="ps_vm")
                    for d in range(NJT):
                        nc.tensor.matmul(
                            ps_v[:M, :], xTm[:, d, :],
                            wv(d)[:, half * 512:(half + 1) * 512],
                            start=(d == 0), stop=(d == NJT - 1),
                        )
                    nc.vector.tensor_tensor(
                        vm[:M, half * 8:(half + 1) * 8, :HD],
                        ps_v[:M, :].rearrange("p (h f) -> p h f", h=8),
                        bvb[:M, half * 512:(half + 1) * 512].rearrange(
                            "p (h f) -> p h f", h=8
                        ),
                        mybir.AluOpType.add,
                    )
                for rb in range(1, 4):
                    nc.sync.dma_start(vm[32 * rb:32 * rb + M, :, :], vm[:M, :, :])
                memp["vm"] = vm

            # ---- per-chunk phases as generators; the main loop weaves
            # chunk ci's attention with chunk ci+1's projections so the PE
            # always has projection matmuls to stream while Act runs Exps.
            state = {}

            def proj_steps(ci):
                xT = x_tiles.pop(ci)
                qT, kT = [], []
                for which, wfn, bias, lst, tg in (
                    (0, wq, bqv, qT, "qT"),
                    (1, wk, bkv, kT, "kT"),
                ):
                    # pp-buf pairs of jt-pair fills, split at the 1MB
                    # weight-DMA boundary (d 0-3 | 4-7) so chunk 0's
                    # matmuls track weight arrival
                    for hw in range(2):
                        pss = []
                        for j2 in range(2):
                            jp = 2 * hw + j2
                            ps_q = pp_pool.tile(
                                [128, 512], F32, tag="pp", name="ps_q"
                            )
                            for u in range(2):
                                jt = 2 * jp + u
                                for d in range(NJT // 2):
                                    nc.tensor.matmul(
                                        ps_q[:, u * 256:(u + 1) * 256],
                                        wfn(d)[:, jt * 128:(jt + 1) * 128],
                                        xT[d][:],
                                        start=(u == 0 and d == 0),
                                        stop=False,
                                    )
                            pss.append(ps_q)
                        for j2 in range(2):
                            jp = 2 * hw + j2
                            ps_q = pss[j2]
                            for u in range(2):
                                jt = 2 * jp + u
                                for d in range(NJT // 2, NJT):
                                    nc.tensor.matmul(
                                        ps_q[:, u * 256:(u + 1) * 256],
                                        wfn(d)[:, jt * 128:(jt + 1) * 128],
                                        xT[d][:],
                                        start=False,
                                        stop=(u == 1 and d == NJT - 1),
                                    )
                            pair_t = qkpool.tile(
                                [128, 2, W], F16, tag=tg, name=f"pair_{tg}"
                            )
                            nc.vector.tensor_tensor(
                                pair_t[:],
                                ps_q[:].rearrange("p (u t) -> p u t", u=2),
                                bias[:, 2 * jp:2 * jp + 2][:, :, None]
                                .to_broadcast((128, 2, W)),
                                mybir.AluOpType.add,
                            )
                            lst.append(pair_t)
                            yield
                    if ci == 0 and which == 1:
                        emit_ktm()
                        yield

                # V projection (token-major fp16, heads + ones col)
                v_sb = vpool.tile([128, 2, H, VW], F16, tag="v_sb")
                nc.gpsimd.memset(v_sb[:, :, :, HD:HD + 1], 1.0)
                for tt in range(2):
                    pss = []
                    for half in range(2):
                        ps_v = pp_pool.tile([128, 512], F32, tag="pp")
                        for d in range(NJT // 2):
                            nc.tensor.matmul(
                                ps_v[:], xT[d][:, tt * 128:(tt + 1) * 128],
                                wv(d)[:, half * 512:(half + 1) * 512],
                                start=(d == 0), stop=False,
                            )
                        pss.append(ps_v)
                    for half in range(2):
                        ps_v = pss[half]
                        for d in range(NJT // 2, NJT):
                            nc.tensor.matmul(
                                ps_v[:], xT[d][:, tt * 128:(tt + 1) * 128],
                                wv(d)[:, half * 512:(half + 1) * 512],
                                start=False, stop=(d == NJT - 1),
                            )
                        nc.vector.tensor_tensor(
                            v_sb[:, tt, half * 8:(half + 1) * 8, :HD],
                            ps_v[:].rearrange("p (h f) -> p h f", h=8),
                            bvb[:, half * 512:(half + 1) * 512].rearrange(
                                "p (h f) -> p h f", h=8
                            ),
                            mybir.AluOpType.add,
                        )
                        yield
                if ci == 0:
                    emit_vm()
                state[ci] = (
                    [qT[jt // 2][:, jt % 2, :] for jt in range(NJT)],
                    [kT[jt // 2][:, jt % 2, :] for jt in range(NJT)],
                    v_sb,
                )

            def attn_steps(ci):
                while ci not in state:
                    yield
                qT, kT, v_sb = state.pop(ci)
                kTm3, vm = memp["kTm3"], memp["vm"]

                # mem scores: all 16 heads in one 2-bank psum, one Exp.
                # Head pair jt lands at 32-aligned partition bases via the
                # block-diagonal stationary and tile_position cols.
                ps_m = ps_pool.tile([128, 1024], F32, tag="ps", name="ps_ms")
                for jt in range(NJT):
                    c0 = 64 * (jt % 2)
                    g = jt // 2
                    nc.tensor.matmul(
                        ps_m[c0:c0 + 64, g * 256:(g + 1) * 256],
                        kTm3[:, jt, :],
                        qT[jt][:],
                        start=True, stop=True,
                        tile_position=(0, c0),
                        skip_group_check=True,
                    )
                em = empool.tile([128, 4, 256], F16, tag="em", name="em")
                nc.scalar.activation(
                    em[:], ps_m[:].rearrange("p (g t) -> p g t", g=4),
                    mybir.ActivationFunctionType.Exp,
                    bias=mskv[:, ci * 3 + 2: ci * 3 + 3],
                )
                yield

                out_sb = opool.tile([128, 2, D], F16, tag="out_sb", name="out_sb")
                eloc = {}   # (hpq, yt) -> [128, 1024] fp16

                def fill(hpq, yt):
                    # local scoresT for 4 heads (one quad, one key half);
                    # both 64-row ab halves quadrant-packed; one Exp
                    ps_s = ps_pool.tile([128, 1024], F32, tag="ps", name="ps_s")
                    for ab in range(2):
                        p0 = 64 * ab
                        for u in range(2):
                            jt = 2 * hpq + u
                            nc.tensor.matmul(
                                ps_s[:, ab * 512 + u * 256:
                                     ab * 512 + (u + 1) * 256],
                                kT[jt][p0:p0 + 64, yt * 128:(yt + 1) * 128],
                                qT[jt][p0:p0 + 64, :],
                                start=(u == 0), stop=(u == 1),
                                tile_position=(p0, 0),
                                skip_group_check=True,
                            )
                    e_t = epool.tile([128, 1024], F16, tag="exps")
                    nc.scalar.activation(
                        e_t[:], ps_s[:],
                        mybir.ActivationFunctionType.Exp,
                        bias=mskv[:, ci * 3 + yt: ci * 3 + yt + 1],
                    )
                    eloc[(hpq, yt)] = e_t

                def unit(hp):
                    # PV for head pair hp: one psum bank (sharing the proj
                    # pool ring), 4 blocks (h, xb) of 65 (64 hd + denom);
                    # strided recip + broadcast mult
                    ps_o = pp_pool.tile([128, 512], F32, tag="pp", name="po")[
                        :, :4 * VW
                    ]
                    for ab in range(2):
                        h = 2 * hp + ab
                        e0 = eloc[(hp // 2, 0)]
                        e1 = eloc[(hp // 2, 1)]
                        base, g = _mem_slot(h)
                        for xb in range(2):
                            o = (ab * 2 + xb) * VW
                            xs = (h % 2) * 512 + (hp % 2) * 256 + xb * 128
                            nc.tensor.matmul(
                                ps_o[:, o:o + VW],
                                e0[:, xs:xs + 128], v_sb[:, 0, h, :],
                                start=True, stop=False,
                            )
                            nc.tensor.matmul(
                                ps_o[:, o:o + VW],
                                e1[:, xs:xs + 128], v_sb[:, 1, h, :],
                                start=False, stop=False,
                            )
                            nc.tensor.matmul(
                                ps_o[:, o:o + VW],
                                em[base:base + M, g, xb * 128:(xb + 1) * 128],
                                vm[base:base + M, h, :],
                                start=False, stop=True,
                                tile_position=(base, 0),
                            )
                    rec = rpool.tile([128, 4], F32, tag="rec", name="rec")
                    nc.vector.reciprocal(
                        rec[:].rearrange("p (k o) -> p k o", o=1),
                        ps_o[:].rearrange("p (k w) -> p k w", w=VW)[
                            :, :, HD:HD + 1
                        ],
                    )
                    nc.vector.tensor_tensor(
                        out_sb[:, :, 2 * hp * HD:(2 * hp + 2) * HD].rearrange(
                            "p x (a f) -> p a x f", a=2
                        ),
                        ps_o[:].rearrange("p (a x w) -> p a x w", a=2, x=2)[
                            :, :, :, :HD
                        ],
                        rec[:].rearrange("p (a x) -> p a x", a=2)[
                            :, :, :, None
                        ].to_broadcast((128, 2, 2, HD)),
                        mybir.AluOpType.mult,
                    )

                def out_dma(qtr):
                    nc.sync.dma_start(
                        out_d.rearrange("(x p) c -> p x c", p=128)[
                            :, 2 * ci:2 * ci + 2, qtr * 256:(qtr + 1) * 256
                        ],
                        out_sb[:, :, qtr * 256:(qtr + 1) * 256],
                    )

                # fills run two head-pairs ahead of PV units so the Exp
                # latency hides behind interleaved projection matmuls
                fill(0, 0); yield
                fill(0, 1); yield
                fill(1, 0); yield
                fill(1, 1); yield
                unit(0); yield
                unit(1); out_dma(0); yield
                fill(2, 0); yield
                fill(2, 1); yield
                unit(2); yield
                unit(3); out_dma(1); yield
                fill(3, 0); yield
                fill(3, 1); yield
                unit(4); yield
                unit(5); out_dma(2); yield
                unit(6); yield
                unit(7); out_dma(3)

            def drain(*gens):
                gens = [g for g in gens if g is not None]
                while gens:
                    nxt = []
                    for g in gens:
                        try:
                            next(g)
                            nxt.append(g)
                        except StopIteration:
                            pass
                    gens = nxt

            # 3-way weave: chunk ci's attention runs with chunk ci+1's
            # projections, and attn(ci+1) joins early (it self-waits on
            # its state) so the attention tail always has matmul filler
            attns_g = [attn_steps(ci) for ci in range(CPC)]
            drain(proj_steps(0))
            for ci in range(CPC):
                gens = [attns_g[ci]]
                must = {id(attns_g[ci])}
                if ci + 1 < CPC:
                    pj = proj_steps(ci + 1)
                    gens += [pj, attns_g[ci + 1]]
                    must.add(id(pj))
                while must:
                    for g in list(gens):
                        try:
                            next(g)
                        except StopIteration:
                            gens.remove(g)
                            must.discard(id(g))

    nc.compile()
    return nc


_NC_CACHE = None


def kernel(hidden_states, attention_mask, self_memory, Wq, bq, Wk, bk, Wv, bv):
    global _NC_CACHE, LAST_RESULTS
    hidden_states = np.asarray(np.asarray(hidden_states), np.float32)
    attention_mask = np.asarray(np.asarray(attention_mask), np.float32)
    self_memory = np.asarray(np.asarray(self_memory), np.float32)
    wqT = np.ascontiguousarray(
        (np.asarray(Wq, np.float32).T * 0.125).astype(np.float16)
    )
    wkT = np.ascontiguousarray(np.asarray(Wk, np.float32).T.astype(np.float16))
    wvT = np.ascontiguousarray(np.asarray(Wv, np.float32).T.astype(np.float16))
    bqv = np.ascontiguousarray(
        np.asarray(bq, np.float32).reshape(NJT, 128).T * 0.125
    )
    bkv = np.ascontiguousarray(np.asarray(bk, np.float32).reshape(NJT, 128).T)
    bvrow = np.asarray(bv, np.float32).astype(np.float16).reshape(1, D)

    # additive mask along the key axis, per (b, c): [yt0 | yt1 | memory]
    am = attention_mask.reshape(B, C, W)
    chunk_has_valid = (am == 0.0).sum(axis=2) > 0
    mem_mask = np.where(chunk_has_valid, 0.0, -10000.0).astype(np.float32)

    if _NC_CACHE is None:
        _NC_CACHE = _build_kernel()
    nc = _NC_CACHE

    x16 = hidden_states.astype(np.float16)
    mem16 = self_memory.astype(np.float16)

    in_maps = []
    for core in range(N_CORES):
        b = core // (N_CORES // B)
        c0 = (core % (N_CORES // B)) * CPC
        mvT = np.zeros((128, CPC * 3), np.float32)
        for ci in range(CPC):
            mvT[:, ci * 3 + 0] = am[b, c0 + ci, 0:128]
            mvT[:, ci * 3 + 1] = am[b, c0 + ci, 128:256]
            mvT[:, ci * 3 + 2] = mem_mask[b, c0 + ci]
        # feature-major pre-transposed x: [128, NJT, TPC]
        xT = np.ascontiguousarray(
            x16[b, c0 * W:(c0 + CPC) * W, :]
            .T.reshape(NJT, 128, TPC).transpose(1, 0, 2)
        ).reshape(128, NJT * TPC)
        memT = np.ascontiguousarray(
            mem16[b].T.reshape(NJT, 128, M).transpose(1, 0, 2)
        ).reshape(128, NJT * M)
        in_maps.append(
            {
                "xT": xT,
                "memT": memT,
                "wqT": wqT,
                "wkT": wkT,
                "wvT": wvT,
                "bqv": bqv,
                "bkv": bkv,
                "bvrow": bvrow,
                "maskvT": mvT,
            }
        )

    res = run_bass_kernel_spmd(nc, in_maps, list(range(N_CORES)), trace=TRACE)
    LAST_RESULTS = res

    out = np.empty((B, S, D), np.float32)
    for core in range(N_CORES):
        b = core // (N_CORES // B)
        c0 = (core % (N_CORES // B)) * CPC
        out[b, c0 * W:(c0 + CPC) * W, :] = res.results[core]["out"].astype(
            np.float32
        )
    return out


# revision 69
# speedup vs baseline: 1.1570x; 1.0045x over previous
"""Bass/Trainium2 kernel for chunked local attention with memory tokens
(BertSelfAttention variant). Self-contained: hardcodes all shapes.

Sharding: 8 cores, each handles 4 of the 32 (batch, chunk) pairs.
  core i -> b = i // 4, chunks 4*(i % 4) .. 4*(i % 4) + 3
No collectives; weights replicated per core; host scatters/gathers.

Per-core device computation (PE operands fp16, accumulation fp32):
  - xT [128, 8, 1024] arrives pre-transposed from host (feature-major),
    sliced per chunk; no device transposes
  - qT[jt] = x@(Wq/8) + bq/8 (feature-major)   kT[jt] = x@Wk + bk
  - v[t, yt, h, 65] token-major with a ones column at 64 that makes the
    PV matmul emit softmax denominators
  - memory-token K is materialized block-diagonally (kTm3 [128, jt, 64]:
    rows 0:64 x cols 0:16 = even head, rows 64:128 x cols 32:48 = odd) so
    one 128-contraction matmul scores 2 heads' mem keys at 32-aligned
    psum partitions; all 16 heads' mem scores share one 2-bank psum and
    a single Exp
  - local scoresT per (head-quad, yt) fill a 2-bank [128, 1024] psum
    (both 64-row ab halves, quadrant-packed) -> one Exp per fill
  - probs = Exp(scoresT + mask[y]) in fp16; additive key mask rides the
    ACT bias operand (per-partition scalar)
  - out_unnorm[x, 4*(hd|den)] per head-pair in one psum bank; strided
    reciprocal + one broadcast multiply normalize and emit fp16 out
Output fp16 on device, upcast to fp32 on host.

Scheduling: generator-based software pipelining weaves chunk ci's
attention with chunk ci+1's projections (and lets attn(ci+1) join as
soon as its projections land) so the PE streams matmuls while ACT runs
the Exps; projection fills split at the 1MB weight-DMA boundary so
chunk 0 tracks weight arrival; tiny warm matmuls keep the PE p-state
ramp clock alive across startup DMA waits; upfront DMAs are ordered by
first consumption on a single queue.
"""

import sys

sys.path.insert(0, "/opt/trn_rl_repo")

import numpy as np

import concourse.tile as tile
from concourse import bacc, mybir
from concourse.bass_utils import run_bass_kernel_spmd

F32 = mybir.dt.float32
F16 = mybir.dt.float16

B, S, D = 2, 4096, 1024
H, HD = 16, 64
W = 256            # attention window (chunk length)
C = S // W         # 16 chunks
M = 16             # memory tokens
N_CORES = 8
CPC = C * B // N_CORES  # 4 chunks per core
TPC = CPC * W           # 1024 chunk tokens per core
NJT = D // 128          # 8 feature tiles
VW = HD + 1             # v width (64 hd + ones col)

TRACE = False
LAST_RESULTS = None


def _mem_slot(h):
    """(partition base, free block) of head h's mem scores / probs."""
    jt, u = h // 2, h % 2
    return 64 * (jt % 2) + 32 * u, jt // 2


def _build_kernel():
    nc = bacc.Bacc(None, target_bir_lowering=False)

    xT_d = nc.declare_dram_parameter("xT", [128, NJT * TPC], F16, isOutput=False)
    memT_d = nc.declare_dram_parameter("memT", [128, NJT * M], F16, isOutput=False)
    wqT_d = nc.declare_dram_parameter("wqT", [D, D], F16, isOutput=False)
    wkT_d = nc.declare_dram_parameter("wkT", [D, D], F16, isOutput=False)
    wvT_d = nc.declare_dram_parameter("wvT", [D, D], F16, isOutput=False)
    bq_d = nc.declare_dram_parameter("bqv", [128, NJT], F32, isOutput=False)
    bk_d = nc.declare_dram_parameter("bkv", [128, NJT], F32, isOutput=False)
    bv_d = nc.declare_dram_parameter("bvrow", [1, D], F16, isOutput=False)
    msk_d = nc.declare_dram_parameter("maskvT", [128, CPC * 3], F32, isOutput=False)
    out_d = nc.declare_dram_parameter("out", [TPC, D], F16, isOutput=True)

    with tile.TileContext(nc) as tc:
        with (
            tc.tile_pool(name="const", bufs=1) as cpool,
            tc.tile_pool(name="wpool", bufs=1) as wpool,
            tc.tile_pool(name="xtpool", bufs=4) as xtpool,
            tc.tile_pool(name="qkpool", bufs=16) as qkpool,
            tc.tile_pool(name="vpool", bufs=4) as vpool,
            tc.tile_pool(name="epool", bufs=12) as epool,
            tc.tile_pool(name="empool", bufs=2) as empool,
            tc.tile_pool(name="opool", bufs=4) as opool,
            tc.tile_pool(name="rpool", bufs=4) as rpool,
            tc.tile_pool(name="pp", bufs=2, space="PSUM") as pp_pool,
            tc.tile_pool(name="ps", bufs=2, space="PSUM") as ps_pool,
            tc.tile_pool(name="po", bufs=2, space="PSUM") as po_pool,
        ):
            x_tiles = {}

            def load_x(ci, eng, split=False):
                x_t = xtpool.tile([128, NJT, W], F16, tag="xT", name="xT")
                xs = xT_d.rearrange("p (o t) -> p o t", t=TPC)[
                    :, :, ci * W:(ci + 1) * W
                ]
                if split:
                    eng.dma_start(x_t[:, 0:4, :], xs[:, 0:4, :])
                    eng.dma_start(x_t[:, 4:8, :], xs[:, 4:8, :])
                else:
                    eng.dma_start(x_t[:], xs)
                x_tiles[ci] = [x_t[:, d, :] for d in range(NJT)]

            w_all = wpool.tile([128, 3 * NJT, D], F16, tag="w_all")

            def load_w(wi, wd, gran=2):
                # row-block granules so chunk-0 matmuls track arrival
                for o in range(0, NJT, gran):
                    nc.sync.dma_start(
                        w_all[:, wi * NJT + o: wi * NJT + o + gran, :],
                        wd.rearrange("(o p) c -> p o c", p=128)[:, o:o + gran, :],
                    )

            def wq(d):
                return w_all[:, d, :]

            def wk(d):
                return w_all[:, NJT + d, :]

            def wv(d):
                return w_all[:, 2 * NJT + d, :]

            # ---- upfront DMAs: one queue, in consumption order; weights
            # in 1MB halves ordered by when the PE stream consumes them.
            load_x(0, nc.sync)
            bvrow = cpool.tile([1, D], F16, tag="bvrow")
            nc.sync.dma_start(bvrow[:], bv_d[:])
            bqv = cpool.tile([128, NJT], F32, tag="bqv")
            nc.sync.dma_start(bqv[:], bq_d[:])
            load_w(0, wqT_d, gran=4)
            xTm = cpool.tile([128, NJT, M], F16, tag="xTm")
            nc.sync.dma_start(xTm[:], memT_d.rearrange("p (o m) -> p o m", m=M))
            load_w(1, wkT_d, gran=4)
            bkv = cpool.tile([128, NJT], F32, tag="bkv")
            nc.sync.dma_start(bkv[:], bk_d[:])
            mskv = cpool.tile([128, CPC * 3], F32, tag="mskv")
            nc.sync.dma_start(mskv[:], msk_d[:])
            nc.sync.dma_start(
                w_all[:, 2 * NJT: 2 * NJT + 4, :],
                wvT_d.rearrange("(o p) c -> p o c", p=128)[:, 0:4, :],
            )
            load_x(1, nc.sync)
            nc.sync.dma_start(
                w_all[:, 2 * NJT + 4: 2 * NJT + 8, :],
                wvT_d.rearrange("(o p) c -> p o c", p=128)[:, 4:8, :],
            )
            load_x(2, nc.sync)
            load_x(3, nc.sync)

            ones1 = cpool.tile([1, 128], F16, tag="ones1")
            nc.vector.memset(ones1[:], 1.0)
            bvb = cpool.tile([128, D], F32, tag="bvb")

            # tiny dep-free matmul: starts the PE p-state ramp clock so the
            # projections hit full clock as soon as their weights land; the
            # bvb ones-matmuls (gated only on the tiny bvrow DMA) keep the
            # PE's idle gaps under the ~3us p-state reset threshold
            ps_warm = pp_pool.tile([128, 512], F32, tag="pp")
            nc.tensor.matmul(
                ps_warm[:16, :16], ones1[:, :16], ones1[:, :16],
                start=True, stop=True,
            )
            x0d0 = x_tiles[0][0]
            nc.tensor.matmul(
                ps_warm[:16, 16:32], x0d0[:16, :16], x0d0[:16, :16],
                start=True, stop=True,
            )
            for half in range(2):
                ps_b = pp_pool.tile([128, 512], F32, tag="pp")
                nc.tensor.matmul(
                    ps_b[:], ones1[:], bvrow[:, half * 512:(half + 1) * 512],
                    start=True, stop=True,
                )
                nc.vector.tensor_copy(
                    bvb[:, half * 512:(half + 1) * 512], ps_b[:]
                )

            # ---- memory tokens (emitted inside chunk 0's flow) ----
            memp = {}

            def emit_ktm():
                # block-diagonal mem-K: [128, jt, 64]; even head rows 0:64 ->
                # cols 0:16, odd head rows 64:128 -> cols 32:48, rest zero
                kTm3 = cpool.tile([128, NJT, 64], F16, tag="kTm3", name="kTm3")
                nc.gpsimd.memset(kTm3[:], 0.0)
                ps_k = pp_pool.tile([128, 512], F32, tag="pp", name="ps_ktm")
                for jt in range(NJT):
                    for d in range(NJT):
                        nc.tensor.matmul(
                            ps_k[:, jt * M:(jt + 1) * M],
                            wk(d)[:, jt * 128:(jt + 1) * 128],
                            xTm[:, d, :],
                            start=(d == 0), stop=(d == NJT - 1),
                            skip_group_check=True,
                        )
                for u in range(2):
                    nc.vector.tensor_tensor(
                        kTm3[64 * u:64 * (u + 1), :, 32 * u:32 * u + 16],
                        ps_k[64 * u:64 * (u + 1), :NJT * M].rearrange(
                            "p (j m) -> p j m", m=M
                        ),
                        bkv[64 * u:64 * (u + 1), :][:, :, None].to_broadcast(
                            (64, NJT, M)
                        ),
                        mybir.AluOpType.add,
                    )
                memp["kTm3"] = kTm3

            def emit_vm():
                # mem-V token-major; replicated at partition bases
                # 0/32/64/96 for the 32-aligned mem-PV stationaries
                vm = cpool.tile([128, M, VW], F16, tag="vm", name="vm")
                nc.vector.memset(vm[:M, :, HD:HD + 1], 1.0)
                for half in range(2):
                    ps_v = pp_pool.tile([128, 512], F32, tag="pp", name="ps_vm")
                    for d in range(NJT):
                        nc.tensor.matmul(
                            ps_v[:M, :], xTm[:, d, :],
                            wv(d)[:, half * 512:(half + 1) * 512],
                            start=(d == 0), stop=(d == NJT - 1),
                        )
                    nc.vector.tensor_tensor(
                        vm[:M, half * 8:(half + 1) * 8, :HD],
                        ps_v[:M, :].rearrange("p (h f) -> p h f", h=8),
                        bvb[:M, half * 512:(half + 1) * 512].rearrange(
                            "p (h f) -> p h f", h=8
                        ),
                        mybir.AluOpType.add,
                    )
                for rb in range(1, 4):
                    nc.sync.dma_start(vm[32 * rb:32 * rb + M, :, :], vm[:M, :, :])
                memp["vm"] = vm

            # ---- per-chunk phases as generators; the main loop weaves
            # chunk ci's attention with chunk ci+1's projections so the PE
            # always has projection matmuls to stream while Act runs Exps.
            state = {}

            def proj_steps(ci):
                xT = x_tiles.pop(ci)
                qT, kT = [], []
                for which, wfn, bias, lst, tg in (
                    (0, wq, bqv, qT, "qT"),
                    (1, wk, bkv, kT, "kT"),
                ):
                    # pp-buf pairs of jt-pair fills, split at the 1MB
                    # weight-DMA boundary (d 0-3 | 4-7) so chunk 0's
                    # matmuls track weight arrival
                    for hw in range(2):
                        pss = []
                        for j2 in range(2):
                            jp = 2 * hw + j2
                            ps_q = pp_pool.tile(
                                [128, 512], F32, tag="pp", name="ps_q"
                            )
                            for u in range(2):
                                jt = 2 * jp + u
                                for d in range(NJT // 2):
                                    nc.tensor.matmul(
                                        ps_q[:, u * 256:(u + 1) * 256],
                                        wfn(d)[:, jt * 128:(jt + 1) * 128],
                                        xT[d][:],
                                        start=(u == 0 and d == 0),
                                        stop=False,
                                    )
                            pss.append(ps_q)
                        for j2 in range(2):
                            jp = 2 * hw + j2
                            ps_q = pss[j2]
                            for u in range(2):
                                jt = 2 * jp + u
                                for d in range(NJT // 2, NJT):
                                    nc.tensor.matmul(
                                        ps_q[:, u * 256:(u + 1) * 256],
                                        wfn(d)[:, jt * 128:(jt + 1) * 128],
                                        xT[d][:],
                                        start=False,
                                        stop=(u == 1 and d == NJT - 1),
                                    )
                            pair_t = qkpool.tile(
                                [128, 2, W], F16, tag=tg, name=f"pair_{tg}"
                            )
                            nc.vector.tensor_tensor(
                                pair_t[:],
                                ps_q[:].rearrange("p (u t) -> p u t", u=2),
                                bias[:, 2 * jp:2 * jp + 2][:, :, None]
                                .to_broadcast((128, 2, W)),
                                mybir.AluOpType.add,
                            )
                            lst.append(pair_t)
                            yield
                    if ci == 0 and which == 1:
                        emit_ktm()
                        yield

                # V projection (token-major fp16, heads + ones col)
                v_sb = vpool.tile([128, 2, H, VW], F16, tag="v_sb")
                nc.gpsimd.memset(v_sb[:, :, :, HD:HD + 1], 1.0)
                for tt in range(2):
                    pss = []
                    for half in range(2):
                        ps_v = pp_pool.tile([128, 512], F32, tag="pp")
                        for d in range(NJT // 2):
                            nc.tensor.matmul(
                                ps_v[:], xT[d][:, tt * 128:(tt + 1) * 128],
                                wv(d)[:, half * 512:(half + 1) * 512],
                                start=(d == 0), stop=False,
                            )
                        pss.append(ps_v)
                    for half in range(2):
                        ps_v = pss[half]
                        for d in range(NJT // 2, NJT):
                            nc.tensor.matmul(
                                ps_v[:], xT[d][:, tt * 128:(tt + 1) * 128],
                                wv(d)[:, half * 512:(half + 1) * 512],
                                start=False, stop=(d == NJT - 1),
                            )
                        nc.vector.tensor_tensor(
                            v_sb[:, tt, half * 8:(half + 1) * 8, :HD],
                            ps_v[:].rearrange("p (h f) -> p h f", h=8),
                            bvb[:, half * 512:(half + 1) * 512].rearrange(
                                "p (h f) -> p h f", h=8
                            ),
                            mybir.AluOpType.add,
                        )
                        yield
                if ci == 0:
                    emit_vm()
                state[ci] = (
                    [qT[jt // 2][:, jt % 2, :] for jt in range(NJT)],
                    [kT[jt // 2][:, jt % 2, :] for jt in range(NJT)],
                    v_sb,
                )

            def attn_steps(ci):
                while ci not in state:
                    yield
                qT, kT, v_sb = state.pop(ci)
                kTm3, vm = memp["kTm3"], memp["vm"]

                # mem scores: all 16 heads in one 2-bank psum, one Exp.
                # Head pair jt lands at 32-aligned partition bases via the
                # block-diagonal stationary and tile_position cols.
                ps_m = ps_pool.tile([128, 1024], F32, tag="ps", name="ps_ms")
                for jt in range(NJT):
                    c0 = 64 * (jt % 2)
                    g = jt // 2
                    nc.tensor.matmul(
                        ps_m[c0:c0 + 64, g * 256:(g + 1) * 256],
                        kTm3[:, jt, :],
                        qT[jt][:],
                        start=True, stop=True,
                        tile_position=(0, c0),
                        skip_group_check=True,
                    )
                em = empool.tile([128, 4, 256], F16, tag="em", name="em")
                nc.scalar.activation(
                    em[:], ps_m[:].rearrange("p (g t) -> p g t", g=4),
                    mybir.ActivationFunctionType.Exp,
                    bias=mskv[:, ci * 3 + 2: ci * 3 + 3],
                )
                yield

                out_sb = opool.tile([128, 2, D], F16, tag="out_sb", name="out_sb")
                eloc = {}   # (hpq, yt) -> [128, 1024] fp16

                def fill(hpq, yt):
                    # local scoresT for 4 heads (one quad, one key half);
                    # both 64-row ab halves quadrant-packed; one Exp
                    ps_s = ps_pool.tile([128, 1024], F32, tag="ps", name="ps_s")
                    for ab in range(2):
                        p0 = 64 * ab
                        for u in range(2):
                            jt = 2 * hpq + u
                            nc.tensor.matmul(
                                ps_s[:, ab * 512 + u * 256:
                                     ab * 512 + (u + 1) * 256],
                                kT[jt][p0:p0 + 64, yt * 128:(yt + 1) * 128],
                                qT[jt][p0:p0 + 64, :],
                                start=(u == 0), stop=(u == 1),
                                tile_position=(p0, 0),
                                skip_group_check=True,
                            )
                    e_t = epool.tile([128, 1024], F16, tag="exps")
                    nc.scalar.activation(
                        e_t[:], ps_s[:],
                        mybir.ActivationFunctionType.Exp,
                        bias=mskv[:, ci * 3 + yt: ci * 3 + yt + 1],
                    )
                    eloc[(hpq, yt)] = e_t

                def unit(hp):
                    # PV for head pair hp: one psum bank, 4 blocks (h, xb)
                    # of 65 (64 hd + denom); strided recip + broadcast mult
                    ps_o = po_pool.tile([128, 4 * VW], F32, tag="po")
                    for ab in range(2):
                        h = 2 * hp + ab
                        e0 = eloc[(hp // 2, 0)]
                        e1 = eloc[(hp // 2, 1)]
                        base, g = _mem_slot(h)
                        for xb in range(2):
                            o = (ab * 2 + xb) * VW
                            xs = (h % 2) * 512 + (hp % 2) * 256 + xb * 128
                            nc.tensor.matmul(
                                ps_o[:, o:o + VW],
                                e0[:, xs:xs + 128], v_sb[:, 0, h, :],
                                start=True, stop=False,
                            )
                            nc.tensor.matmul(
                                ps_o[:, o:o + VW],
                                e1[:, xs:xs + 128], v_sb[:, 1, h, :],
                                start=False, stop=False,
                            )
                            nc.tensor.matmul(
                                ps_o[:, o:o + VW],
                                em[base:base + M, g, xb * 128:(xb + 1) * 128],
                                vm[base:base + M, h, :],
                                start=False, stop=True,
                                tile_position=(base, 0),
                            )
                    rec = rpool.tile([128, 4], F32, tag="rec", name="rec")
                    nc.vector.reciprocal(
                        rec[:].rearrange("p (k o) -> p k o", o=1),
                        ps_o[:].rearrange("p (k w) -> p k w", w=VW)[
                            :, :, HD:HD + 1
                        ],
                    )
                    nc.vector.tensor_tensor(
                        out_sb[:, :, 2 * hp * HD:(2 * hp + 2) * HD].rearrange(
                            "p x (a f) -> p a x f", a=2
                        ),
                        ps_o[:].rearrange("p (a x w) -> p a x w", a=2, x=2)[
                            :, :, :, :HD
                        ],
                        rec[:].rearrange("p (a x) -> p a x", a=2)[
                            :, :, :, None
                        ].to_broadcast((128, 2, 2, HD)),
                        mybir.AluOpType.mult,
                    )

                def out_dma(qtr):
                    nc.sync.dma_start(
                        out_d.rearrange("(x p) c -> p x c", p=128)[
                            :, 2 * ci:2 * ci + 2, qtr * 256:(qtr + 1) * 256
                        ],
                        out_sb[:, :, qtr * 256:(qtr + 1) * 256],
                    )

                # fills run two head-pairs ahead of PV units so the Exp
                # latency hides behind interleaved projection matmuls
                fill(0, 0); yield
                fill(0, 1); yield
                fill(1, 0); yield
                fill(1, 1); yield
                unit(0); yield
                unit(1); out_dma(0); yield
                fill(2, 0); yield
                fill(2, 1); yield
                unit(2); yield
                unit(3); out_dma(1); yield
                fill(3, 0); yield
                fill(3, 1); yield
                unit(4); yield
                unit(5); out_dma(2); yield
                unit(6); yield
                unit(7); out_dma(3)

            def drain(*gens):
                gens = [g for g in gens if g is not None]
                while gens:
                    nxt = []
                    for g in gens:
                        try:
                            next(g)
                            nxt.append(g)
                        except StopIteration:
                            pass
                    gens = nxt

            # 3-way weave: chunk ci's attention runs with chunk ci+1's
            # projections, and attn(ci+1) joins early (it self-waits on
            # its state) so the attention tail always has matmul filler
            attns_g = [attn_steps(ci) for ci in range(CPC)]
            drain(proj_steps(0))
            for ci in range(CPC):
                gens = [attns_g[ci]]
                must = {id(attns_g[ci])}
                if ci + 1 < CPC:
                    pj = proj_steps(ci + 1)
                    gens += [pj, attns_g[ci + 1]]
                    must.add(id(pj))
                while must:
                    for g in list(gens):
                        try:
                            next(g)
                        except StopIteration:
                            gens.remove(g)
                            must.discard(id(g))

    nc.compile()
    return nc


_NC_CACHE = None


def kernel(hidden_states, attention_mask, self_memory, Wq, bq, Wk, bk, Wv, bv):
    global _NC_CACHE, LAST_RESULTS
    hidden_states = np.asarray(np.asarray(hidden_states), np.float32)
    attention_mask = np.asarray(np.asarray(attention_mask), np.float32)
    self_memory = np.asarray(np.asarray(self_memory), np.float32)
    wqT = np.ascontiguousarray(
        (np.asarray(Wq, np.float32).T * 0.125).astype(np.float16)
    )
    wkT = np.ascontiguousarray(np.asarray(Wk, np.float32).T.astype(np.float16))
    wvT = np.ascontiguousarray(np.asarray(Wv, np.float32).T.astype(np.float16))
    bqv = np.ascontiguousarray(
        np.asarray(bq, np.float32).reshape(NJT, 128).T * 0.125
    )
    bkv = np.ascontiguousarray(np.asarray(bk, np.float32).reshape(NJT, 128).T)
    bvrow = np.asarray(bv, np.float32).astype(np.float16).reshape(1, D)

    # additive mask along the key axis, per (b, c): [yt0 | yt1 | memory]
    am = attention_mask.reshape(B, C, W)
    chunk_has_valid = (am == 0.0).sum(axis=2) > 0
    mem_mask = np.where(chunk_has_valid, 0.0, -10000.0).astype(np.float32)

    if _NC_CACHE is None:
        _NC_CACHE = _build_kernel()
    nc = _NC_CACHE

    x16 = hidden_states.astype(np.float16)
    mem16 = self_memory.astype(np.float16)

    in_maps = []
    for core in range(N_CORES):
        b = core // (N_CORES // B)
        c0 = (core % (N_CORES // B)) * CPC
        mvT = np.zeros((128, CPC * 3), np.float32)
        for ci in range(CPC):
            mvT[:, ci * 3 + 0] = am[b, c0 + ci, 0:128]
            mvT[:, ci * 3 + 1] = am[b, c0 + ci, 128:256]
            mvT[:, ci * 3 + 2] = mem_mask[b, c0 + ci]
        # feature-major pre-transposed x: [128, NJT, TPC]
        xT = np.ascontiguousarray(
            x16[b, c0 * W:(c0 + CPC) * W, :]
            .T.reshape(NJT, 128, TPC).transpose(1, 0, 2)
        ).reshape(128, NJT * TPC)
        memT = np.ascontiguousarray(
            mem16[b].T.reshape(NJT, 128, M).transpose(1, 0, 2)
        ).reshape(128, NJT * M)
        in_maps.append(
            {
                "xT": xT,
                "memT": memT,
                "wqT": wqT,
                "wkT": wkT,
                "wvT": wvT,
                "bqv": bqv,
                "bkv": bkv,
                "bvrow": bvrow,
                "maskvT": mvT,
            }
        )

    res = run_bass_kernel_spmd(nc, in_maps, list(range(N_CORES)), trace=TRACE)
    LAST_RESULTS = res

    out = np.empty((B, S, D), np.float32)
    for core in range(N_CORES):
        b = core // (N_CORES // B)
        c0 = (core % (N_CORES // B)) * CPC
        out[b, c0 * W:(c0 + CPC) * W, :] = res.results[core]["out"].astype(
            np.float32
        )
    return out


# revision 75
# speedup vs baseline: 1.1583x; 1.0012x over previous
"""Bass/Trainium2 kernel for chunked local attention with memory tokens
(BertSelfAttention variant). Self-contained: hardcodes all shapes.

Sharding: 8 cores, each handles 4 of the 32 (batch, chunk) pairs.
  core i -> b = i // 4, chunks 4*(i % 4) .. 4*(i % 4) + 3
No collectives; weights replicated per core; host scatters/gathers.

Per-core device computation (PE operands fp16, accumulation fp32):
  - xT [128, 8, 1024] arrives pre-transposed from host (feature-major),
    sliced per chunk; no device transposes
  - qT[jt] = x@(Wq/8) + bq/8 (feature-major)   kT[jt] = x@Wk + bk
  - v[t, yt, h, 65] token-major with a ones column at 64 that makes the
    PV matmul emit softmax denominators
  - memory-token K is materialized block-diagonally (kTm3 [128, jt, 64]:
    rows 0:64 x cols 0:16 = even head, rows 64:128 x cols 32:48 = odd) so
    one 128-contraction matmul scores 2 heads' mem keys at 32-aligned
    psum partitions; all 16 heads' mem scores share one 2-bank psum and
    a single Exp
  - local scoresT per (head-quad, yt) fill a 2-bank [128, 1024] psum
    (both 64-row ab halves, quadrant-packed) -> one Exp per fill
  - probs = Exp(scoresT + mask[y]) in fp16; additive key mask rides the
    ACT bias operand (per-partition scalar)
  - out_unnorm[x, 4*(hd|den)] per head-pair in one psum bank; strided
    reciprocal + one broadcast multiply normalize and emit fp16 out
Output fp16 on device, upcast to fp32 on host.

Scheduling: generator-based software pipelining weaves chunk ci's
attention with chunk ci+1's projections (and lets attn(ci+1) join as
soon as its projections land) so the PE streams matmuls while ACT runs
the Exps; projection fills split at the 1MB weight-DMA boundary so
chunk 0 tracks weight arrival; tiny warm matmuls keep the PE p-state
ramp clock alive across startup DMA waits; upfront DMAs are ordered by
first consumption on a single queue.
"""

import sys

sys.path.insert(0, "/opt/trn_rl_repo")

import numpy as np

import concourse.tile as tile
from concourse import bacc, mybir
from concourse.bass_utils import run_bass_kernel_spmd

F32 = mybir.dt.float32
F16 = mybir.dt.float16

B, S, D = 2, 4096, 1024
H, HD = 16, 64
W = 256            # attention window (chunk length)
C = S // W         # 16 chunks
M = 16             # memory tokens
N_CORES = 8
CPC = C * B // N_CORES  # 4 chunks per core
TPC = CPC * W           # 1024 chunk tokens per core
NJT = D // 128          # 8 feature tiles
VW = HD + 1             # v width (64 hd + ones col)

TRACE = False
LAST_RESULTS = None


def _mem_slot(h):
    """(partition base, free block) of head h's mem scores / probs."""
    jt, u = h // 2, h % 2
    return 64 * (jt % 2) + 32 * u, jt // 2


def _build_kernel():
    nc = bacc.Bacc(None, target_bir_lowering=False)

    xT_d = nc.declare_dram_parameter("xT", [128, NJT * TPC], F16, isOutput=False)
    memT_d = nc.declare_dram_parameter("memT", [128, NJT * M], F16, isOutput=False)
    wqT_d = nc.declare_dram_parameter("wqT", [D, D], F16, isOutput=False)
    wkT_d = nc.declare_dram_parameter("wkT", [D, D], F16, isOutput=False)
    wvT_d = nc.declare_dram_parameter("wvT", [D, D], F16, isOutput=False)
    bq_d = nc.declare_dram_parameter("bqv", [128, NJT], F32, isOutput=False)
    bk_d = nc.declare_dram_parameter("bkv", [128, NJT], F32, isOutput=False)
    bv_d = nc.declare_dram_parameter("bvrow", [1, D], F16, isOutput=False)
    msk_d = nc.declare_dram_parameter("maskvT", [128, CPC * 3], F32, isOutput=False)
    out_d = nc.declare_dram_parameter("out", [TPC, D], F16, isOutput=True)

    with tile.TileContext(nc) as tc:
        with (
            tc.tile_pool(name="const", bufs=1) as cpool,
            tc.tile_pool(name="wpool", bufs=1) as wpool,
            tc.tile_pool(name="xtpool", bufs=4) as xtpool,
            tc.tile_pool(name="qkpool", bufs=16) as qkpool,
            tc.tile_pool(name="vpool", bufs=4) as vpool,
            tc.tile_pool(name="epool", bufs=16) as epool,
            tc.tile_pool(name="empool", bufs=3) as empool,
            tc.tile_pool(name="opool", bufs=4) as opool,
            tc.tile_pool(name="rpool", bufs=4) as rpool,
            tc.tile_pool(name="pp", bufs=2, space="PSUM") as pp_pool,
            tc.tile_pool(name="ps", bufs=2, space="PSUM") as ps_pool,
            tc.tile_pool(name="po", bufs=2, space="PSUM") as po_pool,
        ):
            x_tiles = {}

            def load_x(ci, eng, split=False):
                x_t = xtpool.tile([128, NJT, W], F16, tag="xT", name="xT")
                xs = xT_d.rearrange("p (o t) -> p o t", t=TPC)[
                    :, :, ci * W:(ci + 1) * W
                ]
                if split:
                    eng.dma_start(x_t[:, 0:4, :], xs[:, 0:4, :])
                    eng.dma_start(x_t[:, 4:8, :], xs[:, 4:8, :])
                else:
                    eng.dma_start(x_t[:], xs)
                x_tiles[ci] = [x_t[:, d, :] for d in range(NJT)]

            w_all = wpool.tile([128, 3 * NJT, D], F16, tag="w_all")

            def load_w(wi, wd, gran=2):
                # row-block granules so chunk-0 matmuls track arrival
                for o in range(0, NJT, gran):
                    nc.sync.dma_start(
                        w_all[:, wi * NJT + o: wi * NJT + o + gran, :],
                        wd.rearrange("(o p) c -> p o c", p=128)[:, o:o + gran, :],
                    )

            def wq(d):
                return w_all[:, d, :]

            def wk(d):
                return w_all[:, NJT + d, :]

            def wv(d):
                return w_all[:, 2 * NJT + d, :]

            # ---- upfront DMAs: one queue, in consumption order; weights
            # in 1MB halves ordered by when the PE stream consumes them.
            load_x(0, nc.sync)
            bvrow = cpool.tile([1, D], F16, tag="bvrow")
            nc.sync.dma_start(bvrow[:], bv_d[:])
            bqv = cpool.tile([128, NJT], F32, tag="bqv")
            nc.sync.dma_start(bqv[:], bq_d[:])
            load_w(0, wqT_d, gran=4)
            xTm = cpool.tile([128, NJT, M], F16, tag="xTm")
            nc.sync.dma_start(xTm[:], memT_d.rearrange("p (o m) -> p o m", m=M))
            load_w(1, wkT_d, gran=4)
            bkv = cpool.tile([128, NJT], F32, tag="bkv")
            nc.sync.dma_start(bkv[:], bk_d[:])
            mskv = cpool.tile([128, CPC * 3], F32, tag="mskv")
            nc.sync.dma_start(mskv[:], msk_d[:])
            nc.sync.dma_start(
                w_all[:, 2 * NJT: 2 * NJT + 4, :],
                wvT_d.rearrange("(o p) c -> p o c", p=128)[:, 0:4, :],
            )
            load_x(1, nc.sync)
            nc.sync.dma_start(
                w_all[:, 2 * NJT + 4: 2 * NJT + 8, :],
                wvT_d.rearrange("(o p) c -> p o c", p=128)[:, 4:8, :],
            )
            load_x(2, nc.sync)
            load_x(3, nc.sync)

            ones1 = cpool.tile([1, 128], F16, tag="ones1")
            nc.vector.memset(ones1[:], 1.0)
            bvb = cpool.tile([128, D], F32, tag="bvb")

            # tiny dep-free matmul: starts the PE p-state ramp clock so the
            # projections hit full clock as soon as their weights land; the
            # bvb ones-matmuls (gated only on the tiny bvrow DMA) keep the
            # PE's idle gaps under the ~3us p-state reset threshold
            ps_warm = pp_pool.tile([128, 512], F32, tag="pp")
            nc.tensor.matmul(
                ps_warm[:16, :16], ones1[:, :16], ones1[:, :16],
                start=True, stop=True,
            )
            x0d0 = x_tiles[0][0]
            nc.tensor.matmul(
                ps_warm[:16, 16:32], x0d0[:16, :16], x0d0[:16, :16],
                start=True, stop=True,
            )
            for half in range(2):
                ps_b = pp_pool.tile([128, 512], F32, tag="pp")
                nc.tensor.matmul(
                    ps_b[:], ones1[:], bvrow[:, half * 512:(half + 1) * 512],
                    start=True, stop=True,
                )
                nc.vector.tensor_copy(
                    bvb[:, half * 512:(half + 1) * 512], ps_b[:]
                )

            # ---- memory tokens (emitted inside chunk 0's flow) ----
            memp = {}

            def emit_ktm():
                # block-diagonal mem-K: [128, jt, 64]; even head rows 0:64 ->
                # cols 0:16, odd head rows 64:128 -> cols 32:48, rest zero
                kTm3 = cpool.tile([128, NJT, 64], F16, tag="kTm3", name="kTm3")
                nc.gpsimd.memset(kTm3[:], 0.0)
                ps_k = pp_pool.tile([128, 512], F32, tag="pp", name="ps_ktm")
                for jt in range(NJT):
                    for d in range(NJT):
                        nc.tensor.matmul(
                            ps_k[:, jt * M:(jt + 1) * M],
                            wk(d)[:, jt * 128:(jt + 1) * 128],
                            xTm[:, d, :],
                            start=(d == 0), stop=(d == NJT - 1),
                            skip_group_check=True,
                        )
                for u in range(2):
                    nc.vector.tensor_tensor(
                        kTm3[64 * u:64 * (u + 1), :, 32 * u:32 * u + 16],
                        ps_k[64 * u:64 * (u + 1), :NJT * M].rearrange(
                            "p (j m) -> p j m", m=M
                        ),
                        bkv[64 * u:64 * (u + 1), :][:, :, None].to_broadcast(
                            (64, NJT, M)
                        ),
                        mybir.AluOpType.add,
                    )
                memp["kTm3"] = kTm3

            def emit_vm():
                # mem-V token-major; replicated at partition bases
                # 0/32/64/96 for the 32-aligned mem-PV stationaries
                vm = cpool.tile([128, M, VW], F16, tag="vm", name="vm")
                nc.vector.memset(vm[:M, :, HD:HD + 1], 1.0)
                for half in range(2):
                    ps_v = pp_pool.tile([128, 512], F32, tag="pp", name="ps_vm")
                    for d in range(NJT):
                        nc.tensor.matmul(
                            ps_v[:M, :], xTm[:, d, :],
                            wv(d)[:, half * 512:(half + 1) * 512],
                            start=(d == 0), stop=(d == NJT - 1),
                        )
                    nc.vector.tensor_tensor(
                        vm[:M, half * 8:(half + 1) * 8, :HD],
                        ps_v[:M, :].rearrange("p (h f) -> p h f", h=8),
                        bvb[:M, half * 512:(half + 1) * 512].rearrange(
                            "p (h f) -> p h f", h=8
                        ),
                        mybir.AluOpType.add,
                    )
                for rb in range(1, 4):
                    nc.sync.dma_start(vm[32 * rb:32 * rb + M, :, :], vm[:M, :, :])
                memp["vm"] = vm

            # ---- per-chunk phases as generators; the main loop weaves
            # chunk ci's attention with chunk ci+1's projections so the PE
            # always has projection matmuls to stream while Act runs Exps.
            state = {}

            def proj_steps(ci):
                xT = x_tiles.pop(ci)
                qT, kT = [], []
                for which, wfn, bias, lst, tg in (
                    (0, wq, bqv, qT, "qT"),
                    (1, wk, bkv, kT, "kT"),
                ):
                    # pp-buf pairs of jt-pair fills, split at the 1MB
                    # weight-DMA boundary (d 0-3 | 4-7) so chunk 0's
                    # matmuls track weight arrival
                    for hw in range(2):
                        pss = []
                        for j2 in range(2):
                            jp = 2 * hw + j2
                            ps_q = pp_pool.tile(
                                [128, 512], F32, tag="pp", name="ps_q"
                            )
                            for u in range(2):
                                jt = 2 * jp + u
                                for d in range(NJT // 2):
                                    nc.tensor.matmul(
                                        ps_q[:, u * 256:(u + 1) * 256],
                                        wfn(d)[:, jt * 128:(jt + 1) * 128],
                                        xT[d][:],
                                        start=(u == 0 and d == 0),
                                        stop=False,
                                    )
                            pss.append(ps_q)
                        for j2 in range(2):
                            jp = 2 * hw + j2
                            ps_q = pss[j2]
                            for u in range(2):
                                jt = 2 * jp + u
                                for d in range(NJT // 2, NJT):
                                    nc.tensor.matmul(
                                        ps_q[:, u * 256:(u + 1) * 256],
                                        wfn(d)[:, jt * 128:(jt + 1) * 128],
                                        xT[d][:],
                                        start=False,
                                        stop=(u == 1 and d == NJT - 1),
                                    )
                            pair_t = qkpool.tile(
                                [128, 2, W], F16, tag=tg, name=f"pair_{tg}"
                            )
                            nc.vector.tensor_tensor(
                                pair_t[:],
                                ps_q[:].rearrange("p (u t) -> p u t", u=2),
                                bias[:, 2 * jp:2 * jp + 2][:, :, None]
                                .to_broadcast((128, 2, W)),
                                mybir.AluOpType.add,
                            )
                            lst.append(pair_t)
                            yield
                    if ci == 0 and which == 1:
                        emit_ktm()
                        yield

                # V projection (token-major fp16, heads + ones col)
                v_sb = vpool.tile([128, 2, H, VW], F16, tag="v_sb")
                nc.gpsimd.memset(v_sb[:, :, :, HD:HD + 1], 1.0)
                for tt in range(2):
                    pss = []
                    for half in range(2):
                        ps_v = pp_pool.tile([128, 512], F32, tag="pp")
                        for d in range(NJT // 2):
                            nc.tensor.matmul(
                                ps_v[:], xT[d][:, tt * 128:(tt + 1) * 128],
                                wv(d)[:, half * 512:(half + 1) * 512],
                                start=(d == 0), stop=False,
                            )
                        pss.append(ps_v)
                    for half in range(2):
                        ps_v = pss[half]
                        for d in range(NJT // 2, NJT):
                            nc.tensor.matmul(
                                ps_v[:], xT[d][:, tt * 128:(tt + 1) * 128],
                                wv(d)[:, half * 512:(half + 1) * 512],
                                start=False, stop=(d == NJT - 1),
                            )
                        nc.vector.tensor_tensor(
                            v_sb[:, tt, half * 8:(half + 1) * 8, :HD],
                            ps_v[:].rearrange("p (h f) -> p h f", h=8),
                            bvb[:, half * 512:(half + 1) * 512].rearrange(
                                "p (h f) -> p h f", h=8
                            ),
                            mybir.AluOpType.add,
                        )
                        yield
                if ci == 0:
                    emit_vm()
                state[ci] = (
                    [qT[jt // 2][:, jt % 2, :] for jt in range(NJT)],
                    [kT[jt // 2][:, jt % 2, :] for jt in range(NJT)],
                    v_sb,
                )

            def attn_steps(ci):
                while ci not in state:
                    yield
                qT, kT, v_sb = state.pop(ci)
                kTm3, vm = memp["kTm3"], memp["vm"]

                # mem scores: all 16 heads in one 2-bank psum, one Exp.
                # Head pair jt lands at 32-aligned partition bases via the
                # block-diagonal stationary and tile_position cols.
                ps_m = ps_pool.tile([128, 1024], F32, tag="ps", name="ps_ms")
                for jt in range(NJT):
                    c0 = 64 * (jt % 2)
                    g = jt // 2
                    nc.tensor.matmul(
                        ps_m[c0:c0 + 64, g * 256:(g + 1) * 256],
                        kTm3[:, jt, :],
                        qT[jt][:],
                        start=True, stop=True,
                        tile_position=(0, c0),
                        skip_group_check=True,
                    )
                em = empool.tile([128, 4, 256], F16, tag="em", name="em")
                nc.scalar.activation(
                    em[:], ps_m[:].rearrange("p (g t) -> p g t", g=4),
                    mybir.ActivationFunctionType.Exp,
                    bias=mskv[:, ci * 3 + 2: ci * 3 + 3],
                )
                yield

                out_sb = opool.tile([128, 2, D], F16, tag="out_sb", name="out_sb")
                eloc = {}   # (hpq, yt) -> [128, 1024] fp16

                def fill(hpq, yt):
                    # local scoresT for 4 heads (one quad, one key half);
                    # both 64-row ab halves quadrant-packed; one Exp
                    ps_s = ps_pool.tile([128, 1024], F32, tag="ps", name="ps_s")
                    for ab in range(2):
                        p0 = 64 * ab
                        for u in range(2):
                            jt = 2 * hpq + u
                            nc.tensor.matmul(
                                ps_s[:, ab * 512 + u * 256:
                                     ab * 512 + (u + 1) * 256],
                                kT[jt][p0:p0 + 64, yt * 128:(yt + 1) * 128],
                                qT[jt][p0:p0 + 64, :],
                                start=(u == 0), stop=(u == 1),
                                tile_position=(p0, 0),
                                skip_group_check=True,
                            )
                    e_t = epool.tile([128, 1024], F16, tag="exps")
                    nc.scalar.activation(
                        e_t[:], ps_s[:],
                        mybir.ActivationFunctionType.Exp,
                        bias=mskv[:, ci * 3 + yt: ci * 3 + yt + 1],
                    )
                    eloc[(hpq, yt)] = e_t

                def unit(hp):
                    # PV for head pair hp: one psum bank, 4 blocks (h, xb)
                    # of 65 (64 hd + denom); strided recip + broadcast mult
                    ps_o = po_pool.tile([128, 4 * VW], F32, tag="po")
                    for ab in range(2):
                        h = 2 * hp + ab
                        e0 = eloc[(hp // 2, 0)]
                        e1 = eloc[(hp // 2, 1)]
                        base, g = _mem_slot(h)
                        for xb in range(2):
                            o = (ab * 2 + xb) * VW
                            xs = (h % 2) * 512 + (hp % 2) * 256 + xb * 128
                            nc.tensor.matmul(
                                ps_o[:, o:o + VW],
                                e0[:, xs:xs + 128], v_sb[:, 0, h, :],
                                start=True, stop=False,
                            )
                            nc.tensor.matmul(
                                ps_o[:, o:o + VW],
                                e1[:, xs:xs + 128], v_sb[:, 1, h, :],
                                start=False, stop=False,
                            )
                            nc.tensor.matmul(
                                ps_o[:, o:o + VW],
                                em[base:base + M, g, xb * 128:(xb + 1) * 128],
                                vm[base:base + M, h, :],
                                start=False, stop=True,
                                tile_position=(base, 0),
                            )
                    rec = rpool.tile([128, 4], F32, tag="rec", name="rec")
                    nc.vector.reciprocal(
                        rec[:].rearrange("p (k o) -> p k o", o=1),
                        ps_o[:].rearrange("p (k w) -> p k w", w=VW)[
                            :, :, HD:HD + 1
                        ],
                    )
                    nc.vector.tensor_tensor(
                        out_sb[:, :, 2 * hp * HD:(2 * hp + 2) * HD].rearrange(
                            "p x (a f) -> p a x f", a=2
                        ),
                        ps_o[:].rearrange("p (a x w) -> p a x w", a=2, x=2)[
                            :, :, :, :HD
                        ],
                        rec[:].rearrange("p (a x) -> p a x", a=2)[
                            :, :, :, None
                        ].to_broadcast((128, 2, 2, HD)),
                        mybir.AluOpType.mult,
                    )

                def out_dma(qtr):
                    nc.sync.dma_start(
                        out_d.rearrange("(x p) c -> p x c", p=128)[
                            :, 2 * ci:2 * ci + 2, qtr * 256:(qtr + 1) * 256
                        ],
                        out_sb[:, :, qtr * 256:(qtr + 1) * 256],
                    )

                # fills run two head-pairs ahead of PV units so the Exp
                # latency hides behind interleaved projection matmuls
                fill(0, 0); yield
                fill(0, 1); yield
                fill(1, 0); yield
                fill(1, 1); yield
                unit(0); yield
                unit(1); out_dma(0); yield
                fill(2, 0); yield
                fill(2, 1); yield
                unit(2); yield
                unit(3); out_dma(1); yield
                fill(3, 0); yield
                fill(3, 1); yield
                unit(4); yield
                unit(5); out_dma(2); yield
                unit(6); yield
                unit(7); out_dma(3)

            def drain(*gens):
                gens = [g for g in gens if g is not None]
                while gens:
                    nxt = []
                    for g in gens:
                        try:
                            next(g)
                            nxt.append(g)
                        except StopIteration:
                            pass
                    gens = nxt

            # 3-way weave: chunk ci's attention runs with chunk ci+1's
            # projections, and attn(ci+1) joins early (it self-waits on
            # its state) so the attention tail always has matmul filler
            attns_g = [attn_steps(ci) for ci in range(CPC)]
            drain(proj_steps(0))
            for ci in range(CPC):
                gens = [attns_g[ci]]
                must = {id(attns_g[ci])}
                if ci + 1 < CPC:
                    pj = proj_steps(ci + 1)
                    gens = [pj, attns_g[ci], attns_g[ci + 1]]
                    must.add(id(pj))
                while must:
                    for g in list(gens):
                        try:
                            next(g)
                        except StopIteration:
                            gens.remove(g)
                            must.discard(id(g))

    nc.compile()
    return nc


_NC_CACHE = None


def kernel(hidden_states, attention_mask, self_memory, Wq, bq, Wk, bk, Wv, bv):
    global _NC_CACHE, LAST_RESULTS
    hidden_states = np.asarray(np.asarray(hidden_states), np.float32)
    attention_mask = np.asarray(np.asarray(attention_mask), np.float32)
    self_memory = np.asarray(np.asarray(self_memory), np.float32)
    wqT = np.ascontiguousarray(
        (np.asarray(Wq, np.float32).T * 0.125).astype(np.float16)
    )
    wkT = np.ascontiguousarray(np.asarray(Wk, np.float32).T.astype(np.float16))
    wvT = np.ascontiguousarray(np.asarray(Wv, np.float32).T.astype(np.float16))
    bqv = np.ascontiguousarray(
        np.asarray(bq, np.float32).reshape(NJT, 128).T * 0.125
    )
    bkv = np.ascontiguousarray(np.asarray(bk, np.float32).reshape(NJT, 128).T)
    bvrow = np.asarray(bv, np.float32).astype(np.float16).reshape(1, D)

    # additive mask along the key axis, per (b, c): [yt0 | yt1 | memory]
    am = attention_mask.reshape(B, C, W)
    chunk_has_valid = (am == 0.0).sum(axis=2) > 0
    mem_mask = np.where(chunk_has_valid, 0.0, -10000.0).astype(np.float32)

    if _NC_CACHE is None:
        _NC_CACHE = _build_kernel()
    nc = _NC_CACHE

    x16 = hidden_states.astype(np.float16)
    mem16 = self_memory.astype(np.float16)

    in_maps = []
    for core in range(N_CORES):
        b = core // (N_CORES // B)
        c0 = (core % (N_CORES // B)) * CPC
        mvT = np.zeros((128, CPC * 3), np.float32)
        for ci in range(CPC):
            mvT[:, ci * 3 + 0] = am[b, c0 + ci, 0:128]
            mvT[:, ci * 3 + 1] = am[b, c0 + ci, 128:256]
            mvT[:, ci * 3 + 2] = mem_mask[b, c0 + ci]
        # feature-major pre-transposed x: [128, NJT, TPC]
        xT = np.ascontiguousarray(
            x16[b, c0 * W:(c0 + CPC) * W, :]
            .T.reshape(NJT, 128, TPC).transpose(1, 0, 2)
        ).reshape(128, NJT * TPC)
        memT = np.ascontiguousarray(
            mem16[b].T.reshape(NJT, 128, M).transpose(1, 0, 2)
        ).reshape(128, NJT * M)
        in_maps.append(
            {
                "xT": xT,
                "memT": memT,
                "wqT": wqT,
                "wkT": wkT,
                "wvT": wvT,
                "bqv": bqv,
                "bkv": bkv,
                "bvrow": bvrow,
                "maskvT": mvT,
            }
        )

    res = run_bass_kernel_spmd(nc, in_maps, list(range(N_CORES)), trace=TRACE)
    LAST_RESULTS = res

    out = np.empty((B, S, D), np.float32)
    for core in range(N_CORES):
        b = core // (N_CORES // B)
        c0 = (core % (N_CORES // B)) * CPC
        out[b, c0 * W:(c0 + CPC) * W, :] = res.results[core]["out"].astype(
            np.float32
        )
    return out


# revision 80
# speedup vs baseline: 1.1830x; 1.0213x over previous
"""Bass/Trainium2 kernel for chunked local attention with memory tokens
(BertSelfAttention variant). Self-contained: hardcodes all shapes.

Sharding: 8 cores, each handles 4 of the 32 (batch, chunk) pairs.
  core i -> b = i // 4, chunks 4*(i % 4) .. 4*(i % 4) + 3
No collectives; weights replicated per core; host scatters/gathers.

Per-core device computation (PE operands fp16, accumulation fp32):
  - xT [128, 8, 1024] arrives pre-transposed from host (feature-major),
    sliced per chunk; no device transposes
  - qT[jt] = x@(Wq/8) + bq/8 (feature-major)   kT[jt] = x@Wk + bk
  - v[t, yt, h, 65] token-major with a ones column at 64 that makes the
    PV matmul emit softmax denominators
  - memory-token K is materialized block-diagonally (kTm3 [128, jt, 64]:
    rows 0:64 x cols 0:16 = even head, rows 64:128 x cols 32:48 = odd) so
    one 128-contraction matmul scores 2 heads' mem keys at 32-aligned
    psum partitions; all 16 heads' mem scores share one 2-bank psum and
    a single Exp
  - local scoresT per (head-quad, yt) fill a 2-bank [128, 1024] psum
    (both 64-row ab halves, quadrant-packed) -> one Exp per fill
  - probs = Exp(scoresT + mask[y]) in fp16; additive key mask rides the
    ACT bias operand (per-partition scalar)
  - out_unnorm[x, 4*(hd|den)] per head-pair in one psum bank; strided
    reciprocal + one broadcast multiply normalize and emit fp16 out
Output fp16 on device, upcast to fp32 on host.

Scheduling: generator-based software pipelining weaves chunk ci's
attention with chunk ci+1's projections (and lets attn(ci+1) join as
soon as its projections land) so the PE streams matmuls while ACT runs
the Exps; projection fills split at the 1MB weight-DMA boundary so
chunk 0 tracks weight arrival; tiny warm matmuls keep the PE p-state
ramp clock alive across startup DMA waits; upfront DMAs are ordered by
first consumption on a single queue.
"""

import sys

sys.path.insert(0, "/opt/trn_rl_repo")

import numpy as np

import concourse.tile as tile
from concourse import bacc, mybir
from concourse.bass_utils import run_bass_kernel_spmd

F32 = mybir.dt.float32
F16 = mybir.dt.float16

B, S, D = 2, 4096, 1024
H, HD = 16, 64
W = 256            # attention window (chunk length)
C = S // W         # 16 chunks
M = 16             # memory tokens
N_CORES = 8
CPC = C * B // N_CORES  # 4 chunks per core
TPC = CPC * W           # 1024 chunk tokens per core
NJT = D // 128          # 8 feature tiles
VW = HD + 1             # v width (64 hd + ones col)

TRACE = False
LAST_RESULTS = None


def _mem_slot(h):
    """(partition base, free block) of head h's mem scores / probs."""
    jt, u = h // 2, h % 2
    return 64 * (jt % 2) + 32 * u, jt // 2


def _build_kernel():
    nc = bacc.Bacc(None, target_bir_lowering=False)

    xT_d = nc.declare_dram_parameter("xT", [128, NJT * TPC], F16, isOutput=False)
    memT_d = nc.declare_dram_parameter("memT", [128, NJT * M], F16, isOutput=False)
    wqT_d = nc.declare_dram_parameter("wqT", [D, D], F16, isOutput=False)
    wkT_d = nc.declare_dram_parameter("wkT", [D, D], F16, isOutput=False)
    wvT_d = nc.declare_dram_parameter("wvT", [D, D], F16, isOutput=False)
    bq_d = nc.declare_dram_parameter("bqv", [128, NJT], F32, isOutput=False)
    bk_d = nc.declare_dram_parameter("bkv", [128, NJT], F32, isOutput=False)
    bv_d = nc.declare_dram_parameter("bvrow", [1, D], F16, isOutput=False)
    bvv_d = nc.declare_dram_parameter("bvv", [128, NJT], F32, isOutput=False)
    msk_d = nc.declare_dram_parameter("maskvT", [128, CPC * 3], F32, isOutput=False)
    out_d = nc.declare_dram_parameter("out", [TPC, D], F16, isOutput=True)

    with tile.TileContext(nc) as tc:
        with (
            tc.tile_pool(name="const", bufs=1) as cpool,
            tc.tile_pool(name="wpool", bufs=1) as wpool,
            tc.tile_pool(name="xtpool", bufs=4) as xtpool,
            tc.tile_pool(name="qkpool", bufs=16) as qkpool,
            tc.tile_pool(name="vpool", bufs=4) as vpool,
            tc.tile_pool(name="epool", bufs=16) as epool,
            tc.tile_pool(name="empool", bufs=3) as empool,
            tc.tile_pool(name="opool", bufs=4) as opool,
            tc.tile_pool(name="rpool", bufs=4) as rpool,
            tc.tile_pool(name="pp", bufs=2, space="PSUM") as pp_pool,
            tc.tile_pool(name="ps", bufs=2, space="PSUM") as ps_pool,
            tc.tile_pool(name="po", bufs=2, space="PSUM") as po_pool,
        ):
            x_tiles = {}

            def load_x(ci, eng, split=False):
                x_t = xtpool.tile([128, NJT, W], F16, tag="xT", name="xT")
                xs = xT_d.rearrange("p (o t) -> p o t", t=TPC)[
                    :, :, ci * W:(ci + 1) * W
                ]
                if split:
                    eng.dma_start(x_t[:, 0:4, :], xs[:, 0:4, :])
                    eng.dma_start(x_t[:, 4:8, :], xs[:, 4:8, :])
                else:
                    eng.dma_start(x_t[:], xs)
                x_tiles[ci] = [x_t[:, d, :] for d in range(NJT)]

            w_all = wpool.tile([128, 3 * NJT, D], F16, tag="w_all")

            def load_w(wi, wd, gran=2):
                # row-block granules so chunk-0 matmuls track arrival
                for o in range(0, NJT, gran):
                    nc.sync.dma_start(
                        w_all[:, wi * NJT + o: wi * NJT + o + gran, :],
                        wd.rearrange("(o p) c -> p o c", p=128)[:, o:o + gran, :],
                    )

            def wq(d):
                return w_all[:, d, :]

            def wk(d):
                return w_all[:, NJT + d, :]

            def wv(d):
                return w_all[:, 2 * NJT + d, :]

            # ---- upfront DMAs: one queue, in consumption order; weights
            # in 1MB halves ordered by when the PE stream consumes them.
            load_x(0, nc.sync)
            bvrow = cpool.tile([1, D], F16, tag="bvrow")
            nc.sync.dma_start(bvrow[:], bv_d[:])
            bqv = cpool.tile([128, NJT], F32, tag="bqv")
            nc.sync.dma_start(bqv[:], bq_d[:])
            load_w(0, wqT_d, gran=4)
            xTm = cpool.tile([128, NJT, M], F16, tag="xTm")
            nc.sync.dma_start(xTm[:], memT_d.rearrange("p (o m) -> p o m", m=M))
            load_w(1, wkT_d, gran=4)
            bkv = cpool.tile([128, NJT], F32, tag="bkv")
            nc.sync.dma_start(bkv[:], bk_d[:])
            mskv = cpool.tile([128, CPC * 3], F32, tag="mskv")
            nc.sync.dma_start(mskv[:], msk_d[:])
            bvv = cpool.tile([128, NJT], F32, tag="bvv")
            nc.sync.dma_start(bvv[:], bvv_d[:])
            nc.sync.dma_start(
                w_all[:, 2 * NJT: 2 * NJT + 4, :],
                wvT_d.rearrange("(o p) c -> p o c", p=128)[:, 0:4, :],
            )
            load_x(1, nc.sync)
            nc.sync.dma_start(
                w_all[:, 2 * NJT + 4: 2 * NJT + 8, :],
                wvT_d.rearrange("(o p) c -> p o c", p=128)[:, 4:8, :],
            )
            load_x(2, nc.sync)
            load_x(3, nc.sync)

            ones1 = cpool.tile([1, 128], F16, tag="ones1")
            nc.vector.memset(ones1[:], 1.0)
            bvb = cpool.tile([128, D], F32, tag="bvb")

            # tiny dep-free matmul: starts the PE p-state ramp clock so the
            # projections hit full clock as soon as their weights land; the
            # bvb ones-matmuls (gated only on the tiny bvrow DMA) keep the
            # PE's idle gaps under the ~3us p-state reset threshold
            ps_warm = pp_pool.tile([128, 512], F32, tag="pp")
            nc.tensor.matmul(
                ps_warm[:16, :16], ones1[:, :16], ones1[:, :16],
                start=True, stop=True,
            )
            x0d0 = x_tiles[0][0]
            nc.tensor.matmul(
                ps_warm[:16, 16:32], x0d0[:16, :16], x0d0[:16, :16],
                start=True, stop=True,
            )
            for half in range(2):
                ps_b = pp_pool.tile([128, 512], F32, tag="pp")
                nc.tensor.matmul(
                    ps_b[:], ones1[:], bvrow[:, half * 512:(half + 1) * 512],
                    start=True, stop=True,
                )
                nc.vector.tensor_copy(
                    bvb[:, half * 512:(half + 1) * 512], ps_b[:]
                )

            # ---- memory tokens (emitted inside chunk 0's flow) ----
            memp = {}

            def emit_ktm():
                # block-diagonal mem-K: [128, jt, 64]; even head rows 0:64 ->
                # cols 0:16, odd head rows 64:128 -> cols 32:48, rest zero
                kTm3 = cpool.tile([128, NJT, 64], F16, tag="kTm3", name="kTm3")
                nc.gpsimd.memset(kTm3[:], 0.0)
                ps_k = pp_pool.tile([128, 512], F32, tag="pp", name="ps_ktm")
                for jt in range(NJT):
                    for d in range(NJT):
                        nc.tensor.matmul(
                            ps_k[:, jt * M:(jt + 1) * M],
                            wk(d)[:, jt * 128:(jt + 1) * 128],
                            xTm[:, d, :],
                            start=(d == 0), stop=(d == NJT - 1),
                            skip_group_check=True,
                        )
                for u in range(2):
                    nc.vector.tensor_tensor(
                        kTm3[64 * u:64 * (u + 1), :, 32 * u:32 * u + 16],
                        ps_k[64 * u:64 * (u + 1), :NJT * M].rearrange(
                            "p (j m) -> p j m", m=M
                        ),
                        bkv[64 * u:64 * (u + 1), :][:, :, None].to_broadcast(
                            (64, NJT, M)
                        ),
                        mybir.AluOpType.add,
                    )
                memp["kTm3"] = kTm3

            def emit_vm():
                # mem-V computed feature-major in one cheap psum fill, then
                # token-major via an xbar DMA-transpose of the m-padded
                # [128, jt, 128] layout (dst[p, jt, f] = src[f, jt*128+p]);
                # replicated at partition bases 0/32/64/96 for the
                # 32-aligned mem-PV stationaries
                vm = cpool.tile([128, M, VW], F16, tag="vm", name="vm")
                nc.vector.memset(vm[:M, :, HD:HD + 1], 1.0)
                vmT = cpool.tile([128, NJT, 128], F16, tag="vmT", name="vmT")
                nc.gpsimd.memset(vmT[:], 0.0)
                ps_t = pp_pool.tile([128, 512], F32, tag="pp", name="ps_vm")
                for jt in range(NJT):
                    for d in range(NJT):
                        nc.tensor.matmul(
                            ps_t[:, jt * M:(jt + 1) * M],
                            wv(d)[:, jt * 128:(jt + 1) * 128],
                            xTm[:, d, :],
                            start=(d == 0), stop=(d == NJT - 1),
                            skip_group_check=True,
                        )
                nc.vector.tensor_tensor(
                    vmT[:, :, :M],
                    ps_t[:, :NJT * M].rearrange("p (j m) -> p j m", m=M),
                    bvv[:, :, None].to_broadcast((128, NJT, M)),
                    mybir.AluOpType.add,
                )
                vmB = cpool.tile([128, NJT, 128], F16, tag="vmB", name="vmB")
                nc.sync.dma_start_transpose(
                    vmB[:], vmT[:].rearrange("p j m -> p (j m)")
                )
                nc.vector.tensor_copy(
                    vm[:M, :, :HD].rearrange("m (j u) f -> m j u f", u=2),
                    vmB[:M, :, :].rearrange("m j (u f) -> m j u f", u=2),
                )
                for rb in range(1, 4):
                    nc.sync.dma_start(vm[32 * rb:32 * rb + M, :, :], vm[:M, :, :])
                memp["vm"] = vm

            # ---- per-chunk phases as generators; the main loop weaves
            # chunk ci's attention with chunk ci+1's projections so the PE
            # always has projection matmuls to stream while Act runs Exps.
            state = {}

            def proj_steps(ci):
                xT = x_tiles.pop(ci)
                qT, kT = [], []
                for which, wfn, bias, lst, tg in (
                    (0, wq, bqv, qT, "qT"),
                    (1, wk, bkv, kT, "kT"),
                ):
                    # pp-buf pairs of jt-pair fills, split at the 1MB
                    # weight-DMA boundary (d 0-3 | 4-7) so chunk 0's
                    # matmuls track weight arrival
                    for hw in range(2):
                        pss = []
                        for j2 in range(2):
                            jp = 2 * hw + j2
                            ps_q = pp_pool.tile(
                                [128, 512], F32, tag="pp", name="ps_q"
                            )
                            for u in range(2):
                                jt = 2 * jp + u
                                for d in range(NJT // 2):
                                    nc.tensor.matmul(
                                        ps_q[:, u * 256:(u + 1) * 256],
                                        wfn(d)[:, jt * 128:(jt + 1) * 128],
                                        xT[d][:],
                                        start=(u == 0 and d == 0),
                                        stop=False,
                                    )
                            pss.append(ps_q)
                        for j2 in range(2):
                            jp = 2 * hw + j2
                            ps_q = pss[j2]
                            for u in range(2):
                                jt = 2 * jp + u
                                for d in range(NJT // 2, NJT):
                                    nc.tensor.matmul(
                                        ps_q[:, u * 256:(u + 1) * 256],
                                        wfn(d)[:, jt * 128:(jt + 1) * 128],
                                        xT[d][:],
                                        start=False,
                                        stop=(u == 1 and d == NJT - 1),
                                    )
                            pair_t = qkpool.tile(
                                [128, 2, W], F16, tag=tg, name=f"pair_{tg}"
                            )
                            nc.vector.tensor_tensor(
                                pair_t[:],
                                ps_q[:].rearrange("p (u t) -> p u t", u=2),
                                bias[:, 2 * jp:2 * jp + 2][:, :, None]
                                .to_broadcast((128, 2, W)),
                                mybir.AluOpType.add,
                            )
                            lst.append(pair_t)
                            yield
                    if ci == 0 and which == 1:
                        emit_ktm()
                        yield

                # V projection (token-major fp16, heads + ones col)
                v_sb = vpool.tile([128, 2, H, VW], F16, tag="v_sb")
                nc.gpsimd.memset(v_sb[:, :, :, HD:HD + 1], 1.0)
                for tt in range(2):
                    pss = []
                    for half in range(2):
                        ps_v = pp_pool.tile([128, 512], F32, tag="pp")
                        for d in range(NJT // 2):
                            nc.tensor.matmul(
                                ps_v[:], xT[d][:, tt * 128:(tt + 1) * 128],
                                wv(d)[:, half * 512:(half + 1) * 512],
                                start=(d == 0), stop=False,
                            )
                        pss.append(ps_v)
                    for half in range(2):
                        ps_v = pss[half]
                        for d in range(NJT // 2, NJT):
                            nc.tensor.matmul(
                                ps_v[:], xT[d][:, tt * 128:(tt + 1) * 128],
                                wv(d)[:, half * 512:(half + 1) * 512],
                                start=False, stop=(d == NJT - 1),
                            )
                        nc.vector.tensor_tensor(
                            v_sb[:, tt, half * 8:(half + 1) * 8, :HD],
                            ps_v[:].rearrange("p (h f) -> p h f", h=8),
                            bvb[:, half * 512:(half + 1) * 512].rearrange(
                                "p (h f) -> p h f", h=8
                            ),
                            mybir.AluOpType.add,
                        )
                        yield
                if ci == 0:
                    emit_vm()
                state[ci] = (
                    [qT[jt // 2][:, jt % 2, :] for jt in range(NJT)],
                    [kT[jt // 2][:, jt % 2, :] for jt in range(NJT)],
                    v_sb,
                )

            def attn_steps(ci):
                while ci not in state:
                    yield
                qT, kT, v_sb = state.pop(ci)
                kTm3, vm = memp["kTm3"], memp["vm"]

                # mem scores: all 16 heads in one 2-bank psum, one Exp.
                # Head pair jt lands at 32-aligned partition bases via the
                # block-diagonal stationary and tile_position cols.
                ps_m = ps_pool.tile([128, 1024], F32, tag="ps", name="ps_ms")
                for jt in range(NJT):
                    c0 = 64 * (jt % 2)
                    g = jt // 2
                    nc.tensor.matmul(
                        ps_m[c0:c0 + 64, g * 256:(g + 1) * 256],
                        kTm3[:, jt, :],
                        qT[jt][:],
                        start=True, stop=True,
                        tile_position=(0, c0),
                        skip_group_check=True,
                    )
                em = empool.tile([128, 4, 256], F16, tag="em", name="em")
                nc.scalar.activation(
                    em[:], ps_m[:].rearrange("p (g t) -> p g t", g=4),
                    mybir.ActivationFunctionType.Exp,
                    bias=mskv[:, ci * 3 + 2: ci * 3 + 3],
                )
                yield

                out_sb = opool.tile([128, 2, D], F16, tag="out_sb", name="out_sb")
                eloc = {}   # (hpq, yt) -> [128, 1024] fp16

                def fill(hpq, yt):
                    # local scoresT for 4 heads (one quad, one key half);
                    # both 64-row ab halves quadrant-packed; one Exp
                    ps_s = ps_pool.tile([128, 1024], F32, tag="ps", name="ps_s")
                    for ab in range(2):
                        p0 = 64 * ab
                        for u in range(2):
                            jt = 2 * hpq + u
                            nc.tensor.matmul(
                                ps_s[:, ab * 512 + u * 256:
                                     ab * 512 + (u + 1) * 256],
                                kT[jt][p0:p0 + 64, yt * 128:(yt + 1) * 128],
                                qT[jt][p0:p0 + 64, :],
                                start=(u == 0), stop=(u == 1),
                                tile_position=(p0, 0),
                                skip_group_check=True,
                            )
                    e_t = epool.tile([128, 1024], F16, tag="exps")
                    nc.scalar.activation(
                        e_t[:], ps_s[:],
                        mybir.ActivationFunctionType.Exp,
                        bias=mskv[:, ci * 3 + yt: ci * 3 + yt + 1],
                    )
                    eloc[(hpq, yt)] = e_t

                def unit(hp):
                    # PV for head pair hp: one psum bank, 4 blocks (h, xb)
                    # of 65 (64 hd + denom); strided recip + broadcast mult
                    ps_o = po_pool.tile([128, 4 * VW], F32, tag="po")
                    for ab in range(2):
                        h = 2 * hp + ab
                        e0 = eloc[(hp // 2, 0)]
                        e1 = eloc[(hp // 2, 1)]
                        base, g = _mem_slot(h)
                        for xb in range(2):
                            o = (ab * 2 + xb) * VW
                            xs = (h % 2) * 512 + (hp % 2) * 256 + xb * 128
                            nc.tensor.matmul(
                                ps_o[:, o:o + VW],
                                e0[:, xs:xs + 128], v_sb[:, 0, h, :],
                                start=True, stop=False,
                            )
                            nc.tensor.matmul(
                                ps_o[:, o:o + VW],
                                e1[:, xs:xs + 128], v_sb[:, 1, h, :],
                                start=False, stop=False,
                            )
                            nc.tensor.matmul(
                                ps_o[:, o:o + VW],
                                em[base:base + M, g, xb * 128:(xb + 1) * 128],
                                vm[base:base + M, h, :],
                                start=False, stop=True,
                                tile_position=(base, 0),
                            )
                    rec = rpool.tile([128, 4], F32, tag="rec", name="rec")
                    nc.vector.reciprocal(
                        rec[:].rearrange("p (k o) -> p k o", o=1),
                        ps_o[:].rearrange("p (k w) -> p k w", w=VW)[
                            :, :, HD:HD + 1
                        ],
                    )
                    nc.vector.tensor_tensor(
                        out_sb[:, :, 2 * hp * HD:(2 * hp + 2) * HD].rearrange(
                            "p x (a f) -> p a x f", a=2
                        ),
                        ps_o[:].rearrange("p (a x w) -> p a x w", a=2, x=2)[
                            :, :, :, :HD
                        ],
                        rec[:].rearrange("p (a x) -> p a x", a=2)[
                            :, :, :, None
                        ].to_broadcast((128, 2, 2, HD)),
                        mybir.AluOpType.mult,
                    )

                def out_dma(qtr):
                    nc.sync.dma_start(
                        out_d.rearrange("(x p) c -> p x c", p=128)[
                            :, 2 * ci:2 * ci + 2, qtr * 256:(qtr + 1) * 256
                        ],
                        out_sb[:, :, qtr * 256:(qtr + 1) * 256],
                    )

                # fills run two head-pairs ahead of PV units so the Exp
                # latency hides behind interleaved projection matmuls
                fill(0, 0); yield
                fill(0, 1); yield
                fill(1, 0); yield
                fill(1, 1); yield
                unit(0); yield
                unit(1); out_dma(0); yield
                fill(2, 0); yield
                fill(2, 1); yield
                unit(2); yield
                unit(3); out_dma(1); yield
                fill(3, 0); yield
                fill(3, 1); yield
                unit(4); yield
                unit(5); out_dma(2); yield
                unit(6); yield
                unit(7); out_dma(3)

            def drain(*gens):
                gens = [g for g in gens if g is not None]
                while gens:
                    nxt = []
                    for g in gens:
                        try:
                            next(g)
                            nxt.append(g)
                        except StopIteration:
                            pass
                    gens = nxt

            # 3-way weave: chunk ci's attention runs with chunk ci+1's
            # projections, and attn(ci+1) joins early (it self-waits on
            # its state) so the attention tail always has matmul filler
            attns_g = [attn_steps(ci) for ci in range(CPC)]
            drain(proj_steps(0))
            for ci in range(CPC):
                gens = [attns_g[ci]]
                must = {id(attns_g[ci])}
                if ci + 1 < CPC:
                    pj = proj_steps(ci + 1)
                    gens = [pj, attns_g[ci], attns_g[ci + 1]]
                    must.add(id(pj))
                while must:
                    for g in list(gens):
                        try:
                            next(g)
                        except StopIteration:
                            gens.remove(g)
                            must.discard(id(g))

    nc.compile()
    return nc


_NC_CACHE = None


def kernel(hidden_states, attention_mask, self_memory, Wq, bq, Wk, bk, Wv, bv):
    global _NC_CACHE, LAST_RESULTS
    hidden_states = np.asarray(np.asarray(hidden_states), np.float32)
    attention_mask = np.asarray(np.asarray(attention_mask), np.float32)
    self_memory = np.asarray(np.asarray(self_memory), np.float32)
    wqT = np.ascontiguousarray(
        (np.asarray(Wq, np.float32).T * 0.125).astype(np.float16)
    )
    wkT = np.ascontiguousarray(np.asarray(Wk, np.float32).T.astype(np.float16))
    wvT = np.ascontiguousarray(np.asarray(Wv, np.float32).T.astype(np.float16))
    bqv = np.ascontiguousarray(
        np.asarray(bq, np.float32).reshape(NJT, 128).T * 0.125
    )
    bkv = np.ascontiguousarray(np.asarray(bk, np.float32).reshape(NJT, 128).T)
    bvrow = np.asarray(bv, np.float32).astype(np.float16).reshape(1, D)
    bvv = np.ascontiguousarray(np.asarray(bv, np.float32).reshape(NJT, 128).T)

    # additive mask along the key axis, per (b, c): [yt0 | yt1 | memory]
    am = attention_mask.reshape(B, C, W)
    chunk_has_valid = (am == 0.0).sum(axis=2) > 0
    mem_mask = np.where(chunk_has_valid, 0.0, -10000.0).astype(np.float32)

    if _NC_CACHE is None:
        _NC_CACHE = _build_kernel()
    nc = _NC_CACHE

    x16 = hidden_states.astype(np.float16)
    mem16 = self_memory.astype(np.float16)

    in_maps = []
    for core in range(N_CORES):
        b = core // (N_CORES // B)
        c0 = (core % (N_CORES // B)) * CPC
        mvT = np.zeros((128, CPC * 3), np.float32)
        for ci in range(CPC):
            mvT[:, ci * 3 + 0] = am[b, c0 + ci, 0:128]
            mvT[:, ci * 3 + 1] = am[b, c0 + ci, 128:256]
            mvT[:, ci * 3 + 2] = mem_mask[b, c0 + ci]
        # feature-major pre-transposed x: [128, NJT, TPC]
        xT = np.ascontiguousarray(
            x16[b, c0 * W:(c0 + CPC) * W, :]
            .T.reshape(NJT, 128, TPC).transpose(1, 0, 2)
        ).reshape(128, NJT * TPC)
        memT = np.ascontiguousarray(
            mem16[b].T.reshape(NJT, 128, M).transpose(1, 0, 2)
        ).reshape(128, NJT * M)
        in_maps.append(
            {
                "xT": xT,
                "memT": memT,
                "wqT": wqT,
                "wkT": wkT,
                "wvT": wvT,
                "bqv": bqv,
                "bkv": bkv,
                "bvrow": bvrow,
                "bvv": bvv,
                "maskvT": mvT,
            }
        )

    res = run_bass_kernel_spmd(nc, in_maps, list(range(N_CORES)), trace=TRACE)
    LAST_RESULTS = res

    out = np.empty((B, S, D), np.float32)
    for core in range(N_CORES):
        b = core // (N_CORES // B)
        c0 = (core % (N_CORES // B)) * CPC
        out[b, c0 * W:(c0 + CPC) * W, :] = res.results[core]["out"].astype(
            np.float32
        )
    return out


# revision 81
# speedup vs baseline: 1.1838x; 1.0007x over previous
"""Bass/Trainium2 kernel for chunked local attention with memory tokens
(BertSelfAttention variant). Self-contained: hardcodes all shapes.

Sharding: 8 cores, each handles 4 of the 32 (batch, chunk) pairs.
  core i -> b = i // 4, chunks 4*(i % 4) .. 4*(i % 4) + 3
No collectives; weights replicated per core; host scatters/gathers.

Per-core device computation (PE operands fp16, accumulation fp32):
  - xT [128, 8, 1024] arrives pre-transposed from host (feature-major),
    sliced per chunk; no device transposes
  - qT[jt] = x@(Wq/8) + bq/8 (feature-major)   kT[jt] = x@Wk + bk
  - v[t, yt, h, 65] token-major with a ones column at 64 that makes the
    PV matmul emit softmax denominators
  - memory-token K is materialized block-diagonally (kTm3 [128, jt, 64]:
    rows 0:64 x cols 0:16 = even head, rows 64:128 x cols 32:48 = odd) so
    one 128-contraction matmul scores 2 heads' mem keys at 32-aligned
    psum partitions; all 16 heads' mem scores share one 2-bank psum and
    a single Exp
  - local scoresT per (head-quad, yt) fill a 2-bank [128, 1024] psum
    (both 64-row ab halves, quadrant-packed) -> one Exp per fill
  - probs = Exp(scoresT + mask[y]) in fp16; additive key mask rides the
    ACT bias operand (per-partition scalar)
  - out_unnorm[x, 4*(hd|den)] per head-pair in one psum bank; strided
    reciprocal + one broadcast multiply normalize and emit fp16 out
Output fp16 on device, upcast to fp32 on host.

Scheduling: generator-based software pipelining weaves chunk ci's
attention with chunk ci+1's projections (and lets attn(ci+1) join as
soon as its projections land) so the PE streams matmuls while ACT runs
the Exps; projection fills split at the 1MB weight-DMA boundary so
chunk 0 tracks weight arrival; tiny warm matmuls keep the PE p-state
ramp clock alive across startup DMA waits; upfront DMAs are ordered by
first consumption on a single queue.
"""

import sys

sys.path.insert(0, "/opt/trn_rl_repo")

import numpy as np

import concourse.tile as tile
from concourse import bacc, mybir
from concourse.bass_utils import run_bass_kernel_spmd

F32 = mybir.dt.float32
F16 = mybir.dt.float16

B, S, D = 2, 4096, 1024
H, HD = 16, 64
W = 256            # attention window (chunk length)
C = S // W         # 16 chunks
M = 16             # memory tokens
N_CORES = 8
CPC = C * B // N_CORES  # 4 chunks per core
TPC = CPC * W           # 1024 chunk tokens per core
NJT = D // 128          # 8 feature tiles
VW = HD + 1             # v width (64 hd + ones col)

TRACE = False
LAST_RESULTS = None


def _mem_slot(h):
    """(partition base, free block) of head h's mem scores / probs."""
    jt, u = h // 2, h % 2
    return 64 * (jt % 2) + 32 * u, jt // 2


def _build_kernel():
    nc = bacc.Bacc(None, target_bir_lowering=False)

    xT_d = nc.declare_dram_parameter("xT", [128, NJT * TPC], F16, isOutput=False)
    memT_d = nc.declare_dram_parameter("memT", [128, NJT * M], F16, isOutput=False)
    wqT_d = nc.declare_dram_parameter("wqT", [D, D], F16, isOutput=False)
    wkT_d = nc.declare_dram_parameter("wkT", [D, D], F16, isOutput=False)
    wvT_d = nc.declare_dram_parameter("wvT", [D, D], F16, isOutput=False)
    bq_d = nc.declare_dram_parameter("bqv", [128, NJT], F32, isOutput=False)
    bk_d = nc.declare_dram_parameter("bkv", [128, NJT], F32, isOutput=False)
    bv_d = nc.declare_dram_parameter("bvrow", [1, D], F16, isOutput=False)
    bvv_d = nc.declare_dram_parameter("bvv", [128, NJT], F32, isOutput=False)
    msk_d = nc.declare_dram_parameter("maskvT", [128, CPC * 3], F32, isOutput=False)
    out_d = nc.declare_dram_parameter("out", [TPC, D], F16, isOutput=True)

    with tile.TileContext(nc) as tc:
        with (
            tc.tile_pool(name="const", bufs=1) as cpool,
            tc.tile_pool(name="wpool", bufs=1) as wpool,
            tc.tile_pool(name="xtpool", bufs=4) as xtpool,
            tc.tile_pool(name="qkpool", bufs=16) as qkpool,
            tc.tile_pool(name="vpool", bufs=4) as vpool,
            tc.tile_pool(name="epool", bufs=16) as epool,
            tc.tile_pool(name="empool", bufs=3) as empool,
            tc.tile_pool(name="opool", bufs=4) as opool,
            tc.tile_pool(name="rpool", bufs=4) as rpool,
            tc.tile_pool(name="pp", bufs=2, space="PSUM") as pp_pool,
            tc.tile_pool(name="ps", bufs=2, space="PSUM") as ps_pool,
            tc.tile_pool(name="po", bufs=2, space="PSUM") as po_pool,
        ):
            x_tiles = {}

            def load_x(ci, eng, split=False):
                x_t = xtpool.tile([128, NJT, W], F16, tag="xT", name="xT")
                xs = xT_d.rearrange("p (o t) -> p o t", t=TPC)[
                    :, :, ci * W:(ci + 1) * W
                ]
                if split:
                    eng.dma_start(x_t[:, 0:4, :], xs[:, 0:4, :])
                    eng.dma_start(x_t[:, 4:8, :], xs[:, 4:8, :])
                else:
                    eng.dma_start(x_t[:], xs)
                x_tiles[ci] = [x_t[:, d, :] for d in range(NJT)]

            w_all = wpool.tile([128, 3 * NJT, D], F16, tag="w_all")

            def load_w(wi, wd, gran=2):
                # row-block granules so chunk-0 matmuls track arrival
                for o in range(0, NJT, gran):
                    nc.sync.dma_start(
                        w_all[:, wi * NJT + o: wi * NJT + o + gran, :],
                        wd.rearrange("(o p) c -> p o c", p=128)[:, o:o + gran, :],
                    )

            def wq(d):
                return w_all[:, d, :]

            def wk(d):
                return w_all[:, NJT + d, :]

            def wv(d):
                return w_all[:, 2 * NJT + d, :]

            # ---- upfront DMAs: one queue, in consumption order; weights
            # in 1MB halves ordered by when the PE stream consumes them.
            load_x(0, nc.sync)
            bvrow = cpool.tile([1, D], F16, tag="bvrow")
            nc.sync.dma_start(bvrow[:], bv_d[:])
            bqv = cpool.tile([128, NJT], F32, tag="bqv")
            nc.sync.dma_start(bqv[:], bq_d[:])
            load_w(0, wqT_d, gran=4)
            xTm = cpool.tile([128, NJT, M], F16, tag="xTm")
            nc.sync.dma_start(xTm[:], memT_d.rearrange("p (o m) -> p o m", m=M))
            load_w(1, wkT_d, gran=4)
            bkv = cpool.tile([128, NJT], F32, tag="bkv")
            nc.sync.dma_start(bkv[:], bk_d[:])
            mskv = cpool.tile([128, CPC * 3], F32, tag="mskv")
            nc.sync.dma_start(mskv[:], msk_d[:])
            bvv = cpool.tile([128, NJT], F32, tag="bvv")
            nc.sync.dma_start(bvv[:], bvv_d[:])
            nc.sync.dma_start(
                w_all[:, 2 * NJT: 2 * NJT + 4, :],
                wvT_d.rearrange("(o p) c -> p o c", p=128)[:, 0:4, :],
            )
            load_x(1, nc.sync)
            nc.sync.dma_start(
                w_all[:, 2 * NJT + 4: 2 * NJT + 8, :],
                wvT_d.rearrange("(o p) c -> p o c", p=128)[:, 4:8, :],
            )
            load_x(2, nc.sync)
            load_x(3, nc.sync)

            ones1 = cpool.tile([1, 128], F16, tag="ones1")
            nc.vector.memset(ones1[:], 1.0)
            bvb = cpool.tile([128, D], F32, tag="bvb")

            # tiny dep-free matmul: starts the PE p-state ramp clock so the
            # projections hit full clock as soon as their weights land; the
            # bvb ones-matmuls (gated only on the tiny bvrow DMA) keep the
            # PE's idle gaps under the ~3us p-state reset threshold
            ps_warm = pp_pool.tile([128, 512], F32, tag="pp")
            nc.tensor.matmul(
                ps_warm[:16, :16], ones1[:, :16], ones1[:, :16],
                start=True, stop=True,
            )
            x0d0 = x_tiles[0][0]
            nc.tensor.matmul(
                ps_warm[:16, 16:32], x0d0[:16, :16], x0d0[:16, :16],
                start=True, stop=True,
            )
            for half in range(2):
                ps_b = pp_pool.tile([128, 512], F32, tag="pp")
                nc.tensor.matmul(
                    ps_b[:], ones1[:], bvrow[:, half * 512:(half + 1) * 512],
                    start=True, stop=True,
                )
                nc.vector.tensor_copy(
                    bvb[:, half * 512:(half + 1) * 512], ps_b[:]
                )

            # ---- memory tokens (emitted inside chunk 0's flow) ----
            memp = {}

            def emit_ktm():
                # block-diagonal mem-K: [128, jt, 64]; even head rows 0:64 ->
                # cols 0:16, odd head rows 64:128 -> cols 32:48, rest zero
                kTm3 = cpool.tile([128, NJT, 64], F16, tag="kTm3", name="kTm3")
                nc.gpsimd.memset(kTm3[:], 0.0)
                ps_k = pp_pool.tile([128, 512], F32, tag="pp", name="ps_ktm")
                for jt in range(NJT):
                    for d in range(NJT):
                        nc.tensor.matmul(
                            ps_k[:, jt * M:(jt + 1) * M],
                            wk(d)[:, jt * 128:(jt + 1) * 128],
                            xTm[:, d, :],
                            start=(d == 0), stop=(d == NJT - 1),
                            skip_group_check=True,
                        )
                for u in range(2):
                    nc.vector.tensor_tensor(
                        kTm3[64 * u:64 * (u + 1), :, 32 * u:32 * u + 16],
                        ps_k[64 * u:64 * (u + 1), :NJT * M].rearrange(
                            "p (j m) -> p j m", m=M
                        ),
                        bkv[64 * u:64 * (u + 1), :][:, :, None].to_broadcast(
                            (64, NJT, M)
                        ),
                        mybir.AluOpType.add,
                    )
                memp["kTm3"] = kTm3

            def emit_vm():
                # mem-V computed feature-major in one cheap psum fill, then
                # token-major via an xbar DMA-transpose of the m-padded
                # [128, jt, 128] layout (dst[p, jt, f] = src[f, jt*128+p]);
                # replicated at partition bases 0/32/64/96 for the
                # 32-aligned mem-PV stationaries
                vm = cpool.tile([128, M, VW], F16, tag="vm", name="vm")
                nc.vector.memset(vm[:M, :, HD:HD + 1], 1.0)
                vmT = cpool.tile([128, NJT, 128], F16, tag="vmT", name="vmT")
                nc.gpsimd.memset(vmT[:], 0.0)
                ps_t = pp_pool.tile([128, 512], F32, tag="pp", name="ps_vm")
                for jt in range(NJT):
                    for d in range(NJT):
                        nc.tensor.matmul(
                            ps_t[:, jt * M:(jt + 1) * M],
                            wv(d)[:, jt * 128:(jt + 1) * 128],
                            xTm[:, d, :],
                            start=(d == 0), stop=(d == NJT - 1),
                            skip_group_check=True,
                        )
                nc.vector.tensor_tensor(
                    vmT[:, :, :M],
                    ps_t[:, :NJT * M].rearrange("p (j m) -> p j m", m=M),
                    bvv[:, :, None].to_broadcast((128, NJT, M)),
                    mybir.AluOpType.add,
                )
                vmB = cpool.tile([128, NJT, 128], F16, tag="vmB", name="vmB")
                nc.sync.dma_start_transpose(
                    vmB[:], vmT[:].rearrange("p j m -> p (j m)")
                )
                nc.vector.tensor_copy(
                    vm[:M, :, :HD].rearrange("m (j u) f -> m j u f", u=2),
                    vmB[:M, :, :].rearrange("m j (u f) -> m j u f", u=2),
                )
                for rb in range(1, 4):
                    nc.sync.dma_start(vm[32 * rb:32 * rb + M, :, :], vm[:M, :, :])
                memp["vm"] = vm

            # ---- per-chunk phases as generators; the main loop weaves
            # chunk ci's attention with chunk ci+1's projections so the PE
            # always has projection matmuls to stream while Act runs Exps.
            state = {}

            def proj_steps(ci):
                xT = x_tiles.pop(ci)
                qT, kT = [], []
                for which, wfn, bias, lst, tg in (
                    (0, wq, bqv, qT, "qT"),
                    (1, wk, bkv, kT, "kT"),
                ):
                    # pp-buf pairs of jt-pair fills, split at the 1MB
                    # weight-DMA boundary (d 0-3 | 4-7) so chunk 0's
                    # matmuls track weight arrival
                    for hw in range(2):
                        pss = []
                        for j2 in range(2):
                            jp = 2 * hw + j2
                            ps_q = pp_pool.tile(
                                [128, 512], F32, tag="pp", name="ps_q"
                            )
                            for u in range(2):
                                jt = 2 * jp + u
                                for d in range(NJT // 2):
                                    nc.tensor.matmul(
                                        ps_q[:, u * 256:(u + 1) * 256],
                                        wfn(d)[:, jt * 128:(jt + 1) * 128],
                                        xT[d][:],
                                        start=(u == 0 and d == 0),
                                        stop=False,
                                    )
                            pss.append(ps_q)
                        for j2 in range(2):
                            jp = 2 * hw + j2
                            ps_q = pss[j2]
                            for u in range(2):
                                jt = 2 * jp + u
                                for d in range(NJT // 2, NJT):
                                    nc.tensor.matmul(
                                        ps_q[:, u * 256:(u + 1) * 256],
                                        wfn(d)[:, jt * 128:(jt + 1) * 128],
                                        xT[d][:],
                                        start=False,
                                        stop=(u == 1 and d == NJT - 1),
                                    )
                            pair_t = qkpool.tile(
                                [128, 2, W], F16, tag=tg, name=f"pair_{tg}"
                            )
                            nc.vector.tensor_tensor(
                                pair_t[:],
                                ps_q[:].rearrange("p (u t) -> p u t", u=2),
                                bias[:, 2 * jp:2 * jp + 2][:, :, None]
                                .to_broadcast((128, 2, W)),
                                mybir.AluOpType.add,
                            )
                            lst.append(pair_t)
                            yield
                    if ci == 0 and which == 1:
                        emit_ktm()
                        yield

                # V projection (token-major fp16, heads + ones col)
                v_sb = vpool.tile([128, 2, H, VW], F16, tag="v_sb")
                nc.gpsimd.memset(v_sb[:, :, :, HD:HD + 1], 1.0)
                for tt in range(2):
                    pss = []
                    for half in range(2):
                        ps_v = pp_pool.tile([128, 512], F32, tag="pp")
                        for d in range(NJT // 2):
                            nc.tensor.matmul(
                                ps_v[:], xT[d][:, tt * 128:(tt + 1) * 128],
                                wv(d)[:, half * 512:(half + 1) * 512],
                                start=(d == 0), stop=False,
                            )
                        pss.append(ps_v)
                    for half in range(2):
                        ps_v = pss[half]
                        for d in range(NJT // 2, NJT):
                            nc.tensor.matmul(
                                ps_v[:], xT[d][:, tt * 128:(tt + 1) * 128],
                                wv(d)[:, half * 512:(half + 1) * 512],
                                start=False, stop=(d == NJT - 1),
                            )
                        nc.vector.tensor_tensor(
                            v_sb[:, tt, half * 8:(half + 1) * 8, :HD],
                            ps_v[:].rearrange("p (h f) -> p h f", h=8),
                            bvb[:, half * 512:(half + 1) * 512].rearrange(
                                "p (h f) -> p h f", h=8
                            ),
                            mybir.AluOpType.add,
                        )
                        yield
                    if ci == 0 and tt == 0:
                        emit_vm()
                state[ci] = (
                    [qT[jt // 2][:, jt % 2, :] for jt in range(NJT)],
                    [kT[jt // 2][:, jt % 2, :] for jt in range(NJT)],
                    v_sb,
                )

            def attn_steps(ci):
                while ci not in state:
                    yield
                qT, kT, v_sb = state.pop(ci)
                kTm3, vm = memp["kTm3"], memp["vm"]

                # mem scores: all 16 heads in one 2-bank psum, one Exp.
                # Head pair jt lands at 32-aligned partition bases via the
                # block-diagonal stationary and tile_position cols.
                ps_m = ps_pool.tile([128, 1024], F32, tag="ps", name="ps_ms")
                for jt in range(NJT):
                    c0 = 64 * (jt % 2)
                    g = jt // 2
                    nc.tensor.matmul(
                        ps_m[c0:c0 + 64, g * 256:(g + 1) * 256],
                        kTm3[:, jt, :],
                        qT[jt][:],
                        start=True, stop=True,
                        tile_position=(0, c0),
                        skip_group_check=True,
                    )
                em = empool.tile([128, 4, 256], F16, tag="em", name="em")
                nc.scalar.activation(
                    em[:], ps_m[:].rearrange("p (g t) -> p g t", g=4),
                    mybir.ActivationFunctionType.Exp,
                    bias=mskv[:, ci * 3 + 2: ci * 3 + 3],
                )
                yield

                out_sb = opool.tile([128, 2, D], F16, tag="out_sb", name="out_sb")
                eloc = {}   # (hpq, yt) -> [128, 1024] fp16

                def fill(hpq, yt):
                    # local scoresT for 4 heads (one quad, one key half);
                    # both 64-row ab halves quadrant-packed; one Exp
                    ps_s = ps_pool.tile([128, 1024], F32, tag="ps", name="ps_s")
                    for ab in range(2):
                        p0 = 64 * ab
                        for u in range(2):
                            jt = 2 * hpq + u
                            nc.tensor.matmul(
                                ps_s[:, ab * 512 + u * 256:
                                     ab * 512 + (u + 1) * 256],
                                kT[jt][p0:p0 + 64, yt * 128:(yt + 1) * 128],
                                qT[jt][p0:p0 + 64, :],
                                start=(u == 0), stop=(u == 1),
                                tile_position=(p0, 0),
                                skip_group_check=True,
                            )
                    e_t = epool.tile([128, 1024], F16, tag="exps")
                    nc.scalar.activation(
                        e_t[:], ps_s[:],
                        mybir.ActivationFunctionType.Exp,
                        bias=mskv[:, ci * 3 + yt: ci * 3 + yt + 1],
                    )
                    eloc[(hpq, yt)] = e_t

                def unit(hp):
                    # PV for head pair hp: one psum bank, 4 blocks (h, xb)
                    # of 65 (64 hd + denom); strided recip + broadcast mult
                    ps_o = po_pool.tile([128, 4 * VW], F32, tag="po")
                    for ab in range(2):
                        h = 2 * hp + ab
                        e0 = eloc[(hp // 2, 0)]
                        e1 = eloc[(hp // 2, 1)]
                        base, g = _mem_slot(h)
                        for xb in range(2):
                            o = (ab * 2 + xb) * VW
                            xs = (h % 2) * 512 + (hp % 2) * 256 + xb * 128
                            nc.tensor.matmul(
                                ps_o[:, o:o + VW],
                                e0[:, xs:xs + 128], v_sb[:, 0, h, :],
                                start=True, stop=False,
                            )
                            nc.tensor.matmul(
                                ps_o[:, o:o + VW],
                                e1[:, xs:xs + 128], v_sb[:, 1, h, :],
                                start=False, stop=False,
                            )
                            nc.tensor.matmul(
                                ps_o[:, o:o + VW],
                                em[base:base + M, g, xb * 128:(xb + 1) * 128],
                                vm[base:base + M, h, :],
                                start=False, stop=True,
                                tile_position=(base, 0),
                            )
                    rec = rpool.tile([128, 4], F32, tag="rec", name="rec")
                    nc.vector.reciprocal(
                        rec[:].rearrange("p (k o) -> p k o", o=1),
                        ps_o[:].rearrange("p (k w) -> p k w", w=VW)[
                            :, :, HD:HD + 1
                        ],
                    )
                    nc.vector.tensor_tensor(
                        out_sb[:, :, 2 * hp * HD:(2 * hp + 2) * HD].rearrange(
                            "p x (a f) -> p a x f", a=2
                        ),
                        ps_o[:].rearrange("p (a x w) -> p a x w", a=2, x=2)[
                            :, :, :, :HD
                        ],
                        rec[:].rearrange("p (a x) -> p a x", a=2)[
                            :, :, :, None
                        ].to_broadcast((128, 2, 2, HD)),
                        mybir.AluOpType.mult,
                    )

                def out_dma(qtr):
                    nc.sync.dma_start(
                        out_d.rearrange("(x p) c -> p x c", p=128)[
                            :, 2 * ci:2 * ci + 2, qtr * 256:(qtr + 1) * 256
                        ],
                        out_sb[:, :, qtr * 256:(qtr + 1) * 256],
                    )

                # fills run two head-pairs ahead of PV units so the Exp
                # latency hides behind interleaved projection matmuls
                fill(0, 0); yield
                fill(0, 1); yield
                fill(1, 0); yield
                fill(1, 1); yield
                unit(0); yield
                unit(1); out_dma(0); yield
                fill(2, 0); yield
                fill(2, 1); yield
                unit(2); yield
                unit(3); out_dma(1); yield
                fill(3, 0); yield
                fill(3, 1); yield
                unit(4); yield
                unit(5); out_dma(2); yield
                unit(6); yield
                unit(7); out_dma(3)

            def drain(*gens):
                gens = [g for g in gens if g is not None]
                while gens:
                    nxt = []
                    for g in gens:
                        try:
                            next(g)
                            nxt.append(g)
                        except StopIteration:
                            pass
                    gens = nxt

            # 3-way weave: chunk ci's attention runs with chunk ci+1's
            # projections, and attn(ci+1) joins early (it self-waits on
            # its state) so the attention tail always has matmul filler
            attns_g = [attn_steps(ci) for ci in range(CPC)]
            drain(proj_steps(0))
            for ci in range(CPC):
                gens = [attns_g[ci]]
                must = {id(attns_g[ci])}
                if ci + 1 < CPC:
                    pj = proj_steps(ci + 1)
                    gens = [pj, attns_g[ci], attns_g[ci + 1]]
                    must.add(id(pj))
                while must:
                    for g in list(gens):
                        try:
                            next(g)
                        except StopIteration:
                            gens.remove(g)
                            must.discard(id(g))

    nc.compile()
    return nc


_NC_CACHE = None


def kernel(hidden_states, attention_mask, self_memory, Wq, bq, Wk, bk, Wv, bv):
    global _NC_CACHE, LAST_RESULTS
    hidden_states = np.asarray(np.asarray(hidden_states), np.float32)
    attention_mask = np.asarray(np.asarray(attention_mask), np.float32)
    self_memory = np.asarray(np.asarray(self_memory), np.float32)
    wqT = np.ascontiguousarray(
        (np.asarray(Wq, np.float32).T * 0.125).astype(np.float16)
    )
    wkT = np.ascontiguousarray(np.asarray(Wk, np.float32).T.astype(np.float16))
    wvT = np.ascontiguousarray(np.asarray(Wv, np.float32).T.astype(np.float16))
    bqv = np.ascontiguousarray(
        np.asarray(bq, np.float32).reshape(NJT, 128).T * 0.125
    )
    bkv = np.ascontiguousarray(np.asarray(bk, np.float32).reshape(NJT, 128).T)
    bvrow = np.asarray(bv, np.float32).astype(np.float16).reshape(1, D)
    bvv = np.ascontiguousarray(np.asarray(bv, np.float32).reshape(NJT, 128).T)

    # additive mask along the key axis, per (b, c): [yt0 | yt1 | memory]
    am = attention_mask.reshape(B, C, W)
    chunk_has_valid = (am == 0.0).sum(axis=2) > 0
    mem_mask = np.where(chunk_has_valid, 0.0, -10000.0).astype(np.float32)

    if _NC_CACHE is None:
        _NC_CACHE = _build_kernel()
    nc = _NC_CACHE

    x16 = hidden_states.astype(np.float16)
    mem16 = self_memory.astype(np.float16)

    in_maps = []
    for core in range(N_CORES):
        b = core // (N_CORES // B)
        c0 = (core % (N_CORES // B)) * CPC
        mvT = np.zeros((128, CPC * 3), np.float32)
        for ci in range(CPC):
            mvT[:, ci * 3 + 0] = am[b, c0 + ci, 0:128]
            mvT[:, ci * 3 + 1] = am[b, c0 + ci, 128:256]
            mvT[:, ci * 3 + 2] = mem_mask[b, c0 + ci]
        # feature-major pre-transposed x: [128, NJT, TPC]
        xT = np.ascontiguousarray(
            x16[b, c0 * W:(c0 + CPC) * W, :]
            .T.reshape(NJT, 128, TPC).transpose(1, 0, 2)
        ).reshape(128, NJT * TPC)
        memT = np.ascontiguousarray(
            mem16[b].T.reshape(NJT, 128, M).transpose(1, 0, 2)
        ).reshape(128, NJT * M)
        in_maps.append(
            {
                "xT": xT,
                "memT": memT,
                "wqT": wqT,
                "wkT": wkT,
                "wvT": wvT,
                "bqv": bqv,
                "bkv": bkv,
                "bvrow": bvrow,
                "bvv": bvv,
                "maskvT": mvT,
            }
        )

    res = run_bass_kernel_spmd(nc, in_maps, list(range(N_CORES)), trace=TRACE)
    LAST_RESULTS = res

    out = np.empty((B, S, D), np.float32)
    for core in range(N_CORES):
        b = core // (N_CORES // B)
        c0 = (core % (N_CORES // B)) * CPC
        out[b, c0 * W:(c0 + CPC) * W, :] = res.results[core]["out"].astype(
            np.float32
        )
    return out


# revision 89
# speedup vs baseline: 1.1942x; 1.0087x over previous
"""Bass/Trainium2 kernel for chunked local attention with memory tokens
(BertSelfAttention variant). Self-contained: hardcodes all shapes.

Sharding: 8 cores, each handles 4 of the 32 (batch, chunk) pairs.
  core i -> b = i // 4, chunks 4*(i % 4) .. 4*(i % 4) + 3
No collectives; weights replicated per core; host scatters/gathers.

Per-core device computation (PE operands fp16, accumulation fp32):
  - xT [128, 8, 1024] arrives pre-transposed from host (feature-major),
    sliced per chunk; no device transposes
  - qT[jt] = x@(Wq/8) + bq/8 (feature-major)   kT[jt] = x@Wk + bk
  - v[t, yt, h, 65] token-major with a ones column at 64 that makes the
    PV matmul emit softmax denominators
  - memory-token K is materialized block-diagonally (kTm3 [128, jt, 64]:
    rows 0:64 x cols 0:16 = even head, rows 64:128 x cols 32:48 = odd) so
    one 128-contraction matmul scores 2 heads' mem keys at 32-aligned
    psum partitions; all 16 heads' mem scores share one 2-bank psum and
    a single Exp
  - local scoresT per (head-quad, yt) fill a 2-bank [128, 1024] psum
    (both 64-row ab halves, quadrant-packed) -> one Exp per fill
  - probs = Exp(scoresT + mask[y]) in fp16; additive key mask rides the
    ACT bias operand (per-partition scalar)
  - out_unnorm[x, 4*(hd|den)] per head-pair in one psum bank; strided
    reciprocal + one broadcast multiply normalize and emit fp16 out
Output fp16 on device, upcast to fp32 on host.

Scheduling: generator-based software pipelining weaves chunk ci's
attention with chunk ci+1's projections (and lets attn(ci+1) join as
soon as its projections land) so the PE streams matmuls while ACT runs
the Exps; projection fills split at the 1MB weight-DMA boundary so
chunk 0 tracks weight arrival; tiny warm matmuls keep the PE p-state
ramp clock alive across startup DMA waits; upfront DMAs are ordered by
first consumption on a single queue.
"""

import sys

sys.path.insert(0, "/opt/trn_rl_repo")

import numpy as np

import concourse.tile as tile
from concourse import bacc, mybir
from concourse.bass_utils import run_bass_kernel_spmd

F32 = mybir.dt.float32
F16 = mybir.dt.float16

B, S, D = 2, 4096, 1024
H, HD = 16, 64
W = 256            # attention window (chunk length)
C = S // W         # 16 chunks
M = 16             # memory tokens
N_CORES = 8
CPC = C * B // N_CORES  # 4 chunks per core
TPC = CPC * W           # 1024 chunk tokens per core
NJT = D // 128          # 8 feature tiles
VW = HD + 1             # v width (64 hd + ones col)

TRACE = False
LAST_RESULTS = None


def _mem_slot(h):
    """(partition base, free block) of head h's mem scores / probs."""
    jt, u = h // 2, h % 2
    return 64 * (jt % 2) + 32 * u, jt // 2


def _build_kernel():
    nc = bacc.Bacc(None, target_bir_lowering=False)

    xT_d = nc.declare_dram_parameter("xT", [128, NJT * TPC], F16, isOutput=False)
    memT_d = nc.declare_dram_parameter("memT", [128, NJT * M], F16, isOutput=False)
    wqT_d = nc.declare_dram_parameter("wqT", [D, D], F16, isOutput=False)
    wkT_d = nc.declare_dram_parameter("wkT", [D, D], F16, isOutput=False)
    wvT_d = nc.declare_dram_parameter("wvT", [D, D], F16, isOutput=False)
    bq_d = nc.declare_dram_parameter("bqv", [128, NJT], F32, isOutput=False)
    bk_d = nc.declare_dram_parameter("bkv", [128, NJT], F32, isOutput=False)
    bv_d = nc.declare_dram_parameter("bvrow", [1, D], F16, isOutput=False)
    bvv_d = nc.declare_dram_parameter("bvv", [128, NJT], F32, isOutput=False)
    msk_d = nc.declare_dram_parameter("maskvT", [128, CPC * 3], F32, isOutput=False)
    out_d = nc.declare_dram_parameter("out", [TPC, D], F16, isOutput=True)

    with tile.TileContext(nc) as tc:
        with (
            tc.tile_pool(name="const", bufs=1) as cpool,
            tc.tile_pool(name="wpool", bufs=1) as wpool,
            tc.tile_pool(name="xtpool", bufs=4) as xtpool,
            tc.tile_pool(name="qkpool", bufs=16) as qkpool,
            tc.tile_pool(name="vpool", bufs=4) as vpool,
            tc.tile_pool(name="epool", bufs=16) as epool,
            tc.tile_pool(name="empool", bufs=3) as empool,
            tc.tile_pool(name="opool", bufs=4) as opool,
            tc.tile_pool(name="rpool", bufs=4) as rpool,
            tc.tile_pool(name="pp", bufs=2, space="PSUM") as pp_pool,
            tc.tile_pool(name="ps", bufs=2, space="PSUM") as ps_pool,
            tc.tile_pool(name="po", bufs=2, space="PSUM") as po_pool,
        ):
            x_tiles = {}

            def load_x(ci, eng, split=False):
                x_t = xtpool.tile([128, NJT, W], F16, tag="xT", name="xT")
                xs = xT_d.rearrange("p (o t) -> p o t", t=TPC)[
                    :, :, ci * W:(ci + 1) * W
                ]
                if split:
                    eng.dma_start(x_t[:, 0:4, :], xs[:, 0:4, :])
                    eng.dma_start(x_t[:, 4:8, :], xs[:, 4:8, :])
                else:
                    eng.dma_start(x_t[:], xs)
                x_tiles[ci] = [x_t[:, d, :] for d in range(NJT)]

            w_all = wpool.tile([128, 3 * NJT, D], F16, tag="w_all")

            def load_w(wi, wd, gran=2):
                # row-block granules so chunk-0 matmuls track arrival
                for o in range(0, NJT, gran):
                    nc.sync.dma_start(
                        w_all[:, wi * NJT + o: wi * NJT + o + gran, :],
                        wd.rearrange("(o p) c -> p o c", p=128)[:, o:o + gran, :],
                    )

            def wq(d):
                return w_all[:, d, :]

            def wk(d):
                return w_all[:, NJT + d, :]

            def wv(d):
                return w_all[:, 2 * NJT + d, :]

            # ---- upfront DMAs: one queue, in consumption order; weights
            # in 1MB halves ordered by when the PE stream consumes them.
            load_x(0, nc.sync)
            bvrow = cpool.tile([1, D], F16, tag="bvrow")
            nc.sync.dma_start(bvrow[:], bv_d[:])
            bqv = cpool.tile([128, NJT], F32, tag="bqv")
            nc.sync.dma_start(bqv[:], bq_d[:])
            load_w(0, wqT_d, gran=4)
            xTm = cpool.tile([128, NJT, M], F16, tag="xTm")
            nc.sync.dma_start(xTm[:], memT_d.rearrange("p (o m) -> p o m", m=M))
            load_w(1, wkT_d, gran=4)
            bkv = cpool.tile([128, NJT], F32, tag="bkv")
            nc.sync.dma_start(bkv[:], bk_d[:])
            mskv = cpool.tile([128, CPC * 3], F32, tag="mskv")
            nc.sync.dma_start(mskv[:], msk_d[:])
            bvv = cpool.tile([128, NJT], F32, tag="bvv")
            nc.sync.dma_start(bvv[:], bvv_d[:])
            nc.sync.dma_start(
                w_all[:, 2 * NJT: 2 * NJT + 4, :],
                wvT_d.rearrange("(o p) c -> p o c", p=128)[:, 0:4, :],
            )
            load_x(1, nc.sync)
            nc.sync.dma_start(
                w_all[:, 2 * NJT + 4: 2 * NJT + 8, :],
                wvT_d.rearrange("(o p) c -> p o c", p=128)[:, 4:8, :],
            )
            load_x(2, nc.sync)
            load_x(3, nc.sync)

            ones1 = cpool.tile([1, 128], F16, tag="ones1")
            nc.vector.memset(ones1[:], 1.0)
            bvb = cpool.tile([128, D], F32, tag="bvb")

            # tiny dep-free matmul: starts the PE p-state ramp clock so the
            # projections hit full clock as soon as their weights land; the
            # bvb ones-matmuls (gated only on the tiny bvrow DMA) keep the
            # PE's idle gaps under the ~3us p-state reset threshold
            ps_warm = pp_pool.tile([128, 512], F32, tag="pp")
            nc.tensor.matmul(
                ps_warm[:16, :16], ones1[:, :16], ones1[:, :16],
                start=True, stop=True,
            )
            x0d0 = x_tiles[0][0]
            nc.tensor.matmul(
                ps_warm[:16, 16:32], x0d0[:16, :16], x0d0[:16, :16],
                start=True, stop=True,
            )
            for half in range(2):
                ps_b = pp_pool.tile([128, 512], F32, tag="pp")
                nc.tensor.matmul(
                    ps_b[:], ones1[:], bvrow[:, half * 512:(half + 1) * 512],
                    start=True, stop=True,
                )
                nc.vector.tensor_copy(
                    bvb[:, half * 512:(half + 1) * 512], ps_b[:]
                )

            # ---- memory tokens (emitted inside chunk 0's flow) ----
            memp = {}

            def emit_ktm():
                # block-diagonal mem-K: [128, jt, 64]; even head rows 0:64 ->
                # cols 0:16, odd head rows 64:128 -> cols 32:48, rest zero
                kTm3 = cpool.tile([128, NJT, 64], F16, tag="kTm3", name="kTm3")
                nc.gpsimd.memset(kTm3[:], 0.0)
                ps_k = pp_pool.tile([128, 512], F32, tag="pp", name="ps_ktm")
                for jt in range(NJT):
                    for d in range(NJT):
                        nc.tensor.matmul(
                            ps_k[:, jt * M:(jt + 1) * M],
                            wk(d)[:, jt * 128:(jt + 1) * 128],
                            xTm[:, d, :],
                            start=(d == 0), stop=(d == NJT - 1),
                            skip_group_check=True,
                        )
                for u in range(2):
                    nc.vector.tensor_tensor(
                        kTm3[64 * u:64 * (u + 1), :, 32 * u:32 * u + 16],
                        ps_k[64 * u:64 * (u + 1), :NJT * M].rearrange(
                            "p (j m) -> p j m", m=M
                        ),
                        bkv[64 * u:64 * (u + 1), :][:, :, None].to_broadcast(
                            (64, NJT, M)
                        ),
                        mybir.AluOpType.add,
                    )
                memp["kTm3"] = kTm3

            def emit_vm():
                # mem-V computed feature-major in one cheap psum fill, then
                # token-major via an xbar DMA-transpose of the m-padded
                # [128, jt, 128] layout (dst[p, jt, f] = src[f, jt*128+p]);
                # replicated at partition bases 0/32/64/96 for the
                # 32-aligned mem-PV stationaries
                vm = cpool.tile([128, M, VW], F16, tag="vm", name="vm")
                nc.vector.memset(vm[:M, :, HD:HD + 1], 1.0)
                vmT = cpool.tile([128, NJT, 128], F16, tag="vmT", name="vmT")
                nc.gpsimd.memset(vmT[:], 0.0)
                ps_t = pp_pool.tile([128, 512], F32, tag="pp", name="ps_vm")
                for jt in range(NJT):
                    for d in range(NJT):
                        nc.tensor.matmul(
                            ps_t[:, jt * M:(jt + 1) * M],
                            wv(d)[:, jt * 128:(jt + 1) * 128],
                            xTm[:, d, :],
                            start=(d == 0), stop=(d == NJT - 1),
                            skip_group_check=True,
                        )
                nc.vector.tensor_tensor(
                    vmT[:, :, :M],
                    ps_t[:, :NJT * M].rearrange("p (j m) -> p j m", m=M),
                    bvv[:, :, None].to_broadcast((128, NJT, M)),
                    mybir.AluOpType.add,
                )
                vmB = cpool.tile([128, NJT, 128], F16, tag="vmB", name="vmB")
                nc.sync.dma_start_transpose(
                    vmB[:], vmT[:].rearrange("p j m -> p (j m)")
                )
                nc.vector.tensor_copy(
                    vm[:M, :, :HD].rearrange("m (j u) f -> m j u f", u=2),
                    vmB[:M, :, :].rearrange("m j (u f) -> m j u f", u=2),
                )
                for rb in range(1, 4):
                    nc.sync.dma_start(vm[32 * rb:32 * rb + M, :, :], vm[:M, :, :])
                memp["vm"] = vm

            # ---- per-chunk phases as generators; the main loop weaves
            # chunk ci's attention with chunk ci+1's projections so the PE
            # always has projection matmuls to stream while Act runs Exps.
            # The handoff is split Q/K vs V: score fills only need Q/K, so
            # they start a V-phase early and V matmuls fill attention tails.
            state_qk = {}
            state_v = {}

            def proj_steps(ci):
                xT = x_tiles.pop(ci)
                qT, kT = [], []
                for which, wfn, bias, lst, tg in (
                    (0, wq, bqv, qT, "qT"),
                    (1, wk, bkv, kT, "kT"),
                ):
                    # pp-buf pairs of jt-pair fills, split at the 1MB
                    # weight-DMA boundary (d 0-3 | 4-7) so chunk 0's
                    # matmuls track weight arrival
                    for hw in range(2):
                        pss = []
                        for j2 in range(2):
                            jp = 2 * hw + j2
                            ps_q = pp_pool.tile(
                                [128, 512], F32, tag="pp", name="ps_q"
                            )
                            for u in range(2):
                                jt = 2 * jp + u
                                for d in range(NJT // 2):
                                    nc.tensor.matmul(
                                        ps_q[:, u * 256:(u + 1) * 256],
                                        wfn(d)[:, jt * 128:(jt + 1) * 128],
                                        xT[d][:],
                                        start=(u == 0 and d == 0),
                                        stop=False,
                                    )
                            pss.append(ps_q)
                        for j2 in range(2):
                            jp = 2 * hw + j2
                            ps_q = pss[j2]
                            for u in range(2):
                                jt = 2 * jp + u
                                for d in range(NJT // 2, NJT):
                                    nc.tensor.matmul(
                                        ps_q[:, u * 256:(u + 1) * 256],
                                        wfn(d)[:, jt * 128:(jt + 1) * 128],
                                        xT[d][:],
                                        start=False,
                                        stop=(u == 1 and d == NJT - 1),
                                    )
                            pair_t = qkpool.tile(
                                [128, 2, W], F16, tag=tg, name=f"pair_{tg}"
                            )
                            nc.vector.tensor_tensor(
                                pair_t[:],
                                ps_q[:].rearrange("p (u t) -> p u t", u=2),
                                bias[:, 2 * jp:2 * jp + 2][:, :, None]
                                .to_broadcast((128, 2, W)),
                                mybir.AluOpType.add,
                            )
                            lst.append(pair_t)
                            yield
                    if ci == 0 and which == 1:
                        emit_ktm()
                        yield
                state_qk[ci] = (
                    [qT[jt // 2][:, jt % 2, :] for jt in range(NJT)],
                    [kT[jt // 2][:, jt % 2, :] for jt in range(NJT)],
                )

                # V projection (token-major fp16, heads + ones col)
                v_sb = vpool.tile([128, 2, H, VW], F16, tag="v_sb")
                nc.gpsimd.memset(v_sb[:, :, :, HD:HD + 1], 1.0)
                for tt in range(2):
                    pss = []
                    for half in range(2):
                        ps_v = pp_pool.tile([128, 512], F32, tag="pp")
                        for d in range(NJT // 2):
                            nc.tensor.matmul(
                                ps_v[:], xT[d][:, tt * 128:(tt + 1) * 128],
                                wv(d)[:, half * 512:(half + 1) * 512],
                                start=(d == 0), stop=False,
                            )
                        pss.append(ps_v)
                    for half in range(2):
                        ps_v = pss[half]
                        for d in range(NJT // 2, NJT):
                            nc.tensor.matmul(
                                ps_v[:], xT[d][:, tt * 128:(tt + 1) * 128],
                                wv(d)[:, half * 512:(half + 1) * 512],
                                start=False, stop=(d == NJT - 1),
                            )
                        nc.vector.tensor_tensor(
                            v_sb[:, tt, half * 8:(half + 1) * 8, :HD],
                            ps_v[:].rearrange("p (h f) -> p h f", h=8),
                            bvb[:, half * 512:(half + 1) * 512].rearrange(
                                "p (h f) -> p h f", h=8
                            ),
                            mybir.AluOpType.add,
                        )
                        yield
                    if ci == 0 and tt == 0:
                        emit_vm()
                state_v[ci] = v_sb

            def attn_steps(ci):
                while ci not in state_qk:
                    yield
                qT, kT = state_qk.pop(ci)
                kTm3 = memp["kTm3"]

                # mem scores: all 16 heads in one 2-bank psum, one Exp.
                # Head pair jt lands at 32-aligned partition bases via the
                # block-diagonal stationary and tile_position cols.
                ps_m = ps_pool.tile([128, 1024], F32, tag="ps", name="ps_ms")
                for jt in range(NJT):
                    c0 = 64 * (jt % 2)
                    g = jt // 2
                    nc.tensor.matmul(
                        ps_m[c0:c0 + 64, g * 256:(g + 1) * 256],
                        kTm3[:, jt, :],
                        qT[jt][:],
                        start=True, stop=True,
                        tile_position=(0, c0),
                        skip_group_check=True,
                    )
                em = empool.tile([128, 4, 256], F16, tag="em", name="em")
                nc.scalar.activation(
                    em[:], ps_m[:].rearrange("p (g t) -> p g t", g=4),
                    mybir.ActivationFunctionType.Exp,
                    bias=mskv[:, ci * 3 + 2: ci * 3 + 3],
                )
                yield

                out_sb = opool.tile([128, 2, D], F16, tag="out_sb", name="out_sb")
                eloc = {}   # (hpq, yt) -> [128, 1024] fp16

                def fill(hpq, yt):
                    # local scoresT for 4 heads (one quad, one key half);
                    # both 64-row ab halves quadrant-packed; one Exp
                    ps_s = ps_pool.tile([128, 1024], F32, tag="ps", name="ps_s")
                    for ab in range(2):
                        p0 = 64 * ab
                        for u in range(2):
                            jt = 2 * hpq + u
                            nc.tensor.matmul(
                                ps_s[:, ab * 512 + u * 256:
                                     ab * 512 + (u + 1) * 256],
                                kT[jt][p0:p0 + 64, yt * 128:(yt + 1) * 128],
                                qT[jt][p0:p0 + 64, :],
                                start=(u == 0), stop=(u == 1),
                                tile_position=(p0, 0),
                                skip_group_check=True,
                            )
                    e_t = epool.tile([128, 1024], F16, tag="exps")
                    nc.scalar.activation(
                        e_t[:], ps_s[:],
                        mybir.ActivationFunctionType.Exp,
                        bias=mskv[:, ci * 3 + yt: ci * 3 + yt + 1],
                    )
                    eloc[(hpq, yt)] = e_t

                def unit(hp):
                    # PV for head pair hp: one psum bank, 4 blocks (h, xb)
                    # of 65 (64 hd + denom); strided recip + broadcast mult
                    ps_o = po_pool.tile([128, 4 * VW], F32, tag="po")
                    for ab in range(2):
                        h = 2 * hp + ab
                        e0 = eloc[(hp // 2, 0)]
                        e1 = eloc[(hp // 2, 1)]
                        base, g = _mem_slot(h)
                        for xb in range(2):
                            o = (ab * 2 + xb) * VW
                            xs = (h % 2) * 512 + (hp % 2) * 256 + xb * 128
                            nc.tensor.matmul(
                                ps_o[:, o:o + VW],
                                e0[:, xs:xs + 128], v_sb[:, 0, h, :],
                                start=True, stop=False,
                            )
                            nc.tensor.matmul(
                                ps_o[:, o:o + VW],
                                e1[:, xs:xs + 128], v_sb[:, 1, h, :],
                                start=False, stop=False,
                            )
                            nc.tensor.matmul(
                                ps_o[:, o:o + VW],
                                em[base:base + M, g, xb * 128:(xb + 1) * 128],
                                vm[base:base + M, h, :],
                                start=False, stop=True,
                                tile_position=(base, 0),
                            )
                    rec = rpool.tile([128, 4], F32, tag="rec", name="rec")
                    nc.vector.reciprocal(
                        rec[:].rearrange("p (k o) -> p k o", o=1),
                        ps_o[:].rearrange("p (k w) -> p k w", w=VW)[
                            :, :, HD:HD + 1
                        ],
                    )
                    nc.vector.tensor_tensor(
                        out_sb[:, :, 2 * hp * HD:(2 * hp + 2) * HD].rearrange(
                            "p x (a f) -> p a x f", a=2
                        ),
                        ps_o[:].rearrange("p (a x w) -> p a x w", a=2, x=2)[
                            :, :, :, :HD
                        ],
                        rec[:].rearrange("p (a x) -> p a x", a=2)[
                            :, :, :, None
                        ].to_broadcast((128, 2, 2, HD)),
                        mybir.AluOpType.mult,
                    )

                def out_dma(qtr):
                    nc.sync.dma_start(
                        out_d.rearrange("(x p) c -> p x c", p=128)[
                            :, 2 * ci:2 * ci + 2, qtr * 256:(qtr + 1) * 256
                        ],
                        out_sb[:, :, qtr * 256:(qtr + 1) * 256],
                    )

                # fills run two head-pairs ahead of PV units so the Exp
                # latency hides behind interleaved projection matmuls
                fill(0, 0); yield
                fill(0, 1); yield
                fill(1, 0); yield
                fill(1, 1); yield
                while ci not in state_v:
                    yield
                v_sb = state_v.pop(ci)
                vm = memp["vm"]
                unit(0); yield
                unit(1); out_dma(0); yield
                fill(2, 0); yield
                fill(2, 1); yield
                unit(2); yield
                unit(3); out_dma(1); yield
                fill(3, 0); yield
                fill(3, 1); yield
                unit(4); yield
                unit(5); out_dma(2); yield
                unit(6); yield
                unit(7); out_dma(3)

            def drain(*gens):
                gens = [g for g in gens if g is not None]
                while gens:
                    nxt = []
                    for g in gens:
                        try:
                            next(g)
                            nxt.append(g)
                        except StopIteration:
                            pass
                    gens = nxt

            # 3-way weave: chunk ci's attention runs with chunk ci+1's
            # projections, and attn(ci+1) joins early (it self-waits on
            # its state) so the attention tail always has matmul filler
            attns_g = [attn_steps(ci) for ci in range(CPC)]
            p0 = proj_steps(0)
            gens0 = [p0, attns_g[0]]
            must0 = {id(p0)}
            while must0:
                for g in list(gens0):
                    try:
                        next(g)
                    except StopIteration:
                        gens0.remove(g)
                        must0.discard(id(g))
            for ci in range(CPC):
                gens = [attns_g[ci]]
                must = {id(attns_g[ci])}
                if ci + 1 < CPC:
                    pj = proj_steps(ci + 1)
                    gens = [pj, attns_g[ci], attns_g[ci + 1]]
                    must.add(id(pj))
                while must:
                    for g in list(gens):
                        try:
                            next(g)
                        except StopIteration:
                            gens.remove(g)
                            must.discard(id(g))

    nc.compile()
    return nc


_NC_CACHE = None


def kernel(hidden_states, attention_mask, self_memory, Wq, bq, Wk, bk, Wv, bv):
    global _NC_CACHE, LAST_RESULTS
    hidden_states = np.asarray(np.asarray(hidden_states), np.float32)
    attention_mask = np.asarray(np.asarray(attention_mask), np.float32)
    self_memory = np.asarray(np.asarray(self_memory), np.float32)
    wqT = np.ascontiguousarray(
        (np.asarray(Wq, np.float32).T * 0.125).astype(np.float16)
    )
    wkT = np.ascontiguousarray(np.asarray(Wk, np.float32).T.astype(np.float16))
    wvT = np.ascontiguousarray(np.asarray(Wv, np.float32).T.astype(np.float16))
    bqv = np.ascontiguousarray(
        np.asarray(bq, np.float32).reshape(NJT, 128).T * 0.125
    )
    bkv = np.ascontiguousarray(np.asarray(bk, np.float32).reshape(NJT, 128).T)
    bvrow = np.asarray(bv, np.float32).astype(np.float16).reshape(1, D)
    bvv = np.ascontiguousarray(np.asarray(bv, np.float32).reshape(NJT, 128).T)

    # additive mask along the key axis, per (b, c): [yt0 | yt1 | memory]
    am = attention_mask.reshape(B, C, W)
    chunk_has_valid = (am == 0.0).sum(axis=2) > 0
    mem_mask = np.where(chunk_has_valid, 0.0, -10000.0).astype(np.float32)

    if _NC_CACHE is None:
        _NC_CACHE = _build_kernel()
    nc = _NC_CACHE

    x16 = hidden_states.astype(np.float16)
    mem16 = self_memory.astype(np.float16)

    in_maps = []
    for core in range(N_CORES):
        b = core // (N_CORES // B)
        c0 = (core % (N_CORES // B)) * CPC
        mvT = np.zeros((128, CPC * 3), np.float32)
        for ci in range(CPC):
            mvT[:, ci * 3 + 0] = am[b, c0 + ci, 0:128]
            mvT[:, ci * 3 + 1] = am[b, c0 + ci, 128:256]
            mvT[:, ci * 3 + 2] = mem_mask[b, c0 + ci]
        # feature-major pre-transposed x: [128, NJT, TPC]
        xT = np.ascontiguousarray(
            x16[b, c0 * W:(c0 + CPC) * W, :]
            .T.reshape(NJT, 128, TPC).transpose(1, 0, 2)
        ).reshape(128, NJT * TPC)
        memT = np.ascontiguousarray(
            mem16[b].T.reshape(NJT, 128, M).transpose(1, 0, 2)
        ).reshape(128, NJT * M)
        in_maps.append(
            {
                "xT": xT,
                "memT": memT,
                "wqT": wqT,
                "wkT": wkT,
                "wvT": wvT,
                "bqv": bqv,
                "bkv": bkv,
                "bvrow": bvrow,
                "bvv": bvv,
                "maskvT": mvT,
            }
        )

    res = run_bass_kernel_spmd(nc, in_maps, list(range(N_CORES)), trace=TRACE)
    LAST_RESULTS = res

    out = np.empty((B, S, D), np.float32)
    for core in range(N_CORES):
        b = core // (N_CORES // B)
        c0 = (core % (N_CORES // B)) * CPC
        out[b, c0 * W:(c0 + CPC) * W, :] = res.results[core]["out"].astype(
            np.float32
        )
    return out


# revision 90
# speedup vs baseline: 1.1984x; 1.0036x over previous
"""Bass/Trainium2 kernel for chunked local attention with memory tokens
(BertSelfAttention variant). Self-contained: hardcodes all shapes.

Sharding: 8 cores, each handles 4 of the 32 (batch, chunk) pairs.
  core i -> b = i // 4, chunks 4*(i % 4) .. 4*(i % 4) + 3
No collectives; weights replicated per core; host scatters/gathers.

Per-core device computation (PE operands fp16, accumulation fp32):
  - xT [128, 8, 1024] arrives pre-transposed from host (feature-major),
    sliced per chunk; no device transposes
  - qT[jt] = x@(Wq/8) + bq/8 (feature-major)   kT[jt] = x@Wk + bk
  - v[t, yt, h, 65] token-major with a ones column at 64 that makes the
    PV matmul emit softmax denominators
  - memory-token K is materialized block-diagonally (kTm3 [128, jt, 64]:
    rows 0:64 x cols 0:16 = even head, rows 64:128 x cols 32:48 = odd) so
    one 128-contraction matmul scores 2 heads' mem keys at 32-aligned
    psum partitions; all 16 heads' mem scores share one 2-bank psum and
    a single Exp
  - local scoresT per (head-quad, yt) fill a 2-bank [128, 1024] psum
    (both 64-row ab halves, quadrant-packed) -> one Exp per fill
  - probs = Exp(scoresT + mask[y]) in fp16; additive key mask rides the
    ACT bias operand (per-partition scalar)
  - out_unnorm[x, 4*(hd|den)] per head-pair in one psum bank; strided
    reciprocal + one broadcast multiply normalize and emit fp16 out
Output fp16 on device, upcast to fp32 on host.

Scheduling: generator-based software pipelining weaves chunk ci's
attention with chunk ci+1's projections (and lets attn(ci+1) join as
soon as its projections land) so the PE streams matmuls while ACT runs
the Exps; projection fills split at the 1MB weight-DMA boundary so
chunk 0 tracks weight arrival; tiny warm matmuls keep the PE p-state
ramp clock alive across startup DMA waits; upfront DMAs are ordered by
first consumption on a single queue.
"""

import sys

sys.path.insert(0, "/opt/trn_rl_repo")

import numpy as np

import concourse.tile as tile
from concourse import bacc, mybir
from concourse.bass_utils import run_bass_kernel_spmd

F32 = mybir.dt.float32
F16 = mybir.dt.float16

B, S, D = 2, 4096, 1024
H, HD = 16, 64
W = 256            # attention window (chunk length)
C = S // W         # 16 chunks
M = 16             # memory tokens
N_CORES = 8
CPC = C * B // N_CORES  # 4 chunks per core
TPC = CPC * W           # 1024 chunk tokens per core
NJT = D // 128          # 8 feature tiles
VW = HD + 1             # v width (64 hd + ones col)

TRACE = False
LAST_RESULTS = None


def _mem_slot(h):
    """(partition base, free block) of head h's mem scores / probs."""
    jt, u = h // 2, h % 2
    return 64 * (jt % 2) + 32 * u, jt // 2


def _build_kernel():
    nc = bacc.Bacc(None, target_bir_lowering=False)

    xT_d = nc.declare_dram_parameter("xT", [128, NJT * TPC], F16, isOutput=False)
    memT_d = nc.declare_dram_parameter("memT", [128, NJT * M], F16, isOutput=False)
    wqT_d = nc.declare_dram_parameter("wqT", [D, D], F16, isOutput=False)
    wkT_d = nc.declare_dram_parameter("wkT", [D, D], F16, isOutput=False)
    wvT_d = nc.declare_dram_parameter("wvT", [D, D], F16, isOutput=False)
    bq_d = nc.declare_dram_parameter("bqv", [128, NJT], F32, isOutput=False)
    bk_d = nc.declare_dram_parameter("bkv", [128, NJT], F32, isOutput=False)
    bv_d = nc.declare_dram_parameter("bvrow", [1, D], F16, isOutput=False)
    bvv_d = nc.declare_dram_parameter("bvv", [128, NJT], F32, isOutput=False)
    msk_d = nc.declare_dram_parameter("maskvT", [128, CPC * 3], F32, isOutput=False)
    out_d = nc.declare_dram_parameter("out", [TPC, D], F16, isOutput=True)

    with tile.TileContext(nc) as tc:
        with (
            tc.tile_pool(name="const", bufs=1) as cpool,
            tc.tile_pool(name="wpool", bufs=1) as wpool,
            tc.tile_pool(name="xtpool", bufs=4) as xtpool,
            tc.tile_pool(name="qkpool", bufs=16) as qkpool,
            tc.tile_pool(name="vpool", bufs=4) as vpool,
            tc.tile_pool(name="epool", bufs=16) as epool,
            tc.tile_pool(name="empool", bufs=3) as empool,
            tc.tile_pool(name="opool", bufs=4) as opool,
            tc.tile_pool(name="rpool", bufs=4) as rpool,
            tc.tile_pool(name="pp", bufs=2, space="PSUM") as pp_pool,
            tc.tile_pool(name="ps", bufs=2, space="PSUM") as ps_pool,
            tc.tile_pool(name="po", bufs=2, space="PSUM") as po_pool,
        ):
            x_tiles = {}

            def load_x(ci, eng, split=False):
                x_t = xtpool.tile([128, NJT, W], F16, tag="xT", name="xT")
                xs = xT_d.rearrange("p (o t) -> p o t", t=TPC)[
                    :, :, ci * W:(ci + 1) * W
                ]
                if split:
                    eng.dma_start(x_t[:, 0:4, :], xs[:, 0:4, :])
                    eng.dma_start(x_t[:, 4:8, :], xs[:, 4:8, :])
                else:
                    eng.dma_start(x_t[:], xs)
                x_tiles[ci] = [x_t[:, d, :] for d in range(NJT)]

            w_all = wpool.tile([128, 3 * NJT, D], F16, tag="w_all")

            def load_w(wi, wd, gran=2):
                # row-block granules so chunk-0 matmuls track arrival
                for o in range(0, NJT, gran):
                    nc.sync.dma_start(
                        w_all[:, wi * NJT + o: wi * NJT + o + gran, :],
                        wd.rearrange("(o p) c -> p o c", p=128)[:, o:o + gran, :],
                    )

            def wq(d):
                return w_all[:, d, :]

            def wk(d):
                return w_all[:, NJT + d, :]

            def wv(d):
                return w_all[:, 2 * NJT + d, :]

            # ---- upfront DMAs: one queue, in consumption order; weights
            # in 1MB halves ordered by when the PE stream consumes them.
            load_x(0, nc.sync)
            bvrow = cpool.tile([1, D], F16, tag="bvrow")
            nc.sync.dma_start(bvrow[:], bv_d[:])
            bqv = cpool.tile([128, NJT], F32, tag="bqv")
            nc.sync.dma_start(bqv[:], bq_d[:])
            load_w(0, wqT_d, gran=4)
            xTm = cpool.tile([128, NJT, M], F16, tag="xTm")
            nc.sync.dma_start(xTm[:], memT_d.rearrange("p (o m) -> p o m", m=M))
            load_w(1, wkT_d, gran=4)
            bkv = cpool.tile([128, NJT], F32, tag="bkv")
            nc.sync.dma_start(bkv[:], bk_d[:])
            mskv = cpool.tile([128, CPC * 3], F32, tag="mskv")
            nc.sync.dma_start(mskv[:], msk_d[:])
            bvv = cpool.tile([128, NJT], F32, tag="bvv")
            nc.sync.dma_start(bvv[:], bvv_d[:])
            nc.sync.dma_start(
                w_all[:, 2 * NJT: 2 * NJT + 4, :],
                wvT_d.rearrange("(o p) c -> p o c", p=128)[:, 0:4, :],
            )
            load_x(1, nc.sync)
            nc.sync.dma_start(
                w_all[:, 2 * NJT + 4: 2 * NJT + 8, :],
                wvT_d.rearrange("(o p) c -> p o c", p=128)[:, 4:8, :],
            )
            load_x(2, nc.sync)
            load_x(3, nc.sync)

            ones1 = cpool.tile([1, 128], F16, tag="ones1")
            nc.vector.memset(ones1[:], 1.0)
            bvb = cpool.tile([128, D], F32, tag="bvb")

            # tiny dep-free matmul: starts the PE p-state ramp clock so the
            # projections hit full clock as soon as their weights land; the
            # bvb ones-matmuls (gated only on the tiny bvrow DMA) keep the
            # PE's idle gaps under the ~3us p-state reset threshold
            ps_warm = pp_pool.tile([128, 512], F32, tag="pp")
            nc.tensor.matmul(
                ps_warm[:16, :16], ones1[:, :16], ones1[:, :16],
                start=True, stop=True,
            )
            x0d0 = x_tiles[0][0]
            nc.tensor.matmul(
                ps_warm[:16, 16:32], x0d0[:16, :16], x0d0[:16, :16],
                start=True, stop=True,
            )
            for half in range(2):
                ps_b = pp_pool.tile([128, 512], F32, tag="pp")
                nc.tensor.matmul(
                    ps_b[:], ones1[:], bvrow[:, half * 512:(half + 1) * 512],
                    start=True, stop=True,
                )
                nc.vector.tensor_copy(
                    bvb[:, half * 512:(half + 1) * 512], ps_b[:]
                )

            # ---- memory tokens (emitted inside chunk 0's flow) ----
            memp = {}

            def emit_ktm():
                # block-diagonal mem-K: [128, jt, 64]; even head rows 0:64 ->
                # cols 0:16, odd head rows 64:128 -> cols 32:48, rest zero
                kTm3 = cpool.tile([128, NJT, 64], F16, tag="kTm3", name="kTm3")
                nc.gpsimd.memset(kTm3[:], 0.0)
                ps_k = pp_pool.tile([128, 512], F32, tag="pp", name="ps_ktm")
                for jt in range(NJT):
                    for d in range(NJT):
                        nc.tensor.matmul(
                            ps_k[:, jt * M:(jt + 1) * M],
                            wk(d)[:, jt * 128:(jt + 1) * 128],
                            xTm[:, d, :],
                            start=(d == 0), stop=(d == NJT - 1),
                            skip_group_check=True,
                        )
                for u in range(2):
                    nc.vector.tensor_tensor(
                        kTm3[64 * u:64 * (u + 1), :, 32 * u:32 * u + 16],
                        ps_k[64 * u:64 * (u + 1), :NJT * M].rearrange(
                            "p (j m) -> p j m", m=M
                        ),
                        bkv[64 * u:64 * (u + 1), :][:, :, None].to_broadcast(
                            (64, NJT, M)
                        ),
                        mybir.AluOpType.add,
                    )
                memp["kTm3"] = kTm3

            def emit_vm():
                # mem-V computed feature-major in one cheap psum fill, then
                # token-major via an xbar DMA-transpose of the m-padded
                # [128, jt, 128] layout (dst[p, jt, f] = src[f, jt*128+p]);
                # replicated at partition bases 0/32/64/96 for the
                # 32-aligned mem-PV stationaries
                vm = cpool.tile([128, M, VW], F16, tag="vm", name="vm")
                nc.vector.memset(vm[:M, :, HD:HD + 1], 1.0)
                vmT = cpool.tile([128, NJT, 128], F16, tag="vmT", name="vmT")
                nc.gpsimd.memset(vmT[:], 0.0)
                ps_t = pp_pool.tile([128, 512], F32, tag="pp", name="ps_vm")
                for jt in range(NJT):
                    for d in range(NJT):
                        nc.tensor.matmul(
                            ps_t[:, jt * M:(jt + 1) * M],
                            wv(d)[:, jt * 128:(jt + 1) * 128],
                            xTm[:, d, :],
                            start=(d == 0), stop=(d == NJT - 1),
                            skip_group_check=True,
                        )
                nc.vector.tensor_tensor(
                    vmT[:, :, :M],
                    ps_t[:, :NJT * M].rearrange("p (j m) -> p j m", m=M),
                    bvv[:, :, None].to_broadcast((128, NJT, M)),
                    mybir.AluOpType.add,
                )
                vmB = cpool.tile([128, NJT, 128], F16, tag="vmB", name="vmB")
                nc.sync.dma_start_transpose(
                    vmB[:], vmT[:].rearrange("p j m -> p (j m)")
                )
                nc.vector.tensor_copy(
                    vm[:M, :, :HD].rearrange("m (j u) f -> m j u f", u=2),
                    vmB[:M, :, :].rearrange("m j (u f) -> m j u f", u=2),
                )
                for rb in range(1, 4):
                    nc.sync.dma_start(vm[32 * rb:32 * rb + M, :, :], vm[:M, :, :])
                memp["vm"] = vm

            # ---- per-chunk phases as generators; the main loop weaves
            # chunk ci's attention with chunk ci+1's projections so the PE
            # always has projection matmuls to stream while Act runs Exps.
            # The handoff is split Q/K vs V: score fills only need Q/K, so
            # they start a V-phase early and V matmuls fill attention tails.
            state_qk = {}
            state_v = {}

            def proj_steps(ci):
                xT = x_tiles.pop(ci)
                qT, kT = [], []
                for which, wfn, bias, lst, tg in (
                    (0, wq, bqv, qT, "qT"),
                    (1, wk, bkv, kT, "kT"),
                ):
                    # pp-buf pairs of jt-pair fills, split at the 1MB
                    # weight-DMA boundary (d 0-3 | 4-7) so chunk 0's
                    # matmuls track weight arrival
                    for hw in range(2):
                        pss = []
                        for j2 in range(2):
                            jp = 2 * hw + j2
                            ps_q = pp_pool.tile(
                                [128, 512], F32, tag="pp", name="ps_q"
                            )
                            for u in range(2):
                                jt = 2 * jp + u
                                for d in range(NJT // 2):
                                    nc.tensor.matmul(
                                        ps_q[:, u * 256:(u + 1) * 256],
                                        wfn(d)[:, jt * 128:(jt + 1) * 128],
                                        xT[d][:],
                                        start=(u == 0 and d == 0),
                                        stop=False,
                                    )
                            pss.append(ps_q)
                        for j2 in range(2):
                            jp = 2 * hw + j2
                            ps_q = pss[j2]
                            for u in range(2):
                                jt = 2 * jp + u
                                for d in range(NJT // 2, NJT):
                                    nc.tensor.matmul(
                                        ps_q[:, u * 256:(u + 1) * 256],
                                        wfn(d)[:, jt * 128:(jt + 1) * 128],
                                        xT[d][:],
                                        start=False,
                                        stop=(u == 1 and d == NJT - 1),
                                    )
                            pair_t = qkpool.tile(
                                [128, 2, W], F16, tag=tg, name=f"pair_{tg}"
                            )
                            nc.vector.tensor_tensor(
                                pair_t[:],
                                ps_q[:].rearrange("p (u t) -> p u t", u=2),
                                bias[:, 2 * jp:2 * jp + 2][:, :, None]
                                .to_broadcast((128, 2, W)),
                                mybir.AluOpType.add,
                            )
                            lst.append(pair_t)
                            yield
                    if ci == 0 and which == 1:
                        emit_ktm()
                        yield
                state_qk[ci] = (
                    [qT[jt // 2][:, jt % 2, :] for jt in range(NJT)],
                    [kT[jt // 2][:, jt % 2, :] for jt in range(NJT)],
                )

                # V projection (token-major fp16, heads + ones col)
                v_sb = vpool.tile([128, 2, H, VW], F16, tag="v_sb")
                nc.gpsimd.memset(v_sb[:, :, :, HD:HD + 1], 1.0)
                for tt in range(2):
                    pss = []
                    for half in range(2):
                        ps_v = pp_pool.tile([128, 512], F32, tag="pp")
                        for d in range(NJT // 2):
                            nc.tensor.matmul(
                                ps_v[:], xT[d][:, tt * 128:(tt + 1) * 128],
                                wv(d)[:, half * 512:(half + 1) * 512],
                                start=(d == 0), stop=False,
                            )
                        pss.append(ps_v)
                    for half in range(2):
                        ps_v = pss[half]
                        for d in range(NJT // 2, NJT):
                            nc.tensor.matmul(
                                ps_v[:], xT[d][:, tt * 128:(tt + 1) * 128],
                                wv(d)[:, half * 512:(half + 1) * 512],
                                start=False, stop=(d == NJT - 1),
                            )
                        nc.vector.tensor_tensor(
                            v_sb[:, tt, half * 8:(half + 1) * 8, :HD],
                            ps_v[:].rearrange("p (h f) -> p h f", h=8),
                            bvb[:, half * 512:(half + 1) * 512].rearrange(
                                "p (h f) -> p h f", h=8
                            ),
                            mybir.AluOpType.add,
                        )
                        yield
                    if ci == 0 and tt == 0:
                        emit_vm()
                state_v[ci] = v_sb

            def attn_steps(ci):
                while ci not in state_qk:
                    yield
                qT, kT = state_qk.pop(ci)
                kTm3 = memp["kTm3"]

                # mem scores: all 16 heads in one 2-bank psum, one Exp.
                # Head pair jt lands at 32-aligned partition bases via the
                # block-diagonal stationary and tile_position cols.
                def emit_ms():
                    ps_m = ps_pool.tile([128, 1024], F32, tag="ps", name="ps_ms")
                    for jt in range(NJT):
                        c0 = 64 * (jt % 2)
                        g = jt // 2
                        nc.tensor.matmul(
                            ps_m[c0:c0 + 64, g * 256:(g + 1) * 256],
                            kTm3[:, jt, :],
                            qT[jt][:],
                            start=True, stop=True,
                            tile_position=(0, c0),
                            skip_group_check=True,
                        )
                    em = empool.tile([128, 4, 256], F16, tag="em", name="em")
                    nc.scalar.activation(
                        em[:], ps_m[:].rearrange("p (g t) -> p g t", g=4),
                        mybir.ActivationFunctionType.Exp,
                        bias=mskv[:, ci * 3 + 2: ci * 3 + 3],
                    )
                    return em

                out_sb = opool.tile([128, 2, D], F16, tag="out_sb", name="out_sb")
                eloc = {}   # (hpq, yt) -> [128, 1024] fp16

                def fill(hpq, yt):
                    # local scoresT for 4 heads (one quad, one key half);
                    # both 64-row ab halves quadrant-packed; one Exp
                    ps_s = ps_pool.tile([128, 1024], F32, tag="ps", name="ps_s")
                    for ab in range(2):
                        p0 = 64 * ab
                        for u in range(2):
                            jt = 2 * hpq + u
                            nc.tensor.matmul(
                                ps_s[:, ab * 512 + u * 256:
                                     ab * 512 + (u + 1) * 256],
                                kT[jt][p0:p0 + 64, yt * 128:(yt + 1) * 128],
                                qT[jt][p0:p0 + 64, :],
                                start=(u == 0), stop=(u == 1),
                                tile_position=(p0, 0),
                                skip_group_check=True,
                            )
                    e_t = epool.tile([128, 1024], F16, tag="exps")
                    nc.scalar.activation(
                        e_t[:], ps_s[:],
                        mybir.ActivationFunctionType.Exp,
                        bias=mskv[:, ci * 3 + yt: ci * 3 + yt + 1],
                    )
                    eloc[(hpq, yt)] = e_t

                def unit(hp):
                    # PV for head pair hp: one psum bank, 4 blocks (h, xb)
                    # of 65 (64 hd + denom); strided recip + broadcast mult
                    ps_o = po_pool.tile([128, 4 * VW], F32, tag="po")
                    for ab in range(2):
                        h = 2 * hp + ab
                        e0 = eloc[(hp // 2, 0)]
                        e1 = eloc[(hp // 2, 1)]
                        base, g = _mem_slot(h)
                        for xb in range(2):
                            o = (ab * 2 + xb) * VW
                            xs = (h % 2) * 512 + (hp % 2) * 256 + xb * 128
                            nc.tensor.matmul(
                                ps_o[:, o:o + VW],
                                e0[:, xs:xs + 128], v_sb[:, 0, h, :],
                                start=True, stop=False,
                            )
                            nc.tensor.matmul(
                                ps_o[:, o:o + VW],
                                e1[:, xs:xs + 128], v_sb[:, 1, h, :],
                                start=False, stop=False,
                            )
                            nc.tensor.matmul(
                                ps_o[:, o:o + VW],
                                em[base:base + M, g, xb * 128:(xb + 1) * 128],
                                vm[base:base + M, h, :],
                                start=False, stop=True,
                                tile_position=(base, 0),
                            )
                    rec = rpool.tile([128, 4], F32, tag="rec", name="rec")
                    nc.vector.reciprocal(
                        rec[:].rearrange("p (k o) -> p k o", o=1),
                        ps_o[:].rearrange("p (k w) -> p k w", w=VW)[
                            :, :, HD:HD + 1
                        ],
                    )
                    nc.vector.tensor_tensor(
                        out_sb[:, :, 2 * hp * HD:(2 * hp + 2) * HD].rearrange(
                            "p x (a f) -> p a x f", a=2
                        ),
                        ps_o[:].rearrange("p (a x w) -> p a x w", a=2, x=2)[
                            :, :, :, :HD
                        ],
                        rec[:].rearrange("p (a x) -> p a x", a=2)[
                            :, :, :, None
                        ].to_broadcast((128, 2, 2, HD)),
                        mybir.AluOpType.mult,
                    )

                def out_dma(qtr):
                    nc.sync.dma_start(
                        out_d.rearrange("(x p) c -> p x c", p=128)[
                            :, 2 * ci:2 * ci + 2, qtr * 256:(qtr + 1) * 256
                        ],
                        out_sb[:, :, qtr * 256:(qtr + 1) * 256],
                    )

                # fills run two head-pairs ahead of PV units so the Exp
                # latency hides behind interleaved projection matmuls
                fill(0, 0); yield
                fill(0, 1); yield
                em = emit_ms(); yield
                fill(1, 0); yield
                fill(1, 1); yield
                while ci not in state_v:
                    yield
                v_sb = state_v.pop(ci)
                vm = memp["vm"]
                unit(0); yield
                unit(1); out_dma(0); yield
                fill(2, 0); yield
                fill(2, 1); yield
                unit(2); yield
                unit(3); out_dma(1); yield
                fill(3, 0); yield
                fill(3, 1); yield
                unit(4); yield
                unit(5); out_dma(2); yield
                unit(6); yield
                unit(7); out_dma(3)

            def drain(*gens):
                gens = [g for g in gens if g is not None]
                while gens:
                    nxt = []
                    for g in gens:
                        try:
                            next(g)
                            nxt.append(g)
                        except StopIteration:
                            pass
                    gens = nxt

            # 3-way weave: chunk ci's attention runs with chunk ci+1's
            # projections, and attn(ci+1) joins early (it self-waits on
            # its state) so the attention tail always has matmul filler
            attns_g = [attn_steps(ci) for ci in range(CPC)]
            p0 = proj_steps(0)
            gens0 = [p0, attns_g[0]]
            must0 = {id(p0)}
            while must0:
                for g in list(gens0):
                    try:
                        next(g)
                    except StopIteration:
                        gens0.remove(g)
                        must0.discard(id(g))
            for ci in range(CPC):
                gens = [attns_g[ci]]
                must = {id(attns_g[ci])}
                if ci + 1 < CPC:
                    pj = proj_steps(ci + 1)
                    gens = [pj, attns_g[ci], attns_g[ci + 1]]
                    must.add(id(pj))
                while must:
                    for g in list(gens):
                        try:
                            next(g)
                        except StopIteration:
                            gens.remove(g)
                            must.discard(id(g))

    nc.compile()
    return nc


_NC_CACHE = None


def kernel(hidden_states, attention_mask, self_memory, Wq, bq, Wk, bk, Wv, bv):
    global _NC_CACHE, LAST_RESULTS
    hidden_states = np.asarray(np.asarray(hidden_states), np.float32)
    attention_mask = np.asarray(np.asarray(attention_mask), np.float32)
    self_memory = np.asarray(np.asarray(self_memory), np.float32)
    wqT = np.ascontiguousarray(
        (np.asarray(Wq, np.float32).T * 0.125).astype(np.float16)
    )
    wkT = np.ascontiguousarray(np.asarray(Wk, np.float32).T.astype(np.float16))
    wvT = np.ascontiguousarray(np.asarray(Wv, np.float32).T.astype(np.float16))
    bqv = np.ascontiguousarray(
        np.asarray(bq, np.float32).reshape(NJT, 128).T * 0.125
    )
    bkv = np.ascontiguousarray(np.asarray(bk, np.float32).reshape(NJT, 128).T)
    bvrow = np.asarray(bv, np.float32).astype(np.float16).reshape(1, D)
    bvv = np.ascontiguousarray(np.asarray(bv, np.float32).reshape(NJT, 128).T)

    # additive mask along the key axis, per (b, c): [yt0 | yt1 | memory]
    am = attention_mask.reshape(B, C, W)
    chunk_has_valid = (am == 0.0).sum(axis=2) > 0
    mem_mask = np.where(chunk_has_valid, 0.0, -10000.0).astype(np.float32)

    if _NC_CACHE is None:
        _NC_CACHE = _build_kernel()
    nc = _NC_CACHE

    x16 = hidden_states.astype(np.float16)
    mem16 = self_memory.astype(np.float16)

    in_maps = []
    for core in range(N_CORES):
        b = core // (N_CORES // B)
        c0 = (core % (N_CORES // B)) * CPC
        mvT = np.zeros((128, CPC * 3), np.float32)
        for ci in range(CPC):
            mvT[:, ci * 3 + 0] = am[b, c0 + ci, 0:128]
            mvT[:, ci * 3 + 1] = am[b, c0 + ci, 128:256]
            mvT[:, ci * 3 + 2] = mem_mask[b, c0 + ci]
        # feature-major pre-transposed x: [128, NJT, TPC]
        xT = np.ascontiguousarray(
            x16[b, c0 * W:(c0 + CPC) * W, :]
            .T.reshape(NJT, 128, TPC).transpose(1, 0, 2)
        ).reshape(128, NJT * TPC)
        memT = np.ascontiguousarray(
            mem16[b].T.reshape(NJT, 128, M).transpose(1, 0, 2)
        ).reshape(128, NJT * M)
        in_maps.append(
            {
                "xT": xT,
                "memT": memT,
                "wqT": wqT,
                "wkT": wkT,
                "wvT": wvT,
                "bqv": bqv,
                "bkv": bkv,
                "bvrow": bvrow,
                "bvv": bvv,
                "maskvT": mvT,
            }
        )

    res = run_bass_kernel_spmd(nc, in_maps, list(range(N_CORES)), trace=TRACE)
    LAST_RESULTS = res

    out = np.empty((B, S, D), np.float32)
    for core in range(N_CORES):
        b = core // (N_CORES // B)
        c0 = (core % (N_CORES // B)) * CPC
        out[b, c0 * W:(c0 + CPC) * W, :] = res.results[core]["out"].astype(
            np.float32
        )
    return out


# revision 91
# speedup vs baseline: 1.1991x; 1.0005x over previous
"""Bass/Trainium2 kernel for chunked local attention with memory tokens
(BertSelfAttention variant). Self-contained: hardcodes all shapes.

Sharding: 8 cores, each handles 4 of the 32 (batch, chunk) pairs.
  core i -> b = i // 4, chunks 4*(i % 4) .. 4*(i % 4) + 3
No collectives; weights replicated per core; host scatters/gathers.

Per-core device computation (PE operands fp16, accumulation fp32):
  - xT [128, 8, 1024] arrives pre-transposed from host (feature-major),
    sliced per chunk; no device transposes
  - qT[jt] = x@(Wq/8) + bq/8 (feature-major)   kT[jt] = x@Wk + bk
  - v[t, yt, h, 65] token-major with a ones column at 64 that makes the
    PV matmul emit softmax denominators
  - memory-token K is materialized block-diagonally (kTm3 [128, jt, 64]:
    rows 0:64 x cols 0:16 = even head, rows 64:128 x cols 32:48 = odd) so
    one 128-contraction matmul scores 2 heads' mem keys at 32-aligned
    psum partitions; all 16 heads' mem scores share one 2-bank psum and
    a single Exp
  - local scoresT per (head-quad, yt) fill a 2-bank [128, 1024] psum
    (both 64-row ab halves, quadrant-packed) -> one Exp per fill
  - probs = Exp(scoresT + mask[y]) in fp16; additive key mask rides the
    ACT bias operand (per-partition scalar)
  - out_unnorm[x, 4*(hd|den)] per head-pair in one psum bank; strided
    reciprocal + one broadcast multiply normalize and emit fp16 out
Output fp16 on device, upcast to fp32 on host.

Scheduling: generator-based software pipelining weaves chunk ci's
attention with chunk ci+1's projections (and lets attn(ci+1) join as
soon as its projections land) so the PE streams matmuls while ACT runs
the Exps; projection fills split at the 1MB weight-DMA boundary so
chunk 0 tracks weight arrival; tiny warm matmuls keep the PE p-state
ramp clock alive across startup DMA waits; upfront DMAs are ordered by
first consumption on a single queue.
"""

import sys

sys.path.insert(0, "/opt/trn_rl_repo")

import numpy as np

import concourse.tile as tile
from concourse import bacc, mybir
from concourse.bass_utils import run_bass_kernel_spmd

F32 = mybir.dt.float32
F16 = mybir.dt.float16

B, S, D = 2, 4096, 1024
H, HD = 16, 64
W = 256            # attention window (chunk length)
C = S // W         # 16 chunks
M = 16             # memory tokens
N_CORES = 8
CPC = C * B // N_CORES  # 4 chunks per core
TPC = CPC * W           # 1024 chunk tokens per core
NJT = D // 128          # 8 feature tiles
VW = HD + 1             # v width (64 hd + ones col)

TRACE = False
LAST_RESULTS = None


def _mem_slot(h):
    """(partition base, free block) of head h's mem scores / probs."""
    jt, u = h // 2, h % 2
    return 64 * (jt % 2) + 32 * u, jt // 2


def _build_kernel():
    nc = bacc.Bacc(None, target_bir_lowering=False)

    xT_d = nc.declare_dram_parameter("xT", [128, NJT * TPC], F16, isOutput=False)
    memT_d = nc.declare_dram_parameter("memT", [128, NJT * M], F16, isOutput=False)
    wqT_d = nc.declare_dram_parameter("wqT", [D, D], F16, isOutput=False)
    wkT_d = nc.declare_dram_parameter("wkT", [D, D], F16, isOutput=False)
    wvT_d = nc.declare_dram_parameter("wvT", [D, D], F16, isOutput=False)
    bq_d = nc.declare_dram_parameter("bqv", [128, NJT], F32, isOutput=False)
    bk_d = nc.declare_dram_parameter("bkv", [128, NJT], F32, isOutput=False)
    bv_d = nc.declare_dram_parameter("bvrow", [1, D], F16, isOutput=False)
    bvv_d = nc.declare_dram_parameter("bvv", [128, NJT], F32, isOutput=False)
    msk_d = nc.declare_dram_parameter("maskvT", [128, CPC * 3], F32, isOutput=False)
    out_d = nc.declare_dram_parameter("out", [TPC, D], F16, isOutput=True)

    with tile.TileContext(nc) as tc:
        with (
            tc.tile_pool(name="const", bufs=1) as cpool,
            tc.tile_pool(name="wpool", bufs=1) as wpool,
            tc.tile_pool(name="xtpool", bufs=4) as xtpool,
            tc.tile_pool(name="qkpool", bufs=16) as qkpool,
            tc.tile_pool(name="vpool", bufs=4) as vpool,
            tc.tile_pool(name="epool", bufs=16) as epool,
            tc.tile_pool(name="empool", bufs=3) as empool,
            tc.tile_pool(name="opool", bufs=4) as opool,
            tc.tile_pool(name="rpool", bufs=4) as rpool,
            tc.tile_pool(name="pp", bufs=2, space="PSUM") as pp_pool,
            tc.tile_pool(name="ps", bufs=2, space="PSUM") as ps_pool,
            tc.tile_pool(name="po", bufs=2, space="PSUM") as po_pool,
        ):
            x_tiles = {}

            def load_x(ci, eng, split=False):
                x_t = xtpool.tile([128, NJT, W], F16, tag="xT", name="xT")
                xs = xT_d.rearrange("p (o t) -> p o t", t=TPC)[
                    :, :, ci * W:(ci + 1) * W
                ]
                if split:
                    eng.dma_start(x_t[:, 0:4, :], xs[:, 0:4, :])
                    eng.dma_start(x_t[:, 4:8, :], xs[:, 4:8, :])
                else:
                    eng.dma_start(x_t[:], xs)
                x_tiles[ci] = [x_t[:, d, :] for d in range(NJT)]

            w_all = wpool.tile([128, 3 * NJT, D], F16, tag="w_all")

            def load_w(wi, wd, gran=2):
                # row-block granules so chunk-0 matmuls track arrival
                for o in range(0, NJT, gran):
                    nc.sync.dma_start(
                        w_all[:, wi * NJT + o: wi * NJT + o + gran, :],
                        wd.rearrange("(o p) c -> p o c", p=128)[:, o:o + gran, :],
                    )

            def wq(d):
                return w_all[:, d, :]

            def wk(d):
                return w_all[:, NJT + d, :]

            def wv(d):
                return w_all[:, 2 * NJT + d, :]

            # ---- upfront DMAs: one queue, in consumption order; weights
            # in 1MB halves ordered by when the PE stream consumes them.
            load_x(0, nc.sync)
            bvrow = cpool.tile([1, D], F16, tag="bvrow")
            nc.sync.dma_start(bvrow[:], bv_d[:])
            bqv = cpool.tile([128, NJT], F32, tag="bqv")
            nc.sync.dma_start(bqv[:], bq_d[:])
            load_w(0, wqT_d, gran=4)
            xTm = cpool.tile([128, NJT, M], F16, tag="xTm")
            nc.sync.dma_start(xTm[:], memT_d.rearrange("p (o m) -> p o m", m=M))
            load_w(1, wkT_d, gran=4)
            bkv = cpool.tile([128, NJT], F32, tag="bkv")
            nc.sync.dma_start(bkv[:], bk_d[:])
            mskv = cpool.tile([128, CPC * 3], F32, tag="mskv")
            nc.sync.dma_start(mskv[:], msk_d[:])
            bvv = cpool.tile([128, NJT], F32, tag="bvv")
            nc.sync.dma_start(bvv[:], bvv_d[:])
            load_x(1, nc.sync)
            nc.sync.dma_start(
                w_all[:, 2 * NJT: 2 * NJT + 4, :],
                wvT_d.rearrange("(o p) c -> p o c", p=128)[:, 0:4, :],
            )
            nc.sync.dma_start(
                w_all[:, 2 * NJT + 4: 2 * NJT + 8, :],
                wvT_d.rearrange("(o p) c -> p o c", p=128)[:, 4:8, :],
            )
            load_x(2, nc.sync)
            load_x(3, nc.sync)

            ones1 = cpool.tile([1, 128], F16, tag="ones1")
            nc.vector.memset(ones1[:], 1.0)
            bvb = cpool.tile([128, D], F32, tag="bvb")

            # tiny dep-free matmul: starts the PE p-state ramp clock so the
            # projections hit full clock as soon as their weights land; the
            # bvb ones-matmuls (gated only on the tiny bvrow DMA) keep the
            # PE's idle gaps under the ~3us p-state reset threshold
            ps_warm = pp_pool.tile([128, 512], F32, tag="pp")
            nc.tensor.matmul(
                ps_warm[:16, :16], ones1[:, :16], ones1[:, :16],
                start=True, stop=True,
            )
            x0d0 = x_tiles[0][0]
            nc.tensor.matmul(
                ps_warm[:16, 16:32], x0d0[:16, :16], x0d0[:16, :16],
                start=True, stop=True,
            )
            for half in range(2):
                ps_b = pp_pool.tile([128, 512], F32, tag="pp")
                nc.tensor.matmul(
                    ps_b[:], ones1[:], bvrow[:, half * 512:(half + 1) * 512],
                    start=True, stop=True,
                )
                nc.vector.tensor_copy(
                    bvb[:, half * 512:(half + 1) * 512], ps_b[:]
                )

            # ---- memory tokens (emitted inside chunk 0's flow) ----
            memp = {}

            def emit_ktm():
                # block-diagonal mem-K: [128, jt, 64]; even head rows 0:64 ->
                # cols 0:16, odd head rows 64:128 -> cols 32:48, rest zero
                kTm3 = cpool.tile([128, NJT, 64], F16, tag="kTm3", name="kTm3")
                nc.gpsimd.memset(kTm3[:], 0.0)
                ps_k = pp_pool.tile([128, 512], F32, tag="pp", name="ps_ktm")
                for jt in range(NJT):
                    for d in range(NJT):
                        nc.tensor.matmul(
                            ps_k[:, jt * M:(jt + 1) * M],
                            wk(d)[:, jt * 128:(jt + 1) * 128],
                            xTm[:, d, :],
                            start=(d == 0), stop=(d == NJT - 1),
                            skip_group_check=True,
                        )
                for u in range(2):
                    nc.vector.tensor_tensor(
                        kTm3[64 * u:64 * (u + 1), :, 32 * u:32 * u + 16],
                        ps_k[64 * u:64 * (u + 1), :NJT * M].rearrange(
                            "p (j m) -> p j m", m=M
                        ),
                        bkv[64 * u:64 * (u + 1), :][:, :, None].to_broadcast(
                            (64, NJT, M)
                        ),
                        mybir.AluOpType.add,
                    )
                memp["kTm3"] = kTm3

            def emit_vm():
                # mem-V computed feature-major in one cheap psum fill, then
                # token-major via an xbar DMA-transpose of the m-padded
                # [128, jt, 128] layout (dst[p, jt, f] = src[f, jt*128+p]);
                # replicated at partition bases 0/32/64/96 for the
                # 32-aligned mem-PV stationaries
                vm = cpool.tile([128, M, VW], F16, tag="vm", name="vm")
                nc.vector.memset(vm[:M, :, HD:HD + 1], 1.0)
                vmT = cpool.tile([128, NJT, 128], F16, tag="vmT", name="vmT")
                nc.gpsimd.memset(vmT[:], 0.0)
                ps_t = pp_pool.tile([128, 512], F32, tag="pp", name="ps_vm")
                for jt in range(NJT):
                    for d in range(NJT):
                        nc.tensor.matmul(
                            ps_t[:, jt * M:(jt + 1) * M],
                            wv(d)[:, jt * 128:(jt + 1) * 128],
                            xTm[:, d, :],
                            start=(d == 0), stop=(d == NJT - 1),
                            skip_group_check=True,
                        )
                nc.vector.tensor_tensor(
                    vmT[:, :, :M],
                    ps_t[:, :NJT * M].rearrange("p (j m) -> p j m", m=M),
                    bvv[:, :, None].to_broadcast((128, NJT, M)),
                    mybir.AluOpType.add,
                )
                vmB = cpool.tile([128, NJT, 128], F16, tag="vmB", name="vmB")
                nc.sync.dma_start_transpose(
                    vmB[:], vmT[:].rearrange("p j m -> p (j m)")
                )
                nc.vector.tensor_copy(
                    vm[:M, :, :HD].rearrange("m (j u) f -> m j u f", u=2),
                    vmB[:M, :, :].rearrange("m j (u f) -> m j u f", u=2),
                )
                for rb in range(1, 4):
                    nc.sync.dma_start(vm[32 * rb:32 * rb + M, :, :], vm[:M, :, :])
                memp["vm"] = vm

            # ---- per-chunk phases as generators; the main loop weaves
            # chunk ci's attention with chunk ci+1's projections so the PE
            # always has projection matmuls to stream while Act runs Exps.
            # The handoff is split Q/K vs V: score fills only need Q/K, so
            # they start a V-phase early and V matmuls fill attention tails.
            state_qk = {}
            state_v = {}

            def proj_steps(ci):
                xT = x_tiles.pop(ci)
                qT, kT = [], []
                for which, wfn, bias, lst, tg in (
                    (0, wq, bqv, qT, "qT"),
                    (1, wk, bkv, kT, "kT"),
                ):
                    # pp-buf pairs of jt-pair fills, split at the 1MB
                    # weight-DMA boundary (d 0-3 | 4-7) so chunk 0's
                    # matmuls track weight arrival
                    for hw in range(2):
                        pss = []
                        for j2 in range(2):
                            jp = 2 * hw + j2
                            ps_q = pp_pool.tile(
                                [128, 512], F32, tag="pp", name="ps_q"
                            )
                            for u in range(2):
                                jt = 2 * jp + u
                                for d in range(NJT // 2):
                                    nc.tensor.matmul(
                                        ps_q[:, u * 256:(u + 1) * 256],
                                        wfn(d)[:, jt * 128:(jt + 1) * 128],
                                        xT[d][:],
                                        start=(u == 0 and d == 0),
                                        stop=False,
                                    )
                            pss.append(ps_q)
                        for j2 in range(2):
                            jp = 2 * hw + j2
                            ps_q = pss[j2]
                            for u in range(2):
                                jt = 2 * jp + u
                                for d in range(NJT // 2, NJT):
                                    nc.tensor.matmul(
                                        ps_q[:, u * 256:(u + 1) * 256],
                                        wfn(d)[:, jt * 128:(jt + 1) * 128],
                                        xT[d][:],
                                        start=False,
                                        stop=(u == 1 and d == NJT - 1),
                                    )
                            pair_t = qkpool.tile(
                                [128, 2, W], F16, tag=tg, name=f"pair_{tg}"
                            )
                            nc.vector.tensor_tensor(
                                pair_t[:],
                                ps_q[:].rearrange("p (u t) -> p u t", u=2),
                                bias[:, 2 * jp:2 * jp + 2][:, :, None]
                                .to_broadcast((128, 2, W)),
                                mybir.AluOpType.add,
                            )
                            lst.append(pair_t)
                            yield
                    if ci == 0 and which == 1:
                        emit_ktm()
                        yield
                state_qk[ci] = (
                    [qT[jt // 2][:, jt % 2, :] for jt in range(NJT)],
                    [kT[jt // 2][:, jt % 2, :] for jt in range(NJT)],
                )

                # V projection (token-major fp16, heads + ones col)
                v_sb = vpool.tile([128, 2, H, VW], F16, tag="v_sb")
                nc.gpsimd.memset(v_sb[:, :, :, HD:HD + 1], 1.0)
                for tt in range(2):
                    pss = []
                    for half in range(2):
                        ps_v = pp_pool.tile([128, 512], F32, tag="pp")
                        for d in range(NJT // 2):
                            nc.tensor.matmul(
                                ps_v[:], xT[d][:, tt * 128:(tt + 1) * 128],
                                wv(d)[:, half * 512:(half + 1) * 512],
                                start=(d == 0), stop=False,
                            )
                        pss.append(ps_v)
                    for half in range(2):
                        ps_v = pss[half]
                        for d in range(NJT // 2, NJT):
                            nc.tensor.matmul(
                                ps_v[:], xT[d][:, tt * 128:(tt + 1) * 128],
                                wv(d)[:, half * 512:(half + 1) * 512],
                                start=False, stop=(d == NJT - 1),
                            )
                        nc.vector.tensor_tensor(
                            v_sb[:, tt, half * 8:(half + 1) * 8, :HD],
                            ps_v[:].rearrange("p (h f) -> p h f", h=8),
                            bvb[:, half * 512:(half + 1) * 512].rearrange(
                                "p (h f) -> p h f", h=8
                            ),
                            mybir.AluOpType.add,
                        )
                        yield
                    if ci == 0 and tt == 0:
                        emit_vm()
                state_v[ci] = v_sb

            def attn_steps(ci):
                while ci not in state_qk:
                    yield
                qT, kT = state_qk.pop(ci)
                kTm3 = memp["kTm3"]

                # mem scores: all 16 heads in one 2-bank psum, one Exp.
                # Head pair jt lands at 32-aligned partition bases via the
                # block-diagonal stationary and tile_position cols.
                def emit_ms():
                    ps_m = ps_pool.tile([128, 1024], F32, tag="ps", name="ps_ms")
                    for jt in range(NJT):
                        c0 = 64 * (jt % 2)
                        g = jt // 2
                        nc.tensor.matmul(
                            ps_m[c0:c0 + 64, g * 256:(g + 1) * 256],
                            kTm3[:, jt, :],
                            qT[jt][:],
                            start=True, stop=True,
                            tile_position=(0, c0),
                            skip_group_check=True,
                        )
                    em = empool.tile([128, 4, 256], F16, tag="em", name="em")
                    nc.scalar.activation(
                        em[:], ps_m[:].rearrange("p (g t) -> p g t", g=4),
                        mybir.ActivationFunctionType.Exp,
                        bias=mskv[:, ci * 3 + 2: ci * 3 + 3],
                    )
                    return em

                out_sb = opool.tile([128, 2, D], F16, tag="out_sb", name="out_sb")
                eloc = {}   # (hpq, yt) -> [128, 1024] fp16

                def fill(hpq, yt):
                    # local scoresT for 4 heads (one quad, one key half);
                    # both 64-row ab halves quadrant-packed; one Exp
                    ps_s = ps_pool.tile([128, 1024], F32, tag="ps", name="ps_s")
                    for ab in range(2):
                        p0 = 64 * ab
                        for u in range(2):
                            jt = 2 * hpq + u
                            nc.tensor.matmul(
                                ps_s[:, ab * 512 + u * 256:
                                     ab * 512 + (u + 1) * 256],
                                kT[jt][p0:p0 + 64, yt * 128:(yt + 1) * 128],
                                qT[jt][p0:p0 + 64, :],
                                start=(u == 0), stop=(u == 1),
                                tile_position=(p0, 0),
                                skip_group_check=True,
                            )
                    e_t = epool.tile([128, 1024], F16, tag="exps")
                    nc.scalar.activation(
                        e_t[:], ps_s[:],
                        mybir.ActivationFunctionType.Exp,
                        bias=mskv[:, ci * 3 + yt: ci * 3 + yt + 1],
                    )
                    eloc[(hpq, yt)] = e_t

                def unit(hp):
                    # PV for head pair hp: one psum bank, 4 blocks (h, xb)
                    # of 65 (64 hd + denom); strided recip + broadcast mult
                    ps_o = po_pool.tile([128, 4 * VW], F32, tag="po")
                    for ab in range(2):
                        h = 2 * hp + ab
                        e0 = eloc[(hp // 2, 0)]
                        e1 = eloc[(hp // 2, 1)]
                        base, g = _mem_slot(h)
                        for xb in range(2):
                            o = (ab * 2 + xb) * VW
                            xs = (h % 2) * 512 + (hp % 2) * 256 + xb * 128
                            nc.tensor.matmul(
                                ps_o[:, o:o + VW],
                                e0[:, xs:xs + 128], v_sb[:, 0, h, :],
                                start=True, stop=False,
                            )
                            nc.tensor.matmul(
                                ps_o[:, o:o + VW],
                                e1[:, xs:xs + 128], v_sb[:, 1, h, :],
                                start=False, stop=False,
                            )
                            nc.tensor.matmul(
                                ps_o[:, o:o + VW],
                                em[base:base + M, g, xb * 128:(xb + 1) * 128],
                                vm[base:base + M, h, :],
                                start=False, stop=True,
                                tile_position=(base, 0),
                            )
                    rec = rpool.tile([128, 4], F32, tag="rec", name="rec")
                    nc.vector.reciprocal(
                        rec[:].rearrange("p (k o) -> p k o", o=1),
                        ps_o[:].rearrange("p (k w) -> p k w", w=VW)[
                            :, :, HD:HD + 1
                        ],
                    )
                    nc.vector.tensor_tensor(
                        out_sb[:, :, 2 * hp * HD:(2 * hp + 2) * HD].rearrange(
                            "p x (a f) -> p a x f", a=2
                        ),
                        ps_o[:].rearrange("p (a x w) -> p a x w", a=2, x=2)[
                            :, :, :, :HD
                        ],
                        rec[:].rearrange("p (a x) -> p a x", a=2)[
                            :, :, :, None
                        ].to_broadcast((128, 2, 2, HD)),
                        mybir.AluOpType.mult,
                    )

                def out_dma(qtr):
                    nc.sync.dma_start(
                        out_d.rearrange("(x p) c -> p x c", p=128)[
                            :, 2 * ci:2 * ci + 2, qtr * 256:(qtr + 1) * 256
                        ],
                        out_sb[:, :, qtr * 256:(qtr + 1) * 256],
                    )

                # fills run two head-pairs ahead of PV units so the Exp
                # latency hides behind interleaved projection matmuls
                fill(0, 0); yield
                fill(0, 1); yield
                em = emit_ms(); yield
                fill(1, 0); yield
                fill(1, 1); yield
                while ci not in state_v:
                    yield
                v_sb = state_v.pop(ci)
                vm = memp["vm"]
                unit(0); yield
                unit(1); out_dma(0); yield
                fill(2, 0); yield
                fill(2, 1); yield
                unit(2); yield
                unit(3); out_dma(1); yield
                fill(3, 0); yield
                fill(3, 1); yield
                unit(4); yield
                unit(5); out_dma(2); yield
                unit(6); yield
                unit(7); out_dma(3)

            def drain(*gens):
                gens = [g for g in gens if g is not None]
                while gens:
                    nxt = []
                    for g in gens:
                        try:
                            next(g)
                            nxt.append(g)
                        except StopIteration:
                            pass
                    gens = nxt

            # 3-way weave: chunk ci's attention runs with chunk ci+1's
            # projections, and attn(ci+1) joins early (it self-waits on
            # its state) so the attention tail always has matmul filler
            attns_g = [attn_steps(ci) for ci in range(CPC)]
            p0 = proj_steps(0)
            gens0 = [p0, attns_g[0]]
            must0 = {id(p0)}
            while must0:
                for g in list(gens0):
                    try:
                        next(g)
                    except StopIteration:
                        gens0.remove(g)
                        must0.discard(id(g))
            for ci in range(CPC):
                gens = [attns_g[ci]]
                must = {id(attns_g[ci])}
                if ci + 1 < CPC:
                    pj = proj_steps(ci + 1)
                    gens = [pj, attns_g[ci], attns_g[ci + 1]]
                    must.add(id(pj))
                while must:
                    for g in list(gens):
                        try:
                            next(g)
                        except StopIteration:
                            gens.remove(g)
                            must.discard(id(g))

    nc.compile()
    return nc


_NC_CACHE = None


def kernel(hidden_states, attention_mask, self_memory, Wq, bq, Wk, bk, Wv, bv):
    global _NC_CACHE, LAST_RESULTS
    hidden_states = np.asarray(np.asarray(hidden_states), np.float32)
    attention_mask = np.asarray(np.asarray(attention_mask), np.float32)
    self_memory = np.asarray(np.asarray(self_memory), np.float32)
    wqT = np.ascontiguousarray(
        (np.asarray(Wq, np.float32).T * 0.125).astype(np.float16)
    )
    wkT = np.ascontiguousarray(np.asarray(Wk, np.float32).T.astype(np.float16))
    wvT = np.ascontiguousarray(np.asarray(Wv, np.float32).T.astype(np.float16))
    bqv = np.ascontiguousarray(
        np.asarray(bq, np.float32).reshape(NJT, 128).T * 0.125
    )
    bkv = np.ascontiguousarray(np.asarray(bk, np.float32).reshape(NJT, 128).T)
    bvrow = np.asarray(bv, np.float32).astype(np.float16).reshape(1, D)
    bvv = np.ascontiguousarray(np.asarray(bv, np.float32).reshape(NJT, 128).T)

    # additive mask along the key axis, per (b, c): [yt0 | yt1 | memory]
    am = attention_mask.reshape(B, C, W)
    chunk_has_valid = (am == 0.0).sum(axis=2) > 0
    mem_mask = np.where(chunk_has_valid, 0.0, -10000.0).astype(np.float32)

    if _NC_CACHE is None:
        _NC_CACHE = _build_kernel()
    nc = _NC_CACHE

    x16 = hidden_states.astype(np.float16)
    mem16 = self_memory.astype(np.float16)

    in_maps = []
    for core in range(N_CORES):
        b = core // (N_CORES // B)
        c0 = (core % (N_CORES // B)) * CPC
        mvT = np.zeros((128, CPC * 3), np.float32)
        for ci in range(CPC):
            mvT[:, ci * 3 + 0] = am[b, c0 + ci, 0:128]
            mvT[:, ci * 3 + 1] = am[b, c0 + ci, 128:256]
            mvT[:, ci * 3 + 2] = mem_mask[b, c0 + ci]
        # feature-major pre-transposed x: [128, NJT, TPC]
        xT = np.ascontiguousarray(
            x16[b, c0 * W:(c0 + CPC) * W, :]
            .T.reshape(NJT, 128, TPC).transpose(1, 0, 2)
        ).reshape(128, NJT * TPC)
        memT = np.ascontiguousarray(
            mem16[b].T.reshape(NJT, 128, M).transpose(1, 0, 2)
        ).reshape(128, NJT * M)
        in_maps.append(
            {
                "xT": xT,
                "memT": memT,
                "wqT": wqT,
                "wkT": wkT,
                "wvT": wvT,
                "bqv": bqv,
                "bkv": bkv,
                "bvrow": bvrow,
                "bvv": bvv,
                "maskvT": mvT,
            }
        )

    res = run_bass_kernel_spmd(nc, in_maps, list(range(N_CORES)), trace=TRACE)
    LAST_RESULTS = res

    out = np.empty((B, S, D), np.float32)
    for core in range(N_CORES):
        b = core // (N_CORES // B)
        c0 = (core % (N_CORES // B)) * CPC
        out[b, c0 * W:(c0 + CPC) * W, :] = res.results[core]["out"].astype(
            np.float32
        )
    return out
